# revision 13
# baseline (speedup 1.0000x reference)
"""BLT local encoder (2-layer transformer, patch-equality block-diagonal attention)
on 8 Trainium2 NeuronCores.

v2. Sharding: each of the 4 sequences splits at a patch-run boundary nearest
S/2 -> 8 independent shards, one per core, zero cross-core communication.

Kernel design (per core, L_tok = max shard length ~1032):
- Residual hT kept float32 feature-major [P, 8dc x PTL]; everything else bf16.
- Weights prepacked host-side into SBUF-ready bf16 col/row blocks, streamed
  once per layer (no restreaming), double-buffered.
- One LayerNorm per sublayer, output xh bf16 reused by Q, K and V.
- Full-shard attention: per (head, key-tile j) one score matmul with moving
  dim >= 256; softmax denominator via a ones-column appended to V (row 64 of
  the ctx psum); per-head normalize fused into the psum->SBUF copy.
- Engine split: PE matmuls; DVE normalize/copies/masks; Act square/exp/gelu;
  Pool partition-broadcasts + residual adds.
"""

import numpy as np

import concourse.bass as bass
import concourse.tile as tile
from concourse import bacc, bass_utils, mybir

F32 = mybir.dt.float32
F32R = mybir.dt.float32r
BF16 = mybir.dt.bfloat16
AF = mybir.ActivationFunctionType
OP = mybir.AluOpType

B, S, D, H, F, L = 4, 2048, 1024, 16, 4096, 2
DH = D // H      # 64
DC = D // 128    # 8
FC = F // 128    # 32
EPS = 1e-5
SCALE = 1.0 / np.sqrt(DH)
P = 128
VP = 384         # vocab 260 padded
VC = VP // 128   # 3
NCORES = 8


def _chunks(lt):
    out = []
    o = 0
    while o < lt:
        c = min(512, lt - o)
        out.append((o, c))
        o += c
    return out


def _build(lt, nt, use_lng):
    """lt: tokens per shard; nt: token tiles; use_lng: emit ln gamma/beta ops."""
    ptl = nt * P
    chs = _chunks(lt)
    nc = bacc.Bacc("TRN2", target_bir_lowering=False, debug=False,
                   num_devices=NCORES)

    def din(name, shape, dt=BF16):
        return nc.dram_tensor(name, shape, dt, kind="ExternalInput").ap()

    oht = din("oht", [P, VC * ptl])
    tokemb_d = din("tokemb", [P, VC * D])
    baseT = din("baseT", [P, DC * ptl], F32R)
    masks_d = din("masks", [P, nt * 384])
    # prepacked weights
    kcb_d, qcb_d, ocb_d, vrb_d, w1cb_d, w2cb_d = [], [], [], [], [], []
    for l in range(L):
        kcb_d.append(din(f"kcb{l}", [P, DC * DC * 128]))
        qcb_d.append(din(f"qcb{l}", [P, DC * DC * 128]))
        ocb_d.append(din(f"ocb{l}", [P, DC * DC * 128]))
        vrb_d.append(din(f"vrb{l}", [P, DC * D]))
        w1cb_d.append(din(f"w1cb{l}", [P, 8 * DC * 512]))
        w2cb_d.append(din(f"w2cb{l}", [P, DC * FC * 128]))
    # packed per-feature consts: [P, col] layout, 8 cols per D-vector
    # cols: 0 ones | 1 eps(row0) | then per layer l at 2+64*l:
    #   bq 0:8 bk 8:16 bv 16:24 bo 24:32 b2 32:40 b1 40:72 (unused gap)
    # ln g/b (if use_lng): separate tensor lngb
    cb_d = din("cb", [P, 2 + 96 * L], F32)
    lngb_d = din("lngb", [P, 8 * (2 + 4 * L)], F32) if use_lng else None
    houtT = nc.dram_tensor("houtT", [P, DC * ptl], F32R,
                           kind="ExternalOutput").ap()

    with tile.TileContext(nc) as tc:
        with (
            nc.allow_low_precision(
                reason="bf16 softmax/LN staging validated vs reference"),
            tc.tile_pool(name="pers", bufs=1) as pers,
            tc.tile_pool(name="big", bufs=4) as big,
            tc.tile_pool(name="xhp", bufs=1) as xhp,
            tc.tile_pool(name="wcb", bufs=3) as wcb,
            tc.tile_pool(name="est", bufs=2) as estp,
            tc.tile_pool(name="lnt", bufs=3) as lnp,
            tc.tile_pool(name="sm", bufs=2) as smp,
            tc.tile_pool(name="dv", bufs=2) as dvp,
            tc.tile_pool(name="pp", bufs=8, space="PSUM") as pp,
        ):
            cb = pers.tile([P, 2 + 96 * L], F32, tag="cb")
            nc.sync.dma_start(out=cb, in_=cb_d)
            eps_t = cb[0:1, 1:2]
            ones_r = pers.tile([P, 1], F32R, tag="ones_r")
            nc.vector.tensor_copy(ones_r, cb[:, 0:1])
            ones_b = pers.tile([P, 1], BF16, tag="ones_b")
            nc.vector.tensor_copy(ones_b, cb[:, 0:1])
            if use_lng:
                lngb = pers.tile([P, 8 * (2 + 4 * L)], F32, tag="lngb")
                nc.sync.dma_start(out=lngb, in_=lngb_d)

            masks = pers.tile([P, nt * 384], BF16, tag="masks")
            nc.sync.dma_start(out=masks, in_=masks_d)

            hT = pers.tile([P, DC * ptl], F32R, tag="hT")

            def bcol(l, i):  # bias col i (in 8-col groups) for layer l
                c0 = 2 + 96 * l + 8 * i
                return cb[:, c0:c0 + 8]

            def ln_pass(gi, out_tile, out_dtype_is_h):
                """LayerNorm hT over features -> out_tile (stride ptl).
                gi: index into lngb groups (g at 8*(2*gi), b at +8) or None
                handling via use_lng; out_dtype_is_h: write back into hT."""
                for (t0, cl) in chs:
                    ps1 = pp.tile([1, 512], F32, tag="mm", name="lns1")
                    ps2 = pp.tile([1, 512], F32, tag="mm", name="lns2")
                    for dc in range(DC):
                        hsl = hT[:, dc * ptl + t0:dc * ptl + t0 + cl]
                        sq = lnp.tile([P, 512], BF16, tag="sq", name=f"sq{dc}")
                        nc.scalar.activation(sq[:, 0:cl], hsl, AF.Square)
                        nc.tensor.matmul(ps1[:, 0:cl], lhsT=ones_r, rhs=hsl,
                                         start=(dc == 0), stop=(dc == DC - 1))
                        nc.tensor.matmul(ps2[:, 0:cl], lhsT=ones_b,
                                         rhs=sq[:, 0:cl],
                                         start=(dc == 0), stop=(dc == DC - 1))
                    st = smp.tile([P, 4 * 512], F32, tag="st", name="st")
                    mean = st[0:1, 0:cl]
                    var = st[0:1, 512:512 + cl]
                    rstd = st[0:1, 1024:1024 + cl]
                    mr = st[0:1, 1536:1536 + cl]
                    nc.vector.tensor_scalar_mul(mean, ps1[:, 0:cl], 1.0 / D)
                    nc.vector.tensor_mul(var, mean, mean)
                    nc.vector.scalar_tensor_tensor(
                        var, ps2[:, 0:cl], 1.0 / D, var,
                        op0=OP.mult, op1=OP.subtract)
                    nc.scalar.activation(rstd, var, AF.Sqrt, bias=eps_t)
                    nc.vector.reciprocal(rstd, rstd)
                    nc.vector.tensor_mul(mr, mean, rstd)
                    stb = smp.tile([P, 2 * 512], BF16, tag="stb", name="stb")
                    nc.gpsimd.tensor_copy(stb[0:1, :], st[0:1, 1024:2048])
                    RM = dvp.tile([P, 2 * 512], BF16, tag="rm", name="RM")
                    nc.gpsimd.partition_broadcast(RM[:, 0:cl], stb[0:1, 0:cl])
                    nc.gpsimd.partition_broadcast(RM[:, 512:512 + cl],
                                                  stb[0:1, 512:512 + cl])
                    for dc in range(DC):
                        hsl = hT[:, dc * ptl + t0:dc * ptl + t0 + cl]
                        d1 = lnp.tile([P, 512], BF16, tag="d1", name=f"d1_{dc}")
                        nc.vector.tensor_mul(d1[:, 0:cl], hsl, RM[:, 0:cl])
                        osl = out_tile[:, dc * ptl + t0:dc * ptl + t0 + cl]
                        if use_lng and gi is not None:
                            d2 = lnp.tile([P, 512], BF16, tag="d2",
                                          name=f"d2_{dc}")
                            nc.vector.tensor_sub(d2[:, 0:cl], d1[:, 0:cl],
                                                 RM[:, 512:512 + cl])
                            g0 = 8 * (2 * gi)
                            nc.vector.tensor_scalar(
                                osl, d2[:, 0:cl],
                                lngb[:, g0 + dc:g0 + dc + 1],
                                lngb[:, g0 + 8 + dc:g0 + 8 + dc + 1],
                                op0=OP.mult, op1=OP.add)
                        else:
                            nc.vector.tensor_sub(osl, d1[:, 0:cl],
                                                 RM[:, 512:512 + cl])

            # ---------- embeddings ----------
            ohsb = wcb.tile([P, VC * ptl], BF16, tag="w", name="ohsb")
            nc.sync.dma_start(out=ohsb, in_=oht)
            tesb = wcb.tile([P, VC * D], BF16, tag="w", name="tesb")
            nc.sync.dma_start(out=tesb, in_=tokemb_d)
            for dc in range(DC):
                nc.sync.dma_start(out=hT[:, dc * ptl:(dc + 1) * ptl],
                                  in_=baseT[:, dc * ptl:(dc + 1) * ptl])
            for dc in range(DC):
                for (t0, cl) in chs:
                    pse = pp.tile([P, 512], F32, tag="mm", name="pse")
                    for vc in range(VC):
                        nc.tensor.matmul(
                            pse[:, 0:cl],
                            lhsT=tesb[:, vc * D + dc * 128:vc * D + dc * 128 + 128],
                            rhs=ohsb[:, vc * ptl + t0:vc * ptl + t0 + cl],
                            start=(vc == 0), stop=(vc == VC - 1))
                    hsl = hT[:, dc * ptl + t0:dc * ptl + t0 + cl]
                    nc.vector.tensor_add(hsl, pse[:, 0:cl], hsl)
            ln_pass(None, hT, True)   # LN0 in place (g/b via lngb group 0...)

            # ---------- layers ----------
            for l in range(L):
                xh = xhp.tile([P, DC * ptl], BF16, tag="xh", name=f"xh{l}a")
                ln_pass(2 * l if use_lng else None, xh, False)

                # ---- K/Q/V + attention, interleaved ----
                KT = big.tile([P, DC * ptl], BF16, tag="b18", name=f"KT{l}")
                Vsb = big.tile([P, nt * H * 65], BF16, tag="b18", name=f"Vsb{l}")
                QT = big.tile([P, DC * ptl], BF16, tag="b18", name=f"QT{l}")
                ctxc = big.tile([P, DC * ptl], BF16, tag="b18", name=f"ctx{l}")
                if lt < ptl:
                    nc.vector.memset(
                        Vsb[:, (nt - 1) * H * 65:nt * H * 65], 0.0)
                ones_v = Vsb.rearrange("p (g x) -> p g x", x=65)[:, :, 64:65]
                nc.vector.memset(ones_v, 1.0)

                def v_half(nh):
                    ntg = (nt + 3) // 4
                    for tg in range(ntg):
                        tts = [t for t in range(4 * tg, min(4 * tg + 4, nt))
                               if lt - t * P > 0]
                        pvs = {}
                        for tt in tts:
                            pvs[tt] = pp.tile([P, 512], F32, tag="mm",
                                              name=f"psv{tt}_{nh}")
                        for dc in range(DC):
                            vrb = wcb.tile([P, 512], BF16, tag="w",
                                           name=f"vrb{nh}_{tg}_{dc}")
                            nc.sync.dma_start(
                                out=vrb,
                                in_=vrb_d[l][:, (nh * DC + dc) * 512:
                                             (nh * DC + dc + 1) * 512])
                            for tt in tts:
                                tl = min(P, lt - tt * P)
                                nc.tensor.matmul(
                                    pvs[tt][0:tl, :],
                                    lhsT=xh[:, dc * ptl + tt * P:dc * ptl + tt * P + tl],
                                    rhs=vrb,
                                    start=(dc == 0), stop=(dc == DC - 1))
                        for tt in tts:
                            tl = min(P, lt - tt * P)
                            pv = pvs[tt][0:tl, :].rearrange(
                                "p (h x) -> p h x", h=8)
                            ov = Vsb[0:tl, (tt * H + nh * 8) * 65:
                                     (tt * H + nh * 8 + 8) * 65].rearrange(
                                "p (h x) -> p h x", x=65)[:, :, 0:64]
                            nc.vector.tensor_copy(ov, pv)

                def kq_block(oc):
                    kcb = wcb.tile([P, DC * 128], BF16, tag="w",
                                   name=f"kcb{oc}")
                    nc.sync.dma_start(
                        out=kcb, in_=kcb_d[l][:, oc * D:(oc + 1) * D])
                    for (t0, cl) in chs:
                        ps = pp.tile([P, 512], F32, tag="mm", name=f"psk{oc}")
                        for dc in range(DC):
                            nc.tensor.matmul(
                                ps[:, 0:cl],
                                lhsT=kcb[:, dc * 128:dc * 128 + 128],
                                rhs=xh[:, dc * ptl + t0:dc * ptl + t0 + cl],
                                start=(dc == 0), stop=(dc == DC - 1))
                        nc.vector.tensor_scalar_add(
                            KT[:, oc * ptl + t0:oc * ptl + t0 + cl],
                            ps[:, 0:cl], bcol(l, 1)[:, oc:oc + 1])
                    qcb = wcb.tile([P, DC * 128], BF16, tag="w",
                                   name=f"qcb{oc}")
                    nc.sync.dma_start(
                        out=qcb, in_=qcb_d[l][:, oc * D:(oc + 1) * D])
                    for (t0, cl) in chs:
                        ps = pp.tile([P, 512], F32, tag="mm", name=f"psq{oc}")
                        for dc in range(DC):
                            nc.tensor.matmul(
                                ps[:, 0:cl],
                                lhsT=qcb[:, dc * 128:dc * 128 + 128],
                                rhs=xh[:, dc * ptl + t0:dc * ptl + t0 + cl],
                                start=(dc == 0), stop=(dc == DC - 1))
                        nc.vector.tensor_scalar_add(
                            QT[:, oc * ptl + t0:oc * ptl + t0 + cl],
                            ps[:, 0:cl], bcol(l, 0)[:, oc:oc + 1])
                    if lt < ptl:
                        nc.vector.memset(KT[:, oc * ptl + lt:(oc + 1) * ptl],
                                         0.0)
                        nc.vector.memset(QT[:, oc * ptl + lt:(oc + 1) * ptl],
                                         0.0)

                def head(h):
                    dch, po = h // 2, (h % 2) * 64
                    est = estp.tile([P, nt * 384], BF16, tag="est",
                                    name=f"est{h}")
                    for j in range(nt):
                        lo = max(j - 1, 0)
                        hi = min(j + 1, nt - 1)
                        nq = (hi - lo + 1) * P
                        w0 = min(max(j - 1, 0), nt - 3)
                        pst = pp.tile([P, 384], F32, tag="mm", name=f"pst{j}")
                        nc.tensor.matmul(
                            pst[:, 0:nq],
                            lhsT=KT[po:po + 64, dch * ptl + j * P:dch * ptl + j * P + P],
                            rhs=QT[po:po + 64, dch * ptl + lo * P:dch * ptl + lo * P + nq],
                            start=True, stop=True)
                        esl = est[:, j * 384 + (lo - w0) * P:
                                  j * 384 + (lo - w0) * P + nq]
                        nc.scalar.activation(esl, pst[:, 0:nq], AF.Exp,
                                             scale=float(SCALE))
                    nc.vector.tensor_mul(est, est, masks)
                    for qg in range((nt + 3) // 4):
                        qts = [q for q in range(4 * qg, min(4 * qg + 4, nt))]
                        gw = len(qts) * P
                        psc = pp.tile([65, 512], F32, tag="mm", name=f"psc{qg}")
                        for qi, qt in enumerate(qts):
                            js = [j for j in (qt - 1, qt, qt + 1)
                                  if 0 <= j < nt]
                            for kk, j in enumerate(js):
                                w0 = min(max(j - 1, 0), nt - 3)
                                rsl = est[:, j * 384 + (qt - w0) * P:
                                          j * 384 + (qt - w0) * P + P]
                                nc.tensor.matmul(
                                    psc[:, qi * P:(qi + 1) * P],
                                    lhsT=Vsb[:, (j * H + h) * 65:
                                             (j * H + h) * 65 + 65],
                                    rhs=rsl,
                                    start=(kk == 0), stop=(kk == len(js) - 1))
                        dinv = dvp.tile([1, 512], BF16, tag="dinv",
                                        name=f"dinv{qg}")
                        nc.vector.reciprocal(dinv[:, 0:gw], psc[64:65, 0:gw])
                        dnb = dvp.tile([P, 512], BF16, tag="dnb",
                                       name=f"dnb{qg}")
                        nc.gpsimd.partition_broadcast(dnb[0:64, 0:gw],
                                                      dinv[:, 0:gw])
                        nc.vector.tensor_mul(
                            ctxc[po:po + 64,
                                 dch * ptl + qg * 512:dch * ptl + qg * 512 + gw],
                            psc[0:64, 0:gw], dnb[0:64, 0:gw])

                v_half(0)
                kq_block(0)
                for oc in range(1, DC):
                    if oc == 5:
                        v_half(1)
                    head(2 * oc - 2)
                    kq_block(oc)
                    head(2 * oc - 1)
                head(14)
                head(15)

                # ---- O-projection + residual ----
                for do_ in range(DC):
                    ocb = wcb.tile([P, DC * 128], BF16, tag="w", name=f"ocb{do_}")
                    nc.sync.dma_start(
                        out=ocb, in_=ocb_d[l][:, do_ * D:(do_ + 1) * D])
                    for (t0, cl) in chs:
                        ps = pp.tile([P, 512], F32, tag="mm", name=f"pso{do_}")
                        for dc in range(DC):
                            nc.tensor.matmul(
                                ps[:, 0:cl], lhsT=ocb[:, dc * 128:dc * 128 + 128],
                                rhs=ctxc[:, dc * ptl + t0:dc * ptl + t0 + cl],
                                start=(dc == 0), stop=(dc == DC - 1))
                        hsl = hT[:, do_ * ptl + t0:do_ * ptl + t0 + cl]
                        nc.vector.scalar_tensor_tensor(
                            hsl, ps[:, 0:cl], bcol(l, 3)[:, do_:do_ + 1], hsl,
                            op0=OP.add, op1=OP.add)

                # ---- FFN ----
                xh = xhp.tile([P, DC * ptl], BF16, tag="xh", name=f"xh{l}b")
                ln_pass(2 * l + 1 if use_lng else None, xh, False)
                Us = [big.tile([P, 8 * ptl], BF16, tag="b18", name=f"U{l}_{i}")
                      for i in range(4)]

                def usl(fc, t0, cl):
                    t = Us[fc // 8]
                    k = fc % 8
                    return t[:, k * ptl + t0:k * ptl + t0 + cl]

                for fcb in range(8):
                    w1cb = wcb.tile([P, DC * 512], BF16, tag="w",
                                    name=f"w1cb{fcb}")
                    nc.sync.dma_start(
                        out=w1cb,
                        in_=w1cb_d[l][:, fcb * DC * 512:(fcb + 1) * DC * 512])
                    for fc2 in range(4):
                        fc = fcb * 4 + fc2
                        for (t0, cl) in chs:
                            ps = pp.tile([P, 512], F32, tag="mm",
                                         name=f"psf{fc2}")
                            for dc in range(DC):
                                nc.tensor.matmul(
                                    ps[:, 0:cl],
                                    lhsT=w1cb[:, dc * 512 + fc2 * 128:
                                              dc * 512 + fc2 * 128 + 128],
                                    rhs=xh[:, dc * ptl + t0:dc * ptl + t0 + cl],
                                    start=(dc == 0), stop=(dc == DC - 1))
                            bidx = 5 + fc // 8
                            nc.scalar.activation(
                                usl(fc, t0, cl), ps[:, 0:cl], AF.Gelu,
                                bias=bcol(l, bidx)[:, fc % 8:fc % 8 + 1])
                for do_ in range(DC):
                    w2cb = wcb.tile([P, FC * 128], BF16, tag="w",
                                    name=f"w2cb{do_}")
                    nc.sync.dma_start(
                        out=w2cb,
                        in_=w2cb_d[l][:, do_ * FC * 128:(do_ + 1) * FC * 128])
                    for (t0, cl) in chs:
                        ps = pp.tile([P, 512], F32, tag="mm", name=f"psh{do_}")
                        for fc in range(FC):
                            nc.tensor.matmul(
                                ps[:, 0:cl],
                                lhsT=w2cb[:, fc * 128:fc * 128 + 128],
                                rhs=usl(fc, t0, cl),
                                start=(fc == 0), stop=(fc == FC - 1))
                        hsl = hT[:, do_ * ptl + t0:do_ * ptl + t0 + cl]
                        nc.vector.scalar_tensor_tensor(
                            hsl, ps[:, 0:cl], bcol(l, 4)[:, do_:do_ + 1], hsl,
                            op0=OP.add, op1=OP.add)
                    if l == L - 1:
                        nc.sync.dma_start(
                            out=houtT[:, do_ * ptl:(do_ + 1) * ptl],
                            in_=hT[:, do_ * ptl:(do_ + 1) * ptl])

    nc.compile()
    return nc


_NC_CACHE = {}


def _get_nc(lt=1032, nt=9, use_lng=False):
    key = (lt, nt, use_lng)
    if key not in _NC_CACHE:
        _NC_CACHE[key] = _build(lt, nt, use_lng)
    return _NC_CACHE[key]


def _pack_shared(inputs, lt, nt, use_lng):
    bf = np.dtype("bfloat16") if hasattr(np, "bfloat16") else None
    import ml_dtypes
    BFD = ml_dtypes.bfloat16

    def b16(x):
        return np.ascontiguousarray(np.asarray(x, np.float32).astype(BFD))

    tok = np.asarray(inputs["tok_emb"], np.float32)
    tokp = np.zeros((VP, D), np.float32)
    tokp[:tok.shape[0]] = tok
    tokemb = b16(tokp.reshape(VC, P, D).transpose(1, 0, 2).reshape(P, VC * D))

    shared = {"tokemb": tokemb}
    for l in range(L):
        Wq = np.asarray(inputs["Wq"][l], np.float32)
        Wk = np.asarray(inputs["Wk"][l], np.float32)
        Wv = np.asarray(inputs["Wv"][l], np.float32)
        Wo = np.asarray(inputs["Wo"][l], np.float32)
        W1 = np.asarray(inputs["W1"][l], np.float32)
        W2 = np.asarray(inputs["W2"][l], np.float32)

        def colblocks(W, ocn):  # [D, D] -> [P, ocn*DC*128]
            # block (oc): [p, dc, c] = W[dc*128+p, oc*128+c]
            Wr = W.reshape(DC, P, ocn, 128)  # [dc, p, oc, c]
            return np.ascontiguousarray(
                Wr.transpose(1, 2, 0, 3).reshape(P, ocn * DC * 128))

        shared[f"kcb{l}"] = b16(colblocks(Wk, DC))
        shared[f"qcb{l}"] = b16(colblocks(Wq, DC))
        shared[f"ocb{l}"] = b16(colblocks(Wo, DC))
        # vrb: [p, nh, dc, c] = Wv[dc*128+p, nh*512+c]
        Wvr = Wv.reshape(DC, P, 2, 512)
        shared[f"vrb{l}"] = b16(
            Wvr.transpose(1, 2, 0, 3).reshape(P, 2 * DC * 512))
        # w1cb: [p, fcb, dc, c] = W1[dc*128+p, fcb*512+c]
        W1r = W1.reshape(DC, P, 8, 512)
        shared[f"w1cb{l}"] = b16(
            W1r.transpose(1, 2, 0, 3).reshape(P, 8 * DC * 512))
        # w2cb: [p, do, fc, c] = W2[fc*128+p, do*128+c]
        W2r = W2.reshape(FC, P, DC, 128)
        shared[f"w2cb{l}"] = b16(
            W2r.transpose(1, 2, 0, 3).reshape(P, DC * FC * 128))

    cbw = np.zeros((P, 2 + 96 * L), np.float32)
    cbw[:, 0] = 1.0
    cbw[0, 1] = EPS
    for l in range(L):
        c0 = 2 + 96 * l
        # bv is folded into bo: probs sum to 1, so ctx@Wo + bo with V+bv
        # equals (ctx from plain V)@Wo + (bo + bv@Wo).
        bo_eff = (np.asarray(inputs["bo"][l], np.float32)
                  + np.asarray(inputs["bv"][l], np.float32)
                  @ np.asarray(inputs["Wo"][l], np.float32))
        vals = {"bq": np.asarray(inputs["bq"][l], np.float32),
                "bk": np.asarray(inputs["bk"][l], np.float32),
                "bv": np.zeros(D, np.float32),
                "bo": bo_eff,
                "b2": np.asarray(inputs["b2"][l], np.float32)}
        for i, key in enumerate(("bq", "bk", "bv", "bo", "b2")):
            cbw[:, c0 + 8 * i:c0 + 8 * i + 8] = vals[key].reshape(DC, P).T
        b1v = np.asarray(inputs["b1"][l], np.float32)
        cbw[:, c0 + 40:c0 + 72] = b1v.reshape(FC, P).T
    shared["cb"] = np.ascontiguousarray(cbw)

    if use_lng:
        gb = np.zeros((P, 8 * (2 + 4 * L)), np.float32)
        # group 0: ln0 (handled as gi=None in build... keep identity)
        idx = 0
        for l in range(L):
            for which in range(2):
                gi = 2 * l + which
                g = np.asarray(inputs["ln1_g" if which == 0 else "ln2_g"][l],
                               np.float32)
                bb = np.asarray(inputs["ln1_b" if which == 0 else "ln2_b"][l],
                                np.float32)
                gb[:, 8 * (2 * gi):8 * (2 * gi) + 8] = g.reshape(DC, P).T
                gb[:, 8 * (2 * gi + 1):8 * (2 * gi + 1) + 8] = bb.reshape(DC, P).T
        shared["lngb"] = np.ascontiguousarray(gb)
    return shared


def _prep_core(inputs, b, start, n, lt, nt):
    import ml_dtypes
    BFD = ml_dtypes.bfloat16
    ptl = nt * P

    def b16(x):
        return np.ascontiguousarray(np.asarray(x, np.float32).astype(BFD))

    ids = np.asarray(inputs["input_ids"][b, start:start + n])
    pid = np.asarray(inputs["patch_ids"][b, start:start + n]).astype(np.int64)
    pos_emb = np.asarray(inputs["pos_emb"], np.float32)
    hashes = np.asarray(inputs["hash_embeddings"], np.float32)

    oh = np.zeros((VP, ptl), np.float32)
    oh[ids, np.arange(n)] = 1.0
    oht = b16(oh.reshape(VC, P, ptl).transpose(1, 0, 2).reshape(P, VC * ptl))

    base = np.zeros((ptl, D), np.float32)
    base[:n] = pos_emb[start:start + n] + hashes[b, start:start + n]
    baseT = np.ascontiguousarray(
        base.reshape(ptl, DC, P).transpose(2, 1, 0).reshape(P, DC * ptl))

    pidp = np.empty(ptl, np.int64)
    pidp[:n] = pid
    pidp[n:] = -np.arange(1, ptl - n + 1)

    m = np.zeros((nt, P, 384), np.float32)
    for j in range(nt):
        w0 = np.clip(j - 1, 0, nt - 3) * P
        kk = pidp[j * P:(j + 1) * P]
        qq = pidp[w0:w0 + 384]
        m[j] = (kk[:, None] == qq[None, :]).astype(np.float32)
    masks = b16(m.transpose(1, 0, 2).reshape(P, nt * 384))
    return {"oht": oht, "baseT": baseT, "masks": masks}


def kernel(**inputs):
    pid_all = np.asarray(inputs["patch_ids"])

    shards = []
    for b in range(B):
        pid = np.asarray(pid_all[b])
        bnd = np.nonzero(pid[1:] != pid[:-1])[0] + 1
        cand = bnd[(bnd >= S - 1152) & (bnd <= 1152)]
        if len(cand) == 0:
            raise RuntimeError("no patch boundary near S/2; cannot shard")
        s = int(cand[np.argmin(np.abs(cand - S // 2))])
        shards.append((b, 0, s))
        shards.append((b, s, S - s))

    lt = max(n for _, _, n in shards)
    lt = max(lt, 1026)  # floor so chunk 3 isn't degenerate-tiny
    nt = (lt + P - 1) // P

    use_lng = not (
        all(np.all(np.asarray(inputs[k]) == 1.0)
            for k in ("ln0_g", "ln1_g", "ln2_g")) and
        all(np.all(np.asarray(inputs[k]) == 0.0)
            for k in ("ln0_b", "ln1_b", "ln2_b")))
    if use_lng:
        raise NotImplementedError(
            "non-identity LN affine not supported in fast path")

    shared = _pack_shared(inputs, lt, nt, use_lng)
    in_maps = []
    for b, start, n in shards:
        mcore = dict(shared)
        mcore.update(_prep_core(inputs, b, start, n, lt, nt))
        in_maps.append(mcore)

    nc = _get_nc(lt, nt, use_lng)
    res = bass_utils.run_bass_kernel_spmd(nc, in_maps,
                                          core_ids=list(range(NCORES)))

    ptl = nt * P
    out = np.zeros((B, S, D), np.float32)
    for i, (b, start, n) in enumerate(shards):
        ht = res.results[i]["houtT"]
        hfull = ht.reshape(P, DC, ptl).transpose(2, 1, 0).reshape(ptl, D)
        out[b, start:start + n] = hfull[:n]
    return out


if __name__ == "__main__":
    import sys
    lt = int(sys.argv[1]) if len(sys.argv) > 1 else 1032
    _get_nc(lt, (lt + P - 1) // P, False)
    print("built ok")


# revision 14
# speedup vs baseline: 1.2031x; 1.2031x over previous
"""BLT local encoder (2-layer transformer, patch-equality block-diagonal attention)
on 8 Trainium2 NeuronCores.

v2. Sharding: each of the 4 sequences splits at a patch-run boundary nearest
S/2 -> 8 independent shards, one per core, zero cross-core communication.

Kernel design (per core, L_tok = max shard length ~1032):
- Residual hT kept float32 feature-major [P, 8dc x PTL]; everything else bf16.
- Weights prepacked host-side into SBUF-ready bf16 col/row blocks, streamed
  once per layer (no restreaming), double-buffered.
- One LayerNorm per sublayer, output xh bf16 reused by Q, K and V.
- Full-shard attention: per (head, key-tile j) one score matmul with moving
  dim >= 256; softmax denominator via a ones-column appended to V (row 64 of
  the ctx psum); per-head normalize fused into the psum->SBUF copy.
- Engine split: PE matmuls; DVE normalize/copies/masks; Act square/exp/gelu;
  Pool partition-broadcasts + residual adds.
"""

import numpy as np

import concourse.bass as bass
import concourse.tile as tile
from concourse import bacc, bass_utils, mybir

F32 = mybir.dt.float32
F32R = mybir.dt.float32r
BF16 = mybir.dt.bfloat16
AF = mybir.ActivationFunctionType
OP = mybir.AluOpType

B, S, D, H, F, L = 4, 2048, 1024, 16, 4096, 2
DH = D // H      # 64
DC = D // 128    # 8
FC = F // 128    # 32
EPS = 1e-5
SCALE = 1.0 / np.sqrt(DH)
P = 128
VP = 384         # vocab 260 padded
VC = VP // 128   # 3
NCORES = 8


def _chunks(lt):
    out = []
    o = 0
    while o < lt:
        c = min(512, lt - o)
        out.append((o, c))
        o += c
    return out


def _build(lt, nt, use_lng):
    """lt: tokens per shard; nt: token tiles; use_lng: emit ln gamma/beta ops."""
    ptl = nt * P
    chs = _chunks(lt)
    nc = bacc.Bacc("TRN2", target_bir_lowering=False, debug=False,
                   num_devices=NCORES)

    def din(name, shape, dt=BF16):
        return nc.dram_tensor(name, shape, dt, kind="ExternalInput").ap()

    oht = din("oht", [P, VC * ptl])
    tokemb_d = din("tokemb", [P, VC * D])
    baseT = din("baseT", [P, DC * ptl], F32R)
    masks_d = din("masks", [P, nt * 384])
    # prepacked weights
    kcb_d, qcb_d, ocb_d, vrb_d, w1cb_d, w2cb_d = [], [], [], [], [], []
    for l in range(L):
        kcb_d.append(din(f"kcb{l}", [P, DC * DC * 128]))
        qcb_d.append(din(f"qcb{l}", [P, DC * DC * 128]))
        ocb_d.append(din(f"ocb{l}", [P, DC * DC * 128]))
        vrb_d.append(din(f"vrb{l}", [P, DC * D]))
        w1cb_d.append(din(f"w1cb{l}", [P, 8 * DC * 512]))
        w2cb_d.append(din(f"w2cb{l}", [P, DC * FC * 128]))
    # packed per-feature consts: [P, col] layout, 8 cols per D-vector
    # cols: 0 ones | 1 eps(row0) | then per layer l at 2+64*l:
    #   bq 0:8 bk 8:16 bv 16:24 bo 24:32 b2 32:40 b1 40:72 (unused gap)
    # ln g/b (if use_lng): separate tensor lngb
    cb_d = din("cb", [P, 2 + 96 * L], F32)
    lngb_d = din("lngb", [P, 8 * (2 + 4 * L)], F32) if use_lng else None
    houtT = nc.dram_tensor("houtT", [P, DC * ptl], F32R,
                           kind="ExternalOutput").ap()

    with tile.TileContext(nc) as tc:
        with (
            nc.allow_low_precision(
                reason="bf16 softmax/LN staging validated vs reference"),
            tc.tile_pool(name="pers", bufs=1) as pers,
            tc.tile_pool(name="big", bufs=4) as big,
            tc.tile_pool(name="xhp", bufs=1) as xhp,
            tc.tile_pool(name="wcb", bufs=3) as wcb,
            tc.tile_pool(name="est", bufs=2) as estp,
            tc.tile_pool(name="lnt", bufs=3) as lnp,
            tc.tile_pool(name="sm", bufs=2) as smp,
            tc.tile_pool(name="dv", bufs=2) as dvp,
            tc.tile_pool(name="pp", bufs=8, space="PSUM") as pp,
        ):
            cb = pers.tile([P, 2 + 96 * L], F32, tag="cb")
            nc.sync.dma_start(out=cb, in_=cb_d)
            eps_t = cb[0:1, 1:2]
            ones_r = pers.tile([P, 1], F32R, tag="ones_r")
            nc.vector.tensor_copy(ones_r, cb[:, 0:1])
            ones_b = pers.tile([P, 1], BF16, tag="ones_b")
            nc.vector.tensor_copy(ones_b, cb[:, 0:1])
            if use_lng:
                lngb = pers.tile([P, 8 * (2 + 4 * L)], F32, tag="lngb")
                nc.sync.dma_start(out=lngb, in_=lngb_d)

            masks = pers.tile([P, nt * 384], BF16, tag="masks")
            nc.sync.dma_start(out=masks, in_=masks_d)

            hT = pers.tile([P, DC * ptl], F32R, tag="hT")

            def bcol(l, i):  # bias col i (in 8-col groups) for layer l
                c0 = 2 + 96 * l + 8 * i
                return cb[:, c0:c0 + 8]

            def ln_pass(gi, out_tile, out_dtype_is_h):
                """LayerNorm hT over features -> out_tile (stride ptl).
                gi: index into lngb groups (g at 8*(2*gi), b at +8) or None
                handling via use_lng; out_dtype_is_h: write back into hT."""
                for (t0, cl) in chs:
                    ps1 = pp.tile([1, 512], F32, tag="mm", name="lns1")
                    ps2 = pp.tile([1, 512], F32, tag="mm", name="lns2")
                    for dc in range(DC):
                        hsl = hT[:, dc * ptl + t0:dc * ptl + t0 + cl]
                        sq = lnp.tile([P, 512], BF16, tag="sq", name=f"sq{dc}")
                        nc.scalar.activation(sq[:, 0:cl], hsl, AF.Square)
                        nc.tensor.matmul(ps1[:, 0:cl], lhsT=ones_r, rhs=hsl,
                                         start=(dc == 0), stop=(dc == DC - 1))
                        nc.tensor.matmul(ps2[:, 0:cl], lhsT=ones_b,
                                         rhs=sq[:, 0:cl],
                                         start=(dc == 0), stop=(dc == DC - 1))
                    st = smp.tile([P, 4 * 512], F32, tag="st", name="st")
                    mean = st[0:1, 0:cl]
                    var = st[0:1, 512:512 + cl]
                    rstd = st[0:1, 1024:1024 + cl]
                    mr = st[0:1, 1536:1536 + cl]
                    nc.vector.tensor_scalar_mul(mean, ps1[:, 0:cl], 1.0 / D)
                    nc.vector.tensor_mul(var, mean, mean)
                    nc.vector.scalar_tensor_tensor(
                        var, ps2[:, 0:cl], 1.0 / D, var,
                        op0=OP.mult, op1=OP.subtract)
                    nc.scalar.activation(rstd, var, AF.Sqrt, bias=eps_t)
                    nc.vector.reciprocal(rstd, rstd)
                    nc.vector.tensor_mul(mr, mean, rstd)
                    stb = smp.tile([P, 2 * 512], BF16, tag="stb", name="stb")
                    nc.gpsimd.tensor_copy(stb[0:1, :], st[0:1, 1024:2048])
                    RM = dvp.tile([P, 2 * 512], BF16, tag="rm", name="RM")
                    nc.gpsimd.partition_broadcast(RM[:, 0:cl], stb[0:1, 0:cl])
                    nc.gpsimd.partition_broadcast(RM[:, 512:512 + cl],
                                                  stb[0:1, 512:512 + cl])
                    for dc in range(DC):
                        hsl = hT[:, dc * ptl + t0:dc * ptl + t0 + cl]
                        d1 = lnp.tile([P, 512], BF16, tag="d1", name=f"d1_{dc}")
                        nc.vector.tensor_mul(d1[:, 0:cl], hsl, RM[:, 0:cl])
                        osl = out_tile[:, dc * ptl + t0:dc * ptl + t0 + cl]
                        if use_lng and gi is not None:
                            d2 = lnp.tile([P, 512], BF16, tag="d2",
                                          name=f"d2_{dc}")
                            nc.vector.tensor_sub(d2[:, 0:cl], d1[:, 0:cl],
                                                 RM[:, 512:512 + cl])
                            g0 = 8 * (2 * gi)
                            nc.vector.tensor_scalar(
                                osl, d2[:, 0:cl],
                                lngb[:, g0 + dc:g0 + dc + 1],
                                lngb[:, g0 + 8 + dc:g0 + 8 + dc + 1],
                                op0=OP.mult, op1=OP.add)
                        else:
                            nc.vector.tensor_sub(osl, d1[:, 0:cl],
                                                 RM[:, 512:512 + cl])

            # ---------- embeddings ----------
            ohsb = wcb.tile([P, VC * ptl], BF16, tag="w", name="ohsb")
            nc.sync.dma_start(out=ohsb, in_=oht)
            tesb = wcb.tile([P, VC * D], BF16, tag="w", name="tesb")
            nc.sync.dma_start(out=tesb, in_=tokemb_d)
            for dc in range(DC):
                nc.sync.dma_start(out=hT[:, dc * ptl:(dc + 1) * ptl],
                                  in_=baseT[:, dc * ptl:(dc + 1) * ptl])
            for dc in range(DC):
                for (t0, cl) in chs:
                    pse = pp.tile([P, 512], F32, tag="mm", name="pse")
                    for vc in range(VC):
                        nc.tensor.matmul(
                            pse[:, 0:cl],
                            lhsT=tesb[:, vc * D + dc * 128:vc * D + dc * 128 + 128],
                            rhs=ohsb[:, vc * ptl + t0:vc * ptl + t0 + cl],
                            start=(vc == 0), stop=(vc == VC - 1))
                    hsl = hT[:, dc * ptl + t0:dc * ptl + t0 + cl]
                    nc.vector.tensor_add(hsl, pse[:, 0:cl], hsl)
            ln_pass(None, hT, True)   # LN0 in place (g/b via lngb group 0...)

            # ---------- layers ----------
            for l in range(L):
                xh = xhp.tile([P, DC * ptl], BF16, tag="xh", name=f"xh{l}a")
                ln_pass(2 * l if use_lng else None, xh, False)

                # ---- K/Q/V + attention, interleaved ----
                KT = big.tile([P, DC * ptl], BF16, tag="b18", name=f"KT{l}")
                Vsb = big.tile([P, nt * H * 65], BF16, tag="b18", name=f"Vsb{l}")
                QT = big.tile([P, DC * ptl], BF16, tag="b18", name=f"QT{l}")
                ctxc = big.tile([P, DC * ptl], BF16, tag="b18", name=f"ctx{l}")
                if lt < ptl:
                    nc.vector.memset(
                        Vsb[:, (nt - 1) * H * 65:nt * H * 65], 0.0)
                ones_v = Vsb.rearrange("p (g x) -> p g x", x=65)[:, :, 64:65]
                nc.vector.memset(ones_v, 1.0)

                def v_half(nh):
                    ntg = (nt + 3) // 4
                    for tg in range(ntg):
                        tts = [t for t in range(4 * tg, min(4 * tg + 4, nt))
                               if lt - t * P > 0]
                        pvs = {}
                        for tt in tts:
                            pvs[tt] = pp.tile([P, 512], F32, tag="mm",
                                              name=f"psv{tt}_{nh}")
                        for dc in range(DC):
                            vrb = wcb.tile([P, 512], BF16, tag="w",
                                           name=f"vrb{nh}_{tg}_{dc}")
                            nc.sync.dma_start(
                                out=vrb,
                                in_=vrb_d[l][:, (nh * DC + dc) * 512:
                                             (nh * DC + dc + 1) * 512])
                            for tt in tts:
                                tl = min(P, lt - tt * P)
                                nc.tensor.matmul(
                                    pvs[tt][0:tl, :],
                                    lhsT=xh[:, dc * ptl + tt * P:dc * ptl + tt * P + tl],
                                    rhs=vrb,
                                    start=(dc == 0), stop=(dc == DC - 1))
                        for tt in tts:
                            tl = min(P, lt - tt * P)
                            pv = pvs[tt][0:tl, :].rearrange(
                                "p (h x) -> p h x", h=8)
                            ov = Vsb[0:tl, (tt * H + nh * 8) * 65:
                                     (tt * H + nh * 8 + 8) * 65].rearrange(
                                "p (h x) -> p h x", x=65)[:, :, 0:64]
                            nc.vector.tensor_copy(ov, pv)

                def kq_block(oc):
                    kcb = wcb.tile([P, DC * 128], BF16, tag="w",
                                   name=f"kcb{oc}")
                    nc.sync.dma_start(
                        out=kcb, in_=kcb_d[l][:, oc * D:(oc + 1) * D])
                    for (t0, cl) in chs:
                        ps = pp.tile([P, 512], F32, tag="mm", name=f"psk{oc}")
                        for dc in range(DC):
                            nc.tensor.matmul(
                                ps[:, 0:cl],
                                lhsT=kcb[:, dc * 128:dc * 128 + 128],
                                rhs=xh[:, dc * ptl + t0:dc * ptl + t0 + cl],
                                start=(dc == 0), stop=(dc == DC - 1))
                        nc.vector.tensor_scalar_add(
                            KT[:, oc * ptl + t0:oc * ptl + t0 + cl],
                            ps[:, 0:cl], bcol(l, 1)[:, oc:oc + 1])
                    qcb = wcb.tile([P, DC * 128], BF16, tag="w",
                                   name=f"qcb{oc}")
                    nc.sync.dma_start(
                        out=qcb, in_=qcb_d[l][:, oc * D:(oc + 1) * D])
                    for (t0, cl) in chs:
                        ps = pp.tile([P, 512], F32, tag="mm", name=f"psq{oc}")
                        for dc in range(DC):
                            nc.tensor.matmul(
                                ps[:, 0:cl],
                                lhsT=qcb[:, dc * 128:dc * 128 + 128],
                                rhs=xh[:, dc * ptl + t0:dc * ptl + t0 + cl],
                                start=(dc == 0), stop=(dc == DC - 1))
                        nc.vector.tensor_scalar_add(
                            QT[:, oc * ptl + t0:oc * ptl + t0 + cl],
                            ps[:, 0:cl], bcol(l, 0)[:, oc:oc + 1])
                    if lt < ptl:
                        nc.vector.memset(KT[:, oc * ptl + lt:(oc + 1) * ptl],
                                         0.0)
                        nc.vector.memset(QT[:, oc * ptl + lt:(oc + 1) * ptl],
                                         0.0)

                def head_scores(h):
                    dch, po = h // 2, (h % 2) * 64
                    est = estp.tile([P, nt * 384], BF16, tag="est",
                                    name=f"est{h}")
                    ests[h] = est
                    for j in range(nt):
                        lo = max(j - 1, 0)
                        hi = min(j + 1, nt - 1)
                        nq = (hi - lo + 1) * P
                        w0 = min(max(j - 1, 0), nt - 3)
                        pst = pp.tile([P, 384], F32, tag="mm", name=f"pst{j}")
                        nc.tensor.matmul(
                            pst[:, 0:nq],
                            lhsT=KT[po:po + 64, dch * ptl + j * P:dch * ptl + j * P + P],
                            rhs=QT[po:po + 64, dch * ptl + lo * P:dch * ptl + lo * P + nq],
                            start=True, stop=True)
                        esl = est[:, j * 384 + (lo - w0) * P:
                                  j * 384 + (lo - w0) * P + nq]
                        nc.scalar.activation(esl, pst[:, 0:nq], AF.Exp,
                                             scale=float(SCALE))
                    nc.vector.tensor_mul(est, est, masks)

                def head_ctx(h):
                    dch, po = h // 2, (h % 2) * 64
                    est = ests[h]
                    for qg in range((nt + 3) // 4):
                        qts = [q for q in range(4 * qg, min(4 * qg + 4, nt))]
                        gw = len(qts) * P
                        psc = pp.tile([65, 512], F32, tag="mm", name=f"psc{qg}")
                        for qi, qt in enumerate(qts):
                            js = [j for j in (qt - 1, qt, qt + 1)
                                  if 0 <= j < nt]
                            for kk, j in enumerate(js):
                                w0 = min(max(j - 1, 0), nt - 3)
                                rsl = est[:, j * 384 + (qt - w0) * P:
                                          j * 384 + (qt - w0) * P + P]
                                nc.tensor.matmul(
                                    psc[:, qi * P:(qi + 1) * P],
                                    lhsT=Vsb[:, (j * H + h) * 65:
                                             (j * H + h) * 65 + 65],
                                    rhs=rsl,
                                    start=(kk == 0), stop=(kk == len(js) - 1))
                        dinv = dvp.tile([1, 512], BF16, tag="dinv",
                                        name=f"dinv{qg}")
                        nc.vector.reciprocal(dinv[:, 0:gw], psc[64:65, 0:gw])
                        dnb = dvp.tile([P, 512], BF16, tag="dnb",
                                       name=f"dnb{qg}")
                        nc.gpsimd.partition_broadcast(dnb[0:64, 0:gw],
                                                      dinv[:, 0:gw])
                        nc.vector.tensor_mul(
                            ctxc[po:po + 64,
                                 dch * ptl + qg * 512:dch * ptl + qg * 512 + gw],
                            psc[0:64, 0:gw], dnb[0:64, 0:gw])

                ests = {}
                v_half(0)
                kq_block(0)
                for oc in range(1, DC):
                    if oc == 5:
                        v_half(1)
                    head_scores(2 * oc - 2)
                    head_scores(2 * oc - 1)
                    kq_block(oc)
                    head_ctx(2 * oc - 2)
                    head_ctx(2 * oc - 1)
                head_scores(14)
                head_scores(15)
                head_ctx(14)
                head_ctx(15)

                # ---- O-projection + residual ----
                for do_ in range(DC):
                    ocb = wcb.tile([P, DC * 128], BF16, tag="w", name=f"ocb{do_}")
                    nc.sync.dma_start(
                        out=ocb, in_=ocb_d[l][:, do_ * D:(do_ + 1) * D])
                    for (t0, cl) in chs:
                        ps = pp.tile([P, 512], F32, tag="mm", name=f"pso{do_}")
                        for dc in range(DC):
                            nc.tensor.matmul(
                                ps[:, 0:cl], lhsT=ocb[:, dc * 128:dc * 128 + 128],
                                rhs=ctxc[:, dc * ptl + t0:dc * ptl + t0 + cl],
                                start=(dc == 0), stop=(dc == DC - 1))
                        hsl = hT[:, do_ * ptl + t0:do_ * ptl + t0 + cl]
                        nc.vector.scalar_tensor_tensor(
                            hsl, ps[:, 0:cl], bcol(l, 3)[:, do_:do_ + 1], hsl,
                            op0=OP.add, op1=OP.add)

                # ---- FFN ----
                xh = xhp.tile([P, DC * ptl], BF16, tag="xh", name=f"xh{l}b")
                ln_pass(2 * l + 1 if use_lng else None, xh, False)
                Us = [big.tile([P, 8 * ptl], BF16, tag="b18", name=f"U{l}_{i}")
                      for i in range(4)]

                def usl(fc, t0, cl):
                    t = Us[fc // 8]
                    k = fc % 8
                    return t[:, k * ptl + t0:k * ptl + t0 + cl]

                for fcb in range(8):
                    w1cb = wcb.tile([P, DC * 512], BF16, tag="w",
                                    name=f"w1cb{fcb}")
                    nc.sync.dma_start(
                        out=w1cb,
                        in_=w1cb_d[l][:, fcb * DC * 512:(fcb + 1) * DC * 512])
                    for fc2 in range(4):
                        fc = fcb * 4 + fc2
                        for (t0, cl) in chs:
                            ps = pp.tile([P, 512], F32, tag="mm",
                                         name=f"psf{fc2}")
                            for dc in range(DC):
                                nc.tensor.matmul(
                                    ps[:, 0:cl],
                                    lhsT=w1cb[:, dc * 512 + fc2 * 128:
                                              dc * 512 + fc2 * 128 + 128],
                                    rhs=xh[:, dc * ptl + t0:dc * ptl + t0 + cl],
                                    start=(dc == 0), stop=(dc == DC - 1))
                            bidx = 5 + fc // 8
                            nc.scalar.activation(
                                usl(fc, t0, cl), ps[:, 0:cl], AF.Gelu,
                                bias=bcol(l, bidx)[:, fc % 8:fc % 8 + 1])
                for do_ in range(DC):
                    w2cb = wcb.tile([P, FC * 128], BF16, tag="w",
                                    name=f"w2cb{do_}")
                    nc.sync.dma_start(
                        out=w2cb,
                        in_=w2cb_d[l][:, do_ * FC * 128:(do_ + 1) * FC * 128])
                    for (t0, cl) in chs:
                        ps = pp.tile([P, 512], F32, tag="mm", name=f"psh{do_}")
                        for fc in range(FC):
                            nc.tensor.matmul(
                                ps[:, 0:cl],
                                lhsT=w2cb[:, fc * 128:fc * 128 + 128],
                                rhs=usl(fc, t0, cl),
                                start=(fc == 0), stop=(fc == FC - 1))
                        hsl = hT[:, do_ * ptl + t0:do_ * ptl + t0 + cl]
                        nc.vector.scalar_tensor_tensor(
                            hsl, ps[:, 0:cl], bcol(l, 4)[:, do_:do_ + 1], hsl,
                            op0=OP.add, op1=OP.add)
                    if l == L - 1:
                        nc.sync.dma_start(
                            out=houtT[:, do_ * ptl:(do_ + 1) * ptl],
                            in_=hT[:, do_ * ptl:(do_ + 1) * ptl])

    nc.compile()
    return nc


_NC_CACHE = {}


def _get_nc(lt=1032, nt=9, use_lng=False):
    key = (lt, nt, use_lng)
    if key not in _NC_CACHE:
        _NC_CACHE[key] = _build(lt, nt, use_lng)
    return _NC_CACHE[key]


def _pack_shared(inputs, lt, nt, use_lng):
    bf = np.dtype("bfloat16") if hasattr(np, "bfloat16") else None
    import ml_dtypes
    BFD = ml_dtypes.bfloat16

    def b16(x):
        return np.ascontiguousarray(np.asarray(x, np.float32).astype(BFD))

    tok = np.asarray(inputs["tok_emb"], np.float32)
    tokp = np.zeros((VP, D), np.float32)
    tokp[:tok.shape[0]] = tok
    tokemb = b16(tokp.reshape(VC, P, D).transpose(1, 0, 2).reshape(P, VC * D))

    shared = {"tokemb": tokemb}
    for l in range(L):
        Wq = np.asarray(inputs["Wq"][l], np.float32)
        Wk = np.asarray(inputs["Wk"][l], np.float32)
        Wv = np.asarray(inputs["Wv"][l], np.float32)
        Wo = np.asarray(inputs["Wo"][l], np.float32)
        W1 = np.asarray(inputs["W1"][l], np.float32)
        W2 = np.asarray(inputs["W2"][l], np.float32)

        def colblocks(W, ocn):  # [D, D] -> [P, ocn*DC*128]
            # block (oc): [p, dc, c] = W[dc*128+p, oc*128+c]
            Wr = W.reshape(DC, P, ocn, 128)  # [dc, p, oc, c]
            return np.ascontiguousarray(
                Wr.transpose(1, 2, 0, 3).reshape(P, ocn * DC * 128))

        shared[f"kcb{l}"] = b16(colblocks(Wk, DC))
        shared[f"qcb{l}"] = b16(colblocks(Wq, DC))
        shared[f"ocb{l}"] = b16(colblocks(Wo, DC))
        # vrb: [p, nh, dc, c] = Wv[dc*128+p, nh*512+c]
        Wvr = Wv.reshape(DC, P, 2, 512)
        shared[f"vrb{l}"] = b16(
            Wvr.transpose(1, 2, 0, 3).reshape(P, 2 * DC * 512))
        # w1cb: [p, fcb, dc, c] = W1[dc*128+p, fcb*512+c]
        W1r = W1.reshape(DC, P, 8, 512)
        shared[f"w1cb{l}"] = b16(
            W1r.transpose(1, 2, 0, 3).reshape(P, 8 * DC * 512))
        # w2cb: [p, do, fc, c] = W2[fc*128+p, do*128+c]
        W2r = W2.reshape(FC, P, DC, 128)
        shared[f"w2cb{l}"] = b16(
            W2r.transpose(1, 2, 0, 3).reshape(P, DC * FC * 128))

    cbw = np.zeros((P, 2 + 96 * L), np.float32)
    cbw[:, 0] = 1.0
    cbw[0, 1] = EPS
    for l in range(L):
        c0 = 2 + 96 * l
        # bv is folded into bo: probs sum to 1, so ctx@Wo + bo with V+bv
        # equals (ctx from plain V)@Wo + (bo + bv@Wo).
        bo_eff = (np.asarray(inputs["bo"][l], np.float32)
                  + np.asarray(inputs["bv"][l], np.float32)
                  @ np.asarray(inputs["Wo"][l], np.float32))
        vals = {"bq": np.asarray(inputs["bq"][l], np.float32),
                "bk": np.asarray(inputs["bk"][l], np.float32),
                "bv": np.zeros(D, np.float32),
                "bo": bo_eff,
                "b2": np.asarray(inputs["b2"][l], np.float32)}
        for i, key in enumerate(("bq", "bk", "bv", "bo", "b2")):
            cbw[:, c0 + 8 * i:c0 + 8 * i + 8] = vals[key].reshape(DC, P).T
        b1v = np.asarray(inputs["b1"][l], np.float32)
        cbw[:, c0 + 40:c0 + 72] = b1v.reshape(FC, P).T
    shared["cb"] = np.ascontiguousarray(cbw)

    if use_lng:
        gb = np.zeros((P, 8 * (2 + 4 * L)), np.float32)
        # group 0: ln0 (handled as gi=None in build... keep identity)
        idx = 0
        for l in range(L):
            for which in range(2):
                gi = 2 * l + which
                g = np.asarray(inputs["ln1_g" if which == 0 else "ln2_g"][l],
                               np.float32)
                bb = np.asarray(inputs["ln1_b" if which == 0 else "ln2_b"][l],
                                np.float32)
                gb[:, 8 * (2 * gi):8 * (2 * gi) + 8] = g.reshape(DC, P).T
                gb[:, 8 * (2 * gi + 1):8 * (2 * gi + 1) + 8] = bb.reshape(DC, P).T
        shared["lngb"] = np.ascontiguousarray(gb)
    return shared


def _prep_core(inputs, b, start, n, lt, nt):
    import ml_dtypes
    BFD = ml_dtypes.bfloat16
    ptl = nt * P

    def b16(x):
        return np.ascontiguousarray(np.asarray(x, np.float32).astype(BFD))

    ids = np.asarray(inputs["input_ids"][b, start:start + n])
    pid = np.asarray(inputs["patch_ids"][b, start:start + n]).astype(np.int64)
    pos_emb = np.asarray(inputs["pos_emb"], np.float32)
    hashes = np.asarray(inputs["hash_embeddings"], np.float32)

    oh = np.zeros((VP, ptl), np.float32)
    oh[ids, np.arange(n)] = 1.0
    oht = b16(oh.reshape(VC, P, ptl).transpose(1, 0, 2).reshape(P, VC * ptl))

    base = np.zeros((ptl, D), np.float32)
    base[:n] = pos_emb[start:start + n] + hashes[b, start:start + n]
    baseT = np.ascontiguousarray(
        base.reshape(ptl, DC, P).transpose(2, 1, 0).reshape(P, DC * ptl))

    pidp = np.empty(ptl, np.int64)
    pidp[:n] = pid
    pidp[n:] = -np.arange(1, ptl - n + 1)

    m = np.zeros((nt, P, 384), np.float32)
    for j in range(nt):
        w0 = np.clip(j - 1, 0, nt - 3) * P
        kk = pidp[j * P:(j + 1) * P]
        qq = pidp[w0:w0 + 384]
        m[j] = (kk[:, None] == qq[None, :]).astype(np.float32)
    masks = b16(m.transpose(1, 0, 2).reshape(P, nt * 384))
    return {"oht": oht, "baseT": baseT, "masks": masks}


def kernel(**inputs):
    pid_all = np.asarray(inputs["patch_ids"])

    shards = []
    for b in range(B):
        pid = np.asarray(pid_all[b])
        bnd = np.nonzero(pid[1:] != pid[:-1])[0] + 1
        cand = bnd[(bnd >= S - 1152) & (bnd <= 1152)]
        if len(cand) == 0:
            raise RuntimeError("no patch boundary near S/2; cannot shard")
        s = int(cand[np.argmin(np.abs(cand - S // 2))])
        shards.append((b, 0, s))
        shards.append((b, s, S - s))

    lt = max(n for _, _, n in shards)
    lt = max(lt, 1026)  # floor so chunk 3 isn't degenerate-tiny
    nt = (lt + P - 1) // P

    use_lng = not (
        all(np.all(np.asarray(inputs[k]) == 1.0)
            for k in ("ln0_g", "ln1_g", "ln2_g")) and
        all(np.all(np.asarray(inputs[k]) == 0.0)
            for k in ("ln0_b", "ln1_b", "ln2_b")))
    if use_lng:
        raise NotImplementedError(
            "non-identity LN affine not supported in fast path")

    shared = _pack_shared(inputs, lt, nt, use_lng)
    in_maps = []
    for b, start, n in shards:
        mcore = dict(shared)
        mcore.update(_prep_core(inputs, b, start, n, lt, nt))
        in_maps.append(mcore)

    nc = _get_nc(lt, nt, use_lng)
    res = bass_utils.run_bass_kernel_spmd(nc, in_maps,
                                          core_ids=list(range(NCORES)))

    ptl = nt * P
    out = np.zeros((B, S, D), np.float32)
    for i, (b, start, n) in enumerate(shards):
        ht = res.results[i]["houtT"]
        hfull = ht.reshape(P, DC, ptl).transpose(2, 1, 0).reshape(ptl, D)
        out[b, start:start + n] = hfull[:n]
    return out


if __name__ == "__main__":
    import sys
    lt = int(sys.argv[1]) if len(sys.argv) > 1 else 1032
    _get_nc(lt, (lt + P - 1) // P, False)
    print("built ok")


# revision 16
# speedup vs baseline: 1.2094x; 1.0053x over previous
"""BLT local encoder (2-layer transformer, patch-equality block-diagonal attention)
on 8 Trainium2 NeuronCores.

v2. Sharding: each of the 4 sequences splits at a patch-run boundary nearest
S/2 -> 8 independent shards, one per core, zero cross-core communication.

Kernel design (per core, L_tok = max shard length ~1032):
- Residual hT kept float32 feature-major [P, 8dc x PTL]; everything else bf16.
- Weights prepacked host-side into SBUF-ready bf16 col/row blocks, streamed
  once per layer (no restreaming), double-buffered.
- One LayerNorm per sublayer, output xh bf16 reused by Q, K and V.
- Full-shard attention: per (head, key-tile j) one score matmul with moving
  dim >= 256; softmax denominator via a ones-column appended to V (row 64 of
  the ctx psum); per-head normalize fused into the psum->SBUF copy.
- Engine split: PE matmuls; DVE normalize/copies/masks; Act square/exp/gelu;
  Pool partition-broadcasts + residual adds.
"""

import numpy as np

import concourse.bass as bass
import concourse.tile as tile
from concourse import bacc, bass_utils, mybir

F32 = mybir.dt.float32
F32R = mybir.dt.float32r
BF16 = mybir.dt.bfloat16
AF = mybir.ActivationFunctionType
OP = mybir.AluOpType

B, S, D, H, F, L = 4, 2048, 1024, 16, 4096, 2
DH = D // H      # 64
DC = D // 128    # 8
FC = F // 128    # 32
EPS = 1e-5
SCALE = 1.0 / np.sqrt(DH)
P = 128
VP = 384         # vocab 260 padded
VC = VP // 128   # 3
NCORES = 8


def _chunks(lt):
    out = []
    o = 0
    while o < lt:
        c = min(512, lt - o)
        out.append((o, c))
        o += c
    return out


def _build(lt, nt, use_lng):
    """lt: tokens per shard; nt: token tiles; use_lng: emit ln gamma/beta ops."""
    ptl = nt * P
    chs = _chunks(lt)
    nc = bacc.Bacc("TRN2", target_bir_lowering=False, debug=False,
                   num_devices=NCORES)

    def din(name, shape, dt=BF16):
        return nc.dram_tensor(name, shape, dt, kind="ExternalInput").ap()

    oht = din("oht", [P, VC * ptl])
    tokemb_d = din("tokemb", [P, VC * D])
    baseT = din("baseT", [P, DC * ptl], F32R)
    masks_d = din("masks", [P, nt * 384])
    # prepacked weights
    kcb_d, qcb_d, ocb_d, vrb_d, w1cb_d, w2cb_d = [], [], [], [], [], []
    for l in range(L):
        kcb_d.append(din(f"kcb{l}", [P, DC * DC * 128]))
        qcb_d.append(din(f"qcb{l}", [P, DC * DC * 128]))
        ocb_d.append(din(f"ocb{l}", [P, DC * DC * 128]))
        vrb_d.append(din(f"vrb{l}", [P, DC * D]))
        w1cb_d.append(din(f"w1cb{l}", [P, 8 * DC * 512]))
        w2cb_d.append(din(f"w2cb{l}", [P, DC * FC * 128]))
    # packed per-feature consts: [P, col] layout, 8 cols per D-vector
    # cols: 0 ones | 1 eps(row0) | then per layer l at 2+64*l:
    #   bq 0:8 bk 8:16 bv 16:24 bo 24:32 b2 32:40 b1 40:72 (unused gap)
    # ln g/b (if use_lng): separate tensor lngb
    cb_d = din("cb", [P, 2 + 96 * L], F32)
    lngb_d = din("lngb", [P, 8 * (2 + 4 * L)], F32) if use_lng else None
    houtT = nc.dram_tensor("houtT", [P, DC * ptl], F32R,
                           kind="ExternalOutput").ap()

    with tile.TileContext(nc) as tc:
        with (
            nc.allow_low_precision(
                reason="bf16 softmax/LN staging validated vs reference"),
            tc.tile_pool(name="pers", bufs=1) as pers,
            tc.tile_pool(name="big", bufs=4) as big,
            tc.tile_pool(name="xhp", bufs=1) as xhp,
            tc.tile_pool(name="wcb", bufs=3) as wcb,
            tc.tile_pool(name="est", bufs=2) as estp,
            tc.tile_pool(name="lnt", bufs=3) as lnp,
            tc.tile_pool(name="sm", bufs=2) as smp,
            tc.tile_pool(name="dv", bufs=2) as dvp,
            tc.tile_pool(name="pp", bufs=8, space="PSUM") as pp,
        ):
            cb = pers.tile([P, 2 + 96 * L], F32, tag="cb")
            nc.sync.dma_start(out=cb, in_=cb_d)
            eps_t = cb[0:1, 1:2]
            ones_r = pers.tile([P, 1], F32R, tag="ones_r")
            nc.vector.tensor_copy(ones_r, cb[:, 0:1])
            ones_b = pers.tile([P, 1], BF16, tag="ones_b")
            nc.vector.tensor_copy(ones_b, cb[:, 0:1])
            if use_lng:
                lngb = pers.tile([P, 8 * (2 + 4 * L)], F32, tag="lngb")
                nc.sync.dma_start(out=lngb, in_=lngb_d)

            masks = pers.tile([P, nt * 384], BF16, tag="masks")
            nc.sync.dma_start(out=masks, in_=masks_d)

            hT = pers.tile([P, DC * ptl], F32R, tag="hT")

            def bcol(l, i):  # bias col i (in 8-col groups) for layer l
                c0 = 2 + 96 * l + 8 * i
                return cb[:, c0:c0 + 8]

            def ln_chunk(gi, out_tile, t0, cl):
                ps1 = pp.tile([1, 512], F32, tag="mm", name="lns1")
                ps2 = pp.tile([1, 512], F32, tag="mm", name="lns2")
                for dc in range(DC):
                    hsl = hT[:, dc * ptl + t0:dc * ptl + t0 + cl]
                    sq = lnp.tile([P, 512], BF16, tag="sq", name=f"sq{dc}")
                    nc.scalar.activation(sq[:, 0:cl], hsl, AF.Square)
                    nc.tensor.matmul(ps1[:, 0:cl], lhsT=ones_r, rhs=hsl,
                                     start=(dc == 0), stop=(dc == DC - 1))
                    nc.tensor.matmul(ps2[:, 0:cl], lhsT=ones_b,
                                     rhs=sq[:, 0:cl],
                                     start=(dc == 0), stop=(dc == DC - 1))
                st = smp.tile([P, 2 * 512], F32, tag="st", name="st")
                stb = smp.tile([P, 2 * 512], BF16, tag="stb", name="stb")
                mean = st[0:1, 0:cl]
                var = st[0:1, 512:512 + cl]
                rstd = stb[0:1, 0:cl]
                mr = stb[0:1, 512:512 + cl]
                nc.vector.tensor_scalar_mul(mean, ps1[:, 0:cl], 1.0 / D)
                nc.vector.tensor_mul(var, mean, mean)
                nc.vector.scalar_tensor_tensor(
                    var, ps2[:, 0:cl], 1.0 / D, var,
                    op0=OP.mult, op1=OP.subtract)
                nc.scalar.activation(var, var, AF.Sqrt, bias=eps_t)
                nc.vector.reciprocal(rstd, var)
                nc.vector.tensor_mul(mr, mean, rstd)
                RM = dvp.tile([P, 2 * 512], BF16, tag="rm", name="RM")
                nc.gpsimd.partition_broadcast(RM[:, 0:cl], rstd)
                nc.gpsimd.partition_broadcast(RM[:, 512:512 + cl], mr)
                for dc in range(DC):
                    hsl = hT[:, dc * ptl + t0:dc * ptl + t0 + cl]
                    d1 = lnp.tile([P, 512], BF16, tag="d1", name=f"d1_{dc}")
                    nc.vector.tensor_mul(d1[:, 0:cl], hsl, RM[:, 0:cl])
                    osl = out_tile[:, dc * ptl + t0:dc * ptl + t0 + cl]
                    if use_lng and gi is not None:
                        d2 = lnp.tile([P, 512], BF16, tag="d2",
                                      name=f"d2_{dc}")
                        nc.vector.tensor_sub(d2[:, 0:cl], d1[:, 0:cl],
                                             RM[:, 512:512 + cl])
                        g0 = 8 * (2 * gi)
                        nc.vector.tensor_scalar(
                            osl, d2[:, 0:cl],
                            lngb[:, g0 + dc:g0 + dc + 1],
                            lngb[:, g0 + 8 + dc:g0 + 8 + dc + 1],
                            op0=OP.mult, op1=OP.add)
                    else:
                        nc.vector.tensor_sub(osl, d1[:, 0:cl],
                                             RM[:, 512:512 + cl])

            def ln_pass(gi, out_tile):
                for (t0, cl) in chs:
                    ln_chunk(gi, out_tile, t0, cl)

            # ---------- embeddings ----------
            ohsb = wcb.tile([P, VC * ptl], BF16, tag="w", name="ohsb")
            nc.sync.dma_start(out=ohsb, in_=oht)
            tesb = wcb.tile([P, VC * D], BF16, tag="w", name="tesb")
            nc.sync.dma_start(out=tesb, in_=tokemb_d)
            for dc in range(DC):
                nc.sync.dma_start(out=hT[:, dc * ptl:(dc + 1) * ptl],
                                  in_=baseT[:, dc * ptl:(dc + 1) * ptl])
            for ci, (t0, cl) in enumerate(chs):
                for dc in range(DC):
                    pse = pp.tile([P, 512], F32, tag="mm", name="pse")
                    for vc in range(VC):
                        nc.tensor.matmul(
                            pse[:, 0:cl],
                            lhsT=tesb[:, vc * D + dc * 128:vc * D + dc * 128 + 128],
                            rhs=ohsb[:, vc * ptl + t0:vc * ptl + t0 + cl],
                            start=(vc == 0), stop=(vc == VC - 1))
                    hsl = hT[:, dc * ptl + t0:dc * ptl + t0 + cl]
                    nc.vector.tensor_add(hsl, pse[:, 0:cl], hsl)
                ln_chunk(None, hT, t0, cl)   # LN0 in place

            # ---------- layers ----------
            for l in range(L):
                xh = xhp.tile([P, DC * ptl], BF16, tag="xh", name=f"xh{l}a")

                # ---- K/Q/V + attention, interleaved ----
                KT = big.tile([P, DC * ptl], BF16, tag="b18", name=f"KT{l}")
                Vsb = big.tile([P, nt * H * 65], BF16, tag="b18", name=f"Vsb{l}")
                QT = big.tile([P, DC * ptl], BF16, tag="b18", name=f"QT{l}")
                ctxc = big.tile([P, DC * ptl], BF16, tag="b18", name=f"ctx{l}")
                if lt < ptl:
                    nc.vector.memset(
                        Vsb[:, (nt - 1) * H * 65:nt * H * 65], 0.0)
                ones_v = Vsb.rearrange("p (g x) -> p g x", x=65)[:, :, 64:65]
                nc.vector.memset(ones_v, 1.0)

                def v_tg(nh, tg):
                    if True:
                        tts = [t for t in range(4 * tg, min(4 * tg + 4, nt))
                               if lt - t * P > 0]
                        pvs = {}
                        for tt in tts:
                            pvs[tt] = pp.tile([P, 512], F32, tag="mm",
                                              name=f"psv{tt}_{nh}")
                        for dc in range(DC):
                            vrb = wcb.tile([P, 512], BF16, tag="w",
                                           name=f"vrb{nh}_{tg}_{dc}")
                            nc.sync.dma_start(
                                out=vrb,
                                in_=vrb_d[l][:, (nh * DC + dc) * 512:
                                             (nh * DC + dc + 1) * 512])
                            for tt in tts:
                                tl = min(P, lt - tt * P)
                                nc.tensor.matmul(
                                    pvs[tt][0:tl, :],
                                    lhsT=xh[:, dc * ptl + tt * P:dc * ptl + tt * P + tl],
                                    rhs=vrb,
                                    start=(dc == 0), stop=(dc == DC - 1))
                        for tt in tts:
                            tl = min(P, lt - tt * P)
                            pv = pvs[tt][0:tl, :].rearrange(
                                "p (h x) -> p h x", h=8)
                            ov = Vsb[0:tl, (tt * H + nh * 8) * 65:
                                     (tt * H + nh * 8 + 8) * 65].rearrange(
                                "p (h x) -> p h x", x=65)[:, :, 0:64]
                            nc.vector.tensor_copy(ov, pv)

                def kq_block(oc):
                    kcb = wcb.tile([P, DC * 128], BF16, tag="w",
                                   name=f"kcb{oc}")
                    nc.sync.dma_start(
                        out=kcb, in_=kcb_d[l][:, oc * D:(oc + 1) * D])
                    for (t0, cl) in chs:
                        ps = pp.tile([P, 512], F32, tag="mm", name=f"psk{oc}")
                        for dc in range(DC):
                            nc.tensor.matmul(
                                ps[:, 0:cl],
                                lhsT=kcb[:, dc * 128:dc * 128 + 128],
                                rhs=xh[:, dc * ptl + t0:dc * ptl + t0 + cl],
                                start=(dc == 0), stop=(dc == DC - 1))
                        nc.vector.tensor_scalar_add(
                            KT[:, oc * ptl + t0:oc * ptl + t0 + cl],
                            ps[:, 0:cl], bcol(l, 1)[:, oc:oc + 1])
                    qcb = wcb.tile([P, DC * 128], BF16, tag="w",
                                   name=f"qcb{oc}")
                    nc.sync.dma_start(
                        out=qcb, in_=qcb_d[l][:, oc * D:(oc + 1) * D])
                    for (t0, cl) in chs:
                        ps = pp.tile([P, 512], F32, tag="mm", name=f"psq{oc}")
                        for dc in range(DC):
                            nc.tensor.matmul(
                                ps[:, 0:cl],
                                lhsT=qcb[:, dc * 128:dc * 128 + 128],
                                rhs=xh[:, dc * ptl + t0:dc * ptl + t0 + cl],
                                start=(dc == 0), stop=(dc == DC - 1))
                        nc.vector.tensor_scalar_add(
                            QT[:, oc * ptl + t0:oc * ptl + t0 + cl],
                            ps[:, 0:cl], bcol(l, 0)[:, oc:oc + 1])
                    if lt < ptl:
                        nc.vector.memset(KT[:, oc * ptl + lt:(oc + 1) * ptl],
                                         0.0)
                        nc.vector.memset(QT[:, oc * ptl + lt:(oc + 1) * ptl],
                                         0.0)

                def head_scores(h):
                    dch, po = h // 2, (h % 2) * 64
                    est = estp.tile([P, nt * 384], BF16, tag="est",
                                    name=f"est{h}")
                    ests[h] = est
                    for j in range(nt):
                        lo = max(j - 1, 0)
                        hi = min(j + 1, nt - 1)
                        nq = (hi - lo + 1) * P
                        w0 = min(max(j - 1, 0), nt - 3)
                        pst = pp.tile([P, 384], F32, tag="mm", name=f"pst{j}")
                        nc.tensor.matmul(
                            pst[:, 0:nq],
                            lhsT=KT[po:po + 64, dch * ptl + j * P:dch * ptl + j * P + P],
                            rhs=QT[po:po + 64, dch * ptl + lo * P:dch * ptl + lo * P + nq],
                            start=True, stop=True)
                        esl = est[:, j * 384 + (lo - w0) * P:
                                  j * 384 + (lo - w0) * P + nq]
                        nc.scalar.activation(esl, pst[:, 0:nq], AF.Exp,
                                             scale=float(SCALE))
                    nc.vector.tensor_mul(est, est, masks)

                def head_ctx(h):
                    dch, po = h // 2, (h % 2) * 64
                    est = ests[h]
                    for qg in range((nt + 3) // 4):
                        qts = [q for q in range(4 * qg, min(4 * qg + 4, nt))]
                        gw = len(qts) * P
                        psc = pp.tile([65, 512], F32, tag="mm", name=f"psc{qg}")
                        for qi, qt in enumerate(qts):
                            js = [j for j in (qt - 1, qt, qt + 1)
                                  if 0 <= j < nt]
                            for kk, j in enumerate(js):
                                w0 = min(max(j - 1, 0), nt - 3)
                                rsl = est[:, j * 384 + (qt - w0) * P:
                                          j * 384 + (qt - w0) * P + P]
                                nc.tensor.matmul(
                                    psc[:, qi * P:(qi + 1) * P],
                                    lhsT=Vsb[:, (j * H + h) * 65:
                                             (j * H + h) * 65 + 65],
                                    rhs=rsl,
                                    start=(kk == 0), stop=(kk == len(js) - 1))
                        dinv = dvp.tile([1, 512], BF16, tag="dinv",
                                        name=f"dinv{qg}")
                        nc.vector.reciprocal(dinv[:, 0:gw], psc[64:65, 0:gw])
                        dnb = dvp.tile([P, 512], BF16, tag="dnb",
                                       name=f"dnb{qg}")
                        nc.gpsimd.partition_broadcast(dnb[0:64, 0:gw],
                                                      dinv[:, 0:gw])
                        nc.vector.tensor_mul(
                            ctxc[po:po + 64,
                                 dch * ptl + qg * 512:dch * ptl + qg * 512 + gw],
                            psc[0:64, 0:gw], dnb[0:64, 0:gw])

                ests = {}
                gi1 = 2 * l if use_lng else None
                for ci, (t0, cl) in enumerate(chs):
                    ln_chunk(gi1, xh, t0, cl)
                    v_tg(0, ci)
                kq_block(0)
                for oc in range(1, DC):
                    if oc == 5:
                        for ci in range(len(chs)):
                            v_tg(1, ci)
                    head_scores(2 * oc - 2)
                    head_scores(2 * oc - 1)
                    kq_block(oc)
                    head_ctx(2 * oc - 2)
                    head_ctx(2 * oc - 1)
                head_scores(14)
                head_scores(15)
                head_ctx(14)
                head_ctx(15)

                # ---- O-projection (chunk-outer) + residual + LN2 ----
                xh = xhp.tile([P, DC * ptl], BF16, tag="xh", name=f"xh{l}b")
                gi2 = 2 * l + 1 if use_lng else None
                for ci, (t0, cl) in enumerate(chs):
                    for do_ in range(DC):
                        ocb = wcb.tile([P, DC * 128], BF16, tag="w",
                                       name=f"ocb{ci}_{do_}")
                        nc.sync.dma_start(
                            out=ocb, in_=ocb_d[l][:, do_ * D:(do_ + 1) * D])
                        ps = pp.tile([P, 512], F32, tag="mm", name=f"pso{do_}")
                        for dc in range(DC):
                            nc.tensor.matmul(
                                ps[:, 0:cl], lhsT=ocb[:, dc * 128:dc * 128 + 128],
                                rhs=ctxc[:, dc * ptl + t0:dc * ptl + t0 + cl],
                                start=(dc == 0), stop=(dc == DC - 1))
                        hsl = hT[:, do_ * ptl + t0:do_ * ptl + t0 + cl]
                        nc.vector.scalar_tensor_tensor(
                            hsl, ps[:, 0:cl], bcol(l, 3)[:, do_:do_ + 1], hsl,
                            op0=OP.add, op1=OP.add)
                    ln_chunk(gi2, xh, t0, cl)

                # ---- FFN ----
                Us = [big.tile([P, 8 * ptl], BF16, tag="b18", name=f"U{l}_{i}")
                      for i in range(4)]

                def usl(fc, t0, cl):
                    t = Us[fc // 8]
                    k = fc % 8
                    return t[:, k * ptl + t0:k * ptl + t0 + cl]

                for fcb in range(8):
                    w1cb = wcb.tile([P, DC * 512], BF16, tag="w",
                                    name=f"w1cb{fcb}")
                    nc.sync.dma_start(
                        out=w1cb,
                        in_=w1cb_d[l][:, fcb * DC * 512:(fcb + 1) * DC * 512])
                    for fc2 in range(4):
                        fc = fcb * 4 + fc2
                        for (t0, cl) in chs:
                            ps = pp.tile([P, 512], F32, tag="mm",
                                         name=f"psf{fc2}")
                            for dc in range(DC):
                                nc.tensor.matmul(
                                    ps[:, 0:cl],
                                    lhsT=w1cb[:, dc * 512 + fc2 * 128:
                                              dc * 512 + fc2 * 128 + 128],
                                    rhs=xh[:, dc * ptl + t0:dc * ptl + t0 + cl],
                                    start=(dc == 0), stop=(dc == DC - 1))
                            bidx = 5 + fc // 8
                            nc.scalar.activation(
                                usl(fc, t0, cl), ps[:, 0:cl], AF.Gelu,
                                bias=bcol(l, bidx)[:, fc % 8:fc % 8 + 1])
                for do_ in range(DC):
                    w2cb = wcb.tile([P, FC * 128], BF16, tag="w",
                                    name=f"w2cb{do_}")
                    nc.sync.dma_start(
                        out=w2cb,
                        in_=w2cb_d[l][:, do_ * FC * 128:(do_ + 1) * FC * 128])
                    for (t0, cl) in chs:
                        ps = pp.tile([P, 512], F32, tag="mm", name=f"psh{do_}")
                        for fc in range(FC):
                            nc.tensor.matmul(
                                ps[:, 0:cl],
                                lhsT=w2cb[:, fc * 128:fc * 128 + 128],
                                rhs=usl(fc, t0, cl),
                                start=(fc == 0), stop=(fc == FC - 1))
                        hsl = hT[:, do_ * ptl + t0:do_ * ptl + t0 + cl]
                        nc.vector.scalar_tensor_tensor(
                            hsl, ps[:, 0:cl], bcol(l, 4)[:, do_:do_ + 1], hsl,
                            op0=OP.add, op1=OP.add)
                    if l == L - 1:
                        nc.sync.dma_start(
                            out=houtT[:, do_ * ptl:(do_ + 1) * ptl],
                            in_=hT[:, do_ * ptl:(do_ + 1) * ptl])

    nc.compile()
    return nc


_NC_CACHE = {}


def _get_nc(lt=1032, nt=9, use_lng=False):
    key = (lt, nt, use_lng)
    if key not in _NC_CACHE:
        _NC_CACHE[key] = _build(lt, nt, use_lng)
    return _NC_CACHE[key]


def _pack_shared(inputs, lt, nt, use_lng):
    bf = np.dtype("bfloat16") if hasattr(np, "bfloat16") else None
    import ml_dtypes
    BFD = ml_dtypes.bfloat16

    def b16(x):
        return np.ascontiguousarray(np.asarray(x, np.float32).astype(BFD))

    tok = np.asarray(inputs["tok_emb"], np.float32)
    tokp = np.zeros((VP, D), np.float32)
    tokp[:tok.shape[0]] = tok
    tokemb = b16(tokp.reshape(VC, P, D).transpose(1, 0, 2).reshape(P, VC * D))

    shared = {"tokemb": tokemb}
    for l in range(L):
        Wq = np.asarray(inputs["Wq"][l], np.float32)
        Wk = np.asarray(inputs["Wk"][l], np.float32)
        Wv = np.asarray(inputs["Wv"][l], np.float32)
        Wo = np.asarray(inputs["Wo"][l], np.float32)
        W1 = np.asarray(inputs["W1"][l], np.float32)
        W2 = np.asarray(inputs["W2"][l], np.float32)

        def colblocks(W, ocn):  # [D, D] -> [P, ocn*DC*128]
            # block (oc): [p, dc, c] = W[dc*128+p, oc*128+c]
            Wr = W.reshape(DC, P, ocn, 128)  # [dc, p, oc, c]
            return np.ascontiguousarray(
                Wr.transpose(1, 2, 0, 3).reshape(P, ocn * DC * 128))

        shared[f"kcb{l}"] = b16(colblocks(Wk, DC))
        shared[f"qcb{l}"] = b16(colblocks(Wq, DC))
        shared[f"ocb{l}"] = b16(colblocks(Wo, DC))
        # vrb: [p, nh, dc, c] = Wv[dc*128+p, nh*512+c]
        Wvr = Wv.reshape(DC, P, 2, 512)
        shared[f"vrb{l}"] = b16(
            Wvr.transpose(1, 2, 0, 3).reshape(P, 2 * DC * 512))
        # w1cb: [p, fcb, dc, c] = W1[dc*128+p, fcb*512+c]
        W1r = W1.reshape(DC, P, 8, 512)
        shared[f"w1cb{l}"] = b16(
            W1r.transpose(1, 2, 0, 3).reshape(P, 8 * DC * 512))
        # w2cb: [p, do, fc, c] = W2[fc*128+p, do*128+c]
        W2r = W2.reshape(FC, P, DC, 128)
        shared[f"w2cb{l}"] = b16(
            W2r.transpose(1, 2, 0, 3).reshape(P, DC * FC * 128))

    cbw = np.zeros((P, 2 + 96 * L), np.float32)
    cbw[:, 0] = 1.0
    cbw[0, 1] = EPS
    for l in range(L):
        c0 = 2 + 96 * l
        # bv is folded into bo: probs sum to 1, so ctx@Wo + bo with V+bv
        # equals (ctx from plain V)@Wo + (bo + bv@Wo).
        bo_eff = (np.asarray(inputs["bo"][l], np.float32)
                  + np.asarray(inputs["bv"][l], np.float32)
                  @ np.asarray(inputs["Wo"][l], np.float32))
        vals = {"bq": np.asarray(inputs["bq"][l], np.float32),
                "bk": np.asarray(inputs["bk"][l], np.float32),
                "bv": np.zeros(D, np.float32),
                "bo": bo_eff,
                "b2": np.asarray(inputs["b2"][l], np.float32)}
        for i, key in enumerate(("bq", "bk", "bv", "bo", "b2")):
            cbw[:, c0 + 8 * i:c0 + 8 * i + 8] = vals[key].reshape(DC, P).T
        b1v = np.asarray(inputs["b1"][l], np.float32)
        cbw[:, c0 + 40:c0 + 72] = b1v.reshape(FC, P).T
    shared["cb"] = np.ascontiguousarray(cbw)

    if use_lng:
        gb = np.zeros((P, 8 * (2 + 4 * L)), np.float32)
        # group 0: ln0 (handled as gi=None in build... keep identity)
        idx = 0
        for l in range(L):
            for which in range(2):
                gi = 2 * l + which
                g = np.asarray(inputs["ln1_g" if which == 0 else "ln2_g"][l],
                               np.float32)
                bb = np.asarray(inputs["ln1_b" if which == 0 else "ln2_b"][l],
                                np.float32)
                gb[:, 8 * (2 * gi):8 * (2 * gi) + 8] = g.reshape(DC, P).T
                gb[:, 8 * (2 * gi + 1):8 * (2 * gi + 1) + 8] = bb.reshape(DC, P).T
        shared["lngb"] = np.ascontiguousarray(gb)
    return shared


def _prep_core(inputs, b, start, n, lt, nt):
    import ml_dtypes
    BFD = ml_dtypes.bfloat16
    ptl = nt * P

    def b16(x):
        return np.ascontiguousarray(np.asarray(x, np.float32).astype(BFD))

    ids = np.asarray(inputs["input_ids"][b, start:start + n])
    pid = np.asarray(inputs["patch_ids"][b, start:start + n]).astype(np.int64)
    pos_emb = np.asarray(inputs["pos_emb"], np.float32)
    hashes = np.asarray(inputs["hash_embeddings"], np.float32)

    oh = np.zeros((VP, ptl), np.float32)
    oh[ids, np.arange(n)] = 1.0
    oht = b16(oh.reshape(VC, P, ptl).transpose(1, 0, 2).reshape(P, VC * ptl))

    base = np.zeros((ptl, D), np.float32)
    base[:n] = pos_emb[start:start + n] + hashes[b, start:start + n]
    baseT = np.ascontiguousarray(
        base.reshape(ptl, DC, P).transpose(2, 1, 0).reshape(P, DC * ptl))

    pidp = np.empty(ptl, np.int64)
    pidp[:n] = pid
    pidp[n:] = -np.arange(1, ptl - n + 1)

    m = np.zeros((nt, P, 384), np.float32)
    for j in range(nt):
        w0 = np.clip(j - 1, 0, nt - 3) * P
        kk = pidp[j * P:(j + 1) * P]
        qq = pidp[w0:w0 + 384]
        m[j] = (kk[:, None] == qq[None, :]).astype(np.float32)
    masks = b16(m.transpose(1, 0, 2).reshape(P, nt * 384))
    return {"oht": oht, "baseT": baseT, "masks": masks}


def kernel(**inputs):
    pid_all = np.asarray(inputs["patch_ids"])

    shards = []
    for b in range(B):
        pid = np.asarray(pid_all[b])
        bnd = np.nonzero(pid[1:] != pid[:-1])[0] + 1
        cand = bnd[(bnd >= S - 1152) & (bnd <= 1152)]
        if len(cand) == 0:
            raise RuntimeError("no patch boundary near S/2; cannot shard")
        s = int(cand[np.argmin(np.abs(cand - S // 2))])
        shards.append((b, 0, s))
        shards.append((b, s, S - s))

    lt = max(n for _, _, n in shards)
    lt = max(lt, 1026)  # floor so chunk 3 isn't degenerate-tiny
    nt = (lt + P - 1) // P

    use_lng = not (
        all(np.all(np.asarray(inputs[k]) == 1.0)
            for k in ("ln0_g", "ln1_g", "ln2_g")) and
        all(np.all(np.asarray(inputs[k]) == 0.0)
            for k in ("ln0_b", "ln1_b", "ln2_b")))
    if use_lng:
        raise NotImplementedError(
            "non-identity LN affine not supported in fast path")

    shared = _pack_shared(inputs, lt, nt, use_lng)
    in_maps = []
    for b, start, n in shards:
        mcore = dict(shared)
        mcore.update(_prep_core(inputs, b, start, n, lt, nt))
        in_maps.append(mcore)

    nc = _get_nc(lt, nt, use_lng)
    res = bass_utils.run_bass_kernel_spmd(nc, in_maps,
                                          core_ids=list(range(NCORES)))

    ptl = nt * P
    out = np.zeros((B, S, D), np.float32)
    for i, (b, start, n) in enumerate(shards):
        ht = res.results[i]["houtT"]
        hfull = ht.reshape(P, DC, ptl).transpose(2, 1, 0).reshape(ptl, D)
        out[b, start:start + n] = hfull[:n]
    return out


if __name__ == "__main__":
    import sys
    lt = int(sys.argv[1]) if len(sys.argv) > 1 else 1032
    _get_nc(lt, (lt + P - 1) // P, False)
    print("built ok")


# revision 18
# speedup vs baseline: 1.2517x; 1.0349x over previous
"""BLT local encoder (2-layer transformer, patch-equality block-diagonal attention)
on 8 Trainium2 NeuronCores.

v2. Sharding: each of the 4 sequences splits at a patch-run boundary nearest
S/2 -> 8 independent shards, one per core, zero cross-core communication.

Kernel design (per core, L_tok = max shard length ~1032):
- Residual hT kept float32 feature-major [P, 8dc x PTL]; everything else bf16.
- Weights prepacked host-side into SBUF-ready bf16 col/row blocks, streamed
  once per layer (no restreaming), double-buffered.
- One LayerNorm per sublayer, output xh bf16 reused by Q, K and V.
- Full-shard attention: per (head, key-tile j) one score matmul with moving
  dim >= 256; softmax denominator via a ones-column appended to V (row 64 of
  the ctx psum); per-head normalize fused into the psum->SBUF copy.
- Engine split: PE matmuls; DVE normalize/copies/masks; Act square/exp/gelu;
  Pool partition-broadcasts + residual adds.
"""

import numpy as np

import concourse.bass as bass
import concourse.tile as tile
from concourse import bacc, bass_utils, mybir

F32 = mybir.dt.float32
F32R = mybir.dt.float32r
BF16 = mybir.dt.bfloat16
AF = mybir.ActivationFunctionType
OP = mybir.AluOpType

B, S, D, H, F, L = 4, 2048, 1024, 16, 4096, 2
DH = D // H      # 64
DC = D // 128    # 8
FC = F // 128    # 32
EPS = 1e-5
SCALE = 1.0 / np.sqrt(DH)
P = 128
VP = 384         # vocab 260 padded
VC = VP // 128   # 3
NCORES = 8


def _chunks(lt):
    out = []
    o = 0
    while o < lt:
        c = min(512, lt - o)
        out.append((o, c))
        o += c
    return out


def _build(lt, nt, use_lng):
    """lt: tokens per shard; nt: token tiles; use_lng: emit ln gamma/beta ops."""
    ptl = nt * P
    chs = _chunks(lt)
    nc = bacc.Bacc("TRN2", target_bir_lowering=False, debug=False,
                   num_devices=NCORES)

    def din(name, shape, dt=BF16):
        return nc.dram_tensor(name, shape, dt, kind="ExternalInput").ap()

    oht = din("oht", [P, VC * ptl])
    tokemb_d = din("tokemb", [P, VC * D])
    baseT = din("baseT", [P, DC * ptl], F32R)
    masks_d = din("masks", [P, nt * 384])
    # prepacked weights
    kcb_d, qcb_d, ocb_d, vrb_d, w1cb_d, w2cb_d = [], [], [], [], [], []
    for l in range(L):
        kcb_d.append(din(f"kcb{l}", [P, DC * DC * 128]))
        qcb_d.append(din(f"qcb{l}", [P, DC * DC * 128]))
        ocb_d.append(din(f"ocb{l}", [P, DC * DC * 128]))
        vrb_d.append(din(f"vrb{l}", [P, DC * D]))
        w1cb_d.append(din(f"w1cb{l}", [P, 8 * DC * 512]))
        w2cb_d.append(din(f"w2cb{l}", [P, DC * FC * 128]))
    # packed per-feature consts: [P, col] layout, 8 cols per D-vector
    # cols: 0 ones | 1 eps(row0) | then per layer l at 2+64*l:
    #   bq 0:8 bk 8:16 bv 16:24 bo 24:32 b2 32:40 b1 40:72 (unused gap)
    # ln g/b (if use_lng): separate tensor lngb
    cb_d = din("cb", [P, 2 + 96 * L], F32)
    lngb_d = din("lngb", [P, 8 * (2 + 4 * L)], F32) if use_lng else None
    houtT = nc.dram_tensor("houtT", [P, DC * ptl], F32R,
                           kind="ExternalOutput").ap()

    with tile.TileContext(nc) as tc:
        with (
            nc.allow_low_precision(
                reason="bf16 softmax/LN staging validated vs reference"),
            tc.tile_pool(name="pers", bufs=1) as pers,
            tc.tile_pool(name="big", bufs=4) as big,
            tc.tile_pool(name="xhp", bufs=1) as xhp,
            tc.tile_pool(name="wcb", bufs=3) as wcb,
            tc.tile_pool(name="est", bufs=2) as estp,
            tc.tile_pool(name="lnt", bufs=3) as lnp,
            tc.tile_pool(name="sm", bufs=2) as smp,
            tc.tile_pool(name="dv", bufs=2) as dvp,
            tc.tile_pool(name="pp", bufs=8, space="PSUM") as pp,
        ):
            ohsb = wcb.tile([P, VC * ptl], BF16, tag="w", name="ohsb")
            nc.sync.dma_start(out=ohsb, in_=oht)
            tesb = wcb.tile([P, VC * D], BF16, tag="w", name="tesb")
            nc.sync.dma_start(out=tesb, in_=tokemb_d)
            cb = pers.tile([P, 2 + 96 * L], F32, tag="cb")
            nc.sync.dma_start(out=cb, in_=cb_d)
            eps_t = cb[0:1, 1:2]
            ones_r = pers.tile([P, 1], F32R, tag="ones_r")
            nc.vector.tensor_copy(ones_r, cb[:, 0:1])
            ones_b = pers.tile([P, 1], BF16, tag="ones_b")
            nc.vector.tensor_copy(ones_b, cb[:, 0:1])
            if use_lng:
                lngb = pers.tile([P, 8 * (2 + 4 * L)], F32, tag="lngb")
                nc.sync.dma_start(out=lngb, in_=lngb_d)

            masks = pers.tile([P, nt * 384], BF16, tag="masks")
            nc.sync.dma_start(out=masks, in_=masks_d)

            hT = pers.tile([P, DC * ptl], F32R, tag="hT")

            def bcol(l, i):  # bias col i (in 8-col groups) for layer l
                c0 = 2 + 96 * l + 8 * i
                return cb[:, c0:c0 + 8]

            def ln_stats(rms, ci, t0, cl):
                ps1 = pp.tile([1, 512], F32, tag="mm", name="lns1")
                ps2 = pp.tile([1, 512], F32, tag="mm", name="lns2")
                for dc in range(DC):
                    hsl = hT[:, dc * ptl + t0:dc * ptl + t0 + cl]
                    sq = lnp.tile([P, 512], BF16, tag="sq", name=f"sq{dc}")
                    nc.scalar.activation(sq[:, 0:cl], hsl, AF.Square)
                    nc.tensor.matmul(ps1[:, 0:cl], lhsT=ones_r, rhs=hsl,
                                     start=(dc == 0), stop=(dc == DC - 1))
                    nc.tensor.matmul(ps2[:, 0:cl], lhsT=ones_b,
                                     rhs=sq[:, 0:cl],
                                     start=(dc == 0), stop=(dc == DC - 1))
                st = smp.tile([P, 2 * 512], F32, tag="st", name="st")
                stb = smp.tile([P, 2 * 512], BF16, tag="stb", name="stb")
                mean = st[0:1, 0:cl]
                var = st[0:1, 512:512 + cl]
                rstd = stb[0:1, 0:cl]
                mr = stb[0:1, 512:512 + cl]
                nc.vector.tensor_scalar_mul(mean, ps1[:, 0:cl], 1.0 / D)
                nc.vector.tensor_mul(var, mean, mean)
                nc.vector.scalar_tensor_tensor(
                    var, ps2[:, 0:cl], 1.0 / D, var,
                    op0=OP.mult, op1=OP.subtract)
                nc.scalar.activation(var, var, AF.Sqrt, bias=eps_t)
                nc.vector.reciprocal(rstd, var)
                nc.vector.tensor_mul(mr, mean, rstd)
                RM = dvp.tile([P, 2 * 512], BF16, tag="rm", name="RM", bufs=3)
                nc.gpsimd.partition_broadcast(RM[:, 0:cl], rstd)
                nc.gpsimd.partition_broadcast(RM[:, 512:512 + cl], mr)
                rms[ci] = RM

            def ln_norm(rms, gi, out_tile, ci, t0, cl):
                RM = rms[ci]
                for dc in range(DC):
                    hsl = hT[:, dc * ptl + t0:dc * ptl + t0 + cl]
                    d1 = lnp.tile([P, 512], BF16, tag="d1", name=f"d1_{dc}")
                    nc.vector.tensor_mul(d1[:, 0:cl], hsl, RM[:, 0:cl])
                    osl = out_tile[:, dc * ptl + t0:dc * ptl + t0 + cl]
                    if use_lng and gi is not None:
                        d2 = lnp.tile([P, 512], BF16, tag="d2",
                                      name=f"d2_{dc}")
                        nc.vector.tensor_sub(d2[:, 0:cl], d1[:, 0:cl],
                                             RM[:, 512:512 + cl])
                        g0 = 8 * (2 * gi)
                        nc.vector.tensor_scalar(
                            osl, d2[:, 0:cl],
                            lngb[:, g0 + dc:g0 + dc + 1],
                            lngb[:, g0 + 8 + dc:g0 + 8 + dc + 1],
                            op0=OP.mult, op1=OP.add)
                    else:
                        nc.vector.tensor_sub(osl, d1[:, 0:cl],
                                             RM[:, 512:512 + cl])

            def ln_chunk(gi, out_tile, t0, cl):
                rms = {}
                ln_stats(rms, 0, t0, cl)
                ln_norm(rms, gi, out_tile, 0, t0, cl)

            def ln_pass(gi, out_tile):
                for (t0, cl) in chs:
                    ln_chunk(gi, out_tile, t0, cl)

            # ---------- embeddings ----------
            for dc in range(DC):
                nc.sync.dma_start(out=hT[:, dc * ptl:(dc + 1) * ptl],
                                  in_=baseT[:, dc * ptl:(dc + 1) * ptl])
            rms0 = {}
            for ci, (t0, cl) in enumerate(chs):
                for dc in range(DC):
                    pse = pp.tile([P, 512], F32, tag="mm", name="pse")
                    for vc in range(VC):
                        nc.tensor.matmul(
                            pse[:, 0:cl],
                            lhsT=tesb[:, vc * D + dc * 128:vc * D + dc * 128 + 128],
                            rhs=ohsb[:, vc * ptl + t0:vc * ptl + t0 + cl],
                            start=(vc == 0), stop=(vc == VC - 1))
                    hsl = hT[:, dc * ptl + t0:dc * ptl + t0 + cl]
                    nc.vector.tensor_add(hsl, pse[:, 0:cl], hsl)
                ln_stats(rms0, ci, t0, cl)
            for ci, (t0, cl) in enumerate(chs):
                ln_norm(rms0, None, hT, ci, t0, cl)   # LN0 in place

            # ---------- layers ----------
            for l in range(L):
                xh = xhp.tile([P, DC * ptl], BF16, tag="xh", name=f"xh{l}a")

                # ---- K/Q/V + attention, interleaved ----
                KT = big.tile([P, DC * ptl], BF16, tag="b18", name=f"KT{l}")
                Vsb = big.tile([P, nt * H * 65], BF16, tag="b18", name=f"Vsb{l}")
                QT = big.tile([P, DC * ptl], BF16, tag="b18", name=f"QT{l}")
                ctxc = big.tile([P, DC * ptl], BF16, tag="b18", name=f"ctx{l}")
                if lt < ptl:
                    nc.vector.memset(
                        Vsb[:, (nt - 1) * H * 65:nt * H * 65], 0.0)
                ones_v = Vsb.rearrange("p (g x) -> p g x", x=65)[:, :, 64:65]
                nc.vector.memset(ones_v, 1.0)

                def v_tg(nh, tg):
                    if True:
                        tts = [t for t in range(4 * tg, min(4 * tg + 4, nt))
                               if lt - t * P > 0]
                        pvs = {}
                        for tt in tts:
                            pvs[tt] = pp.tile([P, 512], F32, tag="mm",
                                              name=f"psv{tt}_{nh}")
                        for dc in range(DC):
                            vrb = wcb.tile([P, 512], BF16, tag="w",
                                           name=f"vrb{nh}_{tg}_{dc}")
                            nc.sync.dma_start(
                                out=vrb,
                                in_=vrb_d[l][:, (nh * DC + dc) * 512:
                                             (nh * DC + dc + 1) * 512])
                            for tt in tts:
                                tl = min(P, lt - tt * P)
                                nc.tensor.matmul(
                                    pvs[tt][0:tl, :],
                                    lhsT=xh[:, dc * ptl + tt * P:dc * ptl + tt * P + tl],
                                    rhs=vrb,
                                    start=(dc == 0), stop=(dc == DC - 1))
                        for tt in tts:
                            tl = min(P, lt - tt * P)
                            pv = pvs[tt][0:tl, :].rearrange(
                                "p (h x) -> p h x", h=8)
                            ov = Vsb[0:tl, (tt * H + nh * 8) * 65:
                                     (tt * H + nh * 8 + 8) * 65].rearrange(
                                "p (h x) -> p h x", x=65)[:, :, 0:64]
                            nc.scalar.copy(ov, pv)

                def kq_block(oc):
                    kcb = wcb.tile([P, DC * 128], BF16, tag="w",
                                   name=f"kcb{oc}")
                    nc.sync.dma_start(
                        out=kcb, in_=kcb_d[l][:, oc * D:(oc + 1) * D])
                    for (t0, cl) in chs:
                        ps = pp.tile([P, 512], F32, tag="mm", name=f"psk{oc}")
                        for dc in range(DC):
                            nc.tensor.matmul(
                                ps[:, 0:cl],
                                lhsT=kcb[:, dc * 128:dc * 128 + 128],
                                rhs=xh[:, dc * ptl + t0:dc * ptl + t0 + cl],
                                start=(dc == 0), stop=(dc == DC - 1))
                        nc.vector.tensor_scalar_add(
                            KT[:, oc * ptl + t0:oc * ptl + t0 + cl],
                            ps[:, 0:cl], bcol(l, 1)[:, oc:oc + 1])
                    qcb = wcb.tile([P, DC * 128], BF16, tag="w",
                                   name=f"qcb{oc}")
                    nc.sync.dma_start(
                        out=qcb, in_=qcb_d[l][:, oc * D:(oc + 1) * D])
                    for (t0, cl) in chs:
                        ps = pp.tile([P, 512], F32, tag="mm", name=f"psq{oc}")
                        for dc in range(DC):
                            nc.tensor.matmul(
                                ps[:, 0:cl],
                                lhsT=qcb[:, dc * 128:dc * 128 + 128],
                                rhs=xh[:, dc * ptl + t0:dc * ptl + t0 + cl],
                                start=(dc == 0), stop=(dc == DC - 1))
                        nc.vector.tensor_scalar_add(
                            QT[:, oc * ptl + t0:oc * ptl + t0 + cl],
                            ps[:, 0:cl], bcol(l, 0)[:, oc:oc + 1])
                    if lt < ptl:
                        nc.vector.memset(KT[:, oc * ptl + lt:(oc + 1) * ptl],
                                         0.0)
                        nc.vector.memset(QT[:, oc * ptl + lt:(oc + 1) * ptl],
                                         0.0)

                def head_scores(h):
                    dch, po = h // 2, (h % 2) * 64
                    est = estp.tile([P, nt * 384], BF16, tag="est",
                                    name=f"est{h}")
                    ests[h] = est
                    for j in range(nt):
                        lo = max(j - 1, 0)
                        hi = min(j + 1, nt - 1)
                        nq = (hi - lo + 1) * P
                        w0 = min(max(j - 1, 0), nt - 3)
                        pst = pp.tile([P, 384], F32, tag="mm", name=f"pst{j}")
                        nc.tensor.matmul(
                            pst[:, 0:nq],
                            lhsT=KT[po:po + 64, dch * ptl + j * P:dch * ptl + j * P + P],
                            rhs=QT[po:po + 64, dch * ptl + lo * P:dch * ptl + lo * P + nq],
                            start=True, stop=True)
                        esl = est[:, j * 384 + (lo - w0) * P:
                                  j * 384 + (lo - w0) * P + nq]
                        nc.scalar.activation(esl, pst[:, 0:nq], AF.Exp,
                                             scale=float(SCALE))
                    nc.vector.tensor_mul(est, est, masks)

                def head_ctx(h):
                    dch, po = h // 2, (h % 2) * 64
                    est = ests[h]
                    for qg in range((nt + 3) // 4):
                        qts = [q for q in range(4 * qg, min(4 * qg + 4, nt))]
                        gw = len(qts) * P
                        psc = pp.tile([65, 512], F32, tag="mm", name=f"psc{qg}")
                        for qi, qt in enumerate(qts):
                            js = [j for j in (qt - 1, qt, qt + 1)
                                  if 0 <= j < nt]
                            for kk, j in enumerate(js):
                                w0 = min(max(j - 1, 0), nt - 3)
                                rsl = est[:, j * 384 + (qt - w0) * P:
                                          j * 384 + (qt - w0) * P + P]
                                nc.tensor.matmul(
                                    psc[:, qi * P:(qi + 1) * P],
                                    lhsT=Vsb[:, (j * H + h) * 65:
                                             (j * H + h) * 65 + 65],
                                    rhs=rsl,
                                    start=(kk == 0), stop=(kk == len(js) - 1))
                        dinv = dvp.tile([1, 512], BF16, tag="dinv",
                                        name=f"dinv{qg}")
                        nc.vector.reciprocal(dinv[:, 0:gw], psc[64:65, 0:gw])
                        dnb = dvp.tile([P, 512], BF16, tag="dnb",
                                       name=f"dnb{qg}")
                        nc.gpsimd.partition_broadcast(dnb[0:64, 0:gw],
                                                      dinv[:, 0:gw])
                        nc.vector.tensor_mul(
                            ctxc[po:po + 64,
                                 dch * ptl + qg * 512:dch * ptl + qg * 512 + gw],
                            psc[0:64, 0:gw], dnb[0:64, 0:gw])

                ests = {}
                gi1 = 2 * l if use_lng else None
                rms1 = {}
                for ci, (t0, cl) in enumerate(chs):
                    ln_stats(rms1, ci, t0, cl)
                for ci, (t0, cl) in enumerate(chs):
                    ln_norm(rms1, gi1, xh, ci, t0, cl)
                    v_tg(0, ci)
                kq_block(0)
                for oc in range(1, DC):
                    if oc == 5:
                        for ci in range(len(chs)):
                            v_tg(1, ci)
                    head_scores(2 * oc - 2)
                    head_scores(2 * oc - 1)
                    kq_block(oc)
                    head_ctx(2 * oc - 2)
                    head_ctx(2 * oc - 1)
                head_scores(14)
                head_scores(15)
                head_ctx(14)
                head_ctx(15)

                # ---- O-projection (chunk-outer) + residual + LN2 ----
                xh = xhp.tile([P, DC * ptl], BF16, tag="xh", name=f"xh{l}b")
                gi2 = 2 * l + 1 if use_lng else None
                rms2 = {}
                for ci, (t0, cl) in enumerate(chs):
                    for do_ in range(DC):
                        ocb = wcb.tile([P, DC * 128], BF16, tag="w",
                                       name=f"ocb{ci}_{do_}")
                        nc.sync.dma_start(
                            out=ocb, in_=ocb_d[l][:, do_ * D:(do_ + 1) * D])
                        ps = pp.tile([P, 512], F32, tag="mm", name=f"pso{do_}")
                        for dc in range(DC):
                            nc.tensor.matmul(
                                ps[:, 0:cl], lhsT=ocb[:, dc * 128:dc * 128 + 128],
                                rhs=ctxc[:, dc * ptl + t0:dc * ptl + t0 + cl],
                                start=(dc == 0), stop=(dc == DC - 1))
                        hsl = hT[:, do_ * ptl + t0:do_ * ptl + t0 + cl]
                        nc.vector.scalar_tensor_tensor(
                            hsl, ps[:, 0:cl], bcol(l, 3)[:, do_:do_ + 1], hsl,
                            op0=OP.add, op1=OP.add)
                    ln_stats(rms2, ci, t0, cl)
                for ci, (t0, cl) in enumerate(chs):
                    ln_norm(rms2, gi2, xh, ci, t0, cl)

                # ---- FFN ----
                Us = [big.tile([P, 8 * ptl], BF16, tag="b18", name=f"U{l}_{i}")
                      for i in range(4)]

                def usl(fc, t0, cl):
                    t = Us[fc // 8]
                    k = fc % 8
                    return t[:, k * ptl + t0:k * ptl + t0 + cl]

                for fcb in range(8):
                    w1cb = wcb.tile([P, DC * 512], BF16, tag="w",
                                    name=f"w1cb{fcb}")
                    nc.sync.dma_start(
                        out=w1cb,
                        in_=w1cb_d[l][:, fcb * DC * 512:(fcb + 1) * DC * 512])
                    for fc2 in range(4):
                        fc = fcb * 4 + fc2
                        for (t0, cl) in chs:
                            ps = pp.tile([P, 512], F32, tag="mm",
                                         name=f"psf{fc2}")
                            for dc in range(DC):
                                nc.tensor.matmul(
                                    ps[:, 0:cl],
                                    lhsT=w1cb[:, dc * 512 + fc2 * 128:
                                              dc * 512 + fc2 * 128 + 128],
                                    rhs=xh[:, dc * ptl + t0:dc * ptl + t0 + cl],
                                    start=(dc == 0), stop=(dc == DC - 1))
                            bidx = 5 + fc // 8
                            nc.scalar.activation(
                                usl(fc, t0, cl), ps[:, 0:cl], AF.Gelu,
                                bias=bcol(l, bidx)[:, fc % 8:fc % 8 + 1])
                for do_ in range(DC):
                    w2cb = wcb.tile([P, FC * 128], BF16, tag="w",
                                    name=f"w2cb{do_}")
                    nc.sync.dma_start(
                        out=w2cb,
                        in_=w2cb_d[l][:, do_ * FC * 128:(do_ + 1) * FC * 128])
                    for (t0, cl) in chs:
                        ps = pp.tile([P, 512], F32, tag="mm", name=f"psh{do_}")
                        for fc in range(FC):
                            nc.tensor.matmul(
                                ps[:, 0:cl],
                                lhsT=w2cb[:, fc * 128:fc * 128 + 128],
                                rhs=usl(fc, t0, cl),
                                start=(fc == 0), stop=(fc == FC - 1))
                        hsl = hT[:, do_ * ptl + t0:do_ * ptl + t0 + cl]
                        nc.vector.scalar_tensor_tensor(
                            hsl, ps[:, 0:cl], bcol(l, 4)[:, do_:do_ + 1], hsl,
                            op0=OP.add, op1=OP.add)
                    if l == L - 1:
                        nc.sync.dma_start(
                            out=houtT[:, do_ * ptl:(do_ + 1) * ptl],
                            in_=hT[:, do_ * ptl:(do_ + 1) * ptl])

    nc.compile()
    return nc


_NC_CACHE = {}


def _get_nc(lt=1032, nt=9, use_lng=False):
    key = (lt, nt, use_lng)
    if key not in _NC_CACHE:
        _NC_CACHE[key] = _build(lt, nt, use_lng)
    return _NC_CACHE[key]


def _pack_shared(inputs, lt, nt, use_lng):
    bf = np.dtype("bfloat16") if hasattr(np, "bfloat16") else None
    import ml_dtypes
    BFD = ml_dtypes.bfloat16

    def b16(x):
        return np.ascontiguousarray(np.asarray(x, np.float32).astype(BFD))

    tok = np.asarray(inputs["tok_emb"], np.float32)
    tokp = np.zeros((VP, D), np.float32)
    tokp[:tok.shape[0]] = tok
    tokemb = b16(tokp.reshape(VC, P, D).transpose(1, 0, 2).reshape(P, VC * D))

    shared = {"tokemb": tokemb}
    for l in range(L):
        Wq = np.asarray(inputs["Wq"][l], np.float32)
        Wk = np.asarray(inputs["Wk"][l], np.float32)
        Wv = np.asarray(inputs["Wv"][l], np.float32)
        Wo = np.asarray(inputs["Wo"][l], np.float32)
        W1 = np.asarray(inputs["W1"][l], np.float32)
        W2 = np.asarray(inputs["W2"][l], np.float32)

        def colblocks(W, ocn):  # [D, D] -> [P, ocn*DC*128]
            # block (oc): [p, dc, c] = W[dc*128+p, oc*128+c]
            Wr = W.reshape(DC, P, ocn, 128)  # [dc, p, oc, c]
            return np.ascontiguousarray(
                Wr.transpose(1, 2, 0, 3).reshape(P, ocn * DC * 128))

        shared[f"kcb{l}"] = b16(colblocks(Wk, DC))
        shared[f"qcb{l}"] = b16(colblocks(Wq, DC))
        shared[f"ocb{l}"] = b16(colblocks(Wo, DC))
        # vrb: [p, nh, dc, c] = Wv[dc*128+p, nh*512+c]
        Wvr = Wv.reshape(DC, P, 2, 512)
        shared[f"vrb{l}"] = b16(
            Wvr.transpose(1, 2, 0, 3).reshape(P, 2 * DC * 512))
        # w1cb: [p, fcb, dc, c] = W1[dc*128+p, fcb*512+c]
        W1r = W1.reshape(DC, P, 8, 512)
        shared[f"w1cb{l}"] = b16(
            W1r.transpose(1, 2, 0, 3).reshape(P, 8 * DC * 512))
        # w2cb: [p, do, fc, c] = W2[fc*128+p, do*128+c]
        W2r = W2.reshape(FC, P, DC, 128)
        shared[f"w2cb{l}"] = b16(
            W2r.transpose(1, 2, 0, 3).reshape(P, DC * FC * 128))

    cbw = np.zeros((P, 2 + 96 * L), np.float32)
    cbw[:, 0] = 1.0
    cbw[0, 1] = EPS
    for l in range(L):
        c0 = 2 + 96 * l
        # bv is folded into bo: probs sum to 1, so ctx@Wo + bo with V+bv
        # equals (ctx from plain V)@Wo + (bo + bv@Wo).
        bo_eff = (np.asarray(inputs["bo"][l], np.float32)
                  + np.asarray(inputs["bv"][l], np.float32)
                  @ np.asarray(inputs["Wo"][l], np.float32))
        vals = {"bq": np.asarray(inputs["bq"][l], np.float32),
                "bk": np.asarray(inputs["bk"][l], np.float32),
                "bv": np.zeros(D, np.float32),
                "bo": bo_eff,
                "b2": np.asarray(inputs["b2"][l], np.float32)}
        for i, key in enumerate(("bq", "bk", "bv", "bo", "b2")):
            cbw[:, c0 + 8 * i:c0 + 8 * i + 8] = vals[key].reshape(DC, P).T
        b1v = np.asarray(inputs["b1"][l], np.float32)
        cbw[:, c0 + 40:c0 + 72] = b1v.reshape(FC, P).T
    shared["cb"] = np.ascontiguousarray(cbw)

    if use_lng:
        gb = np.zeros((P, 8 * (2 + 4 * L)), np.float32)
        # group 0: ln0 (handled as gi=None in build... keep identity)
        idx = 0
        for l in range(L):
            for which in range(2):
                gi = 2 * l + which
                g = np.asarray(inputs["ln1_g" if which == 0 else "ln2_g"][l],
                               np.float32)
                bb = np.asarray(inputs["ln1_b" if which == 0 else "ln2_b"][l],
                                np.float32)
                gb[:, 8 * (2 * gi):8 * (2 * gi) + 8] = g.reshape(DC, P).T
                gb[:, 8 * (2 * gi + 1):8 * (2 * gi + 1) + 8] = bb.reshape(DC, P).T
        shared["lngb"] = np.ascontiguousarray(gb)
    return shared


def _prep_core(inputs, b, start, n, lt, nt):
    import ml_dtypes
    BFD = ml_dtypes.bfloat16
    ptl = nt * P

    def b16(x):
        return np.ascontiguousarray(np.asarray(x, np.float32).astype(BFD))

    ids = np.asarray(inputs["input_ids"][b, start:start + n])
    pid = np.asarray(inputs["patch_ids"][b, start:start + n]).astype(np.int64)
    pos_emb = np.asarray(inputs["pos_emb"], np.float32)
    hashes = np.asarray(inputs["hash_embeddings"], np.float32)

    oh = np.zeros((VP, ptl), np.float32)
    oh[ids, np.arange(n)] = 1.0
    oht = b16(oh.reshape(VC, P, ptl).transpose(1, 0, 2).reshape(P, VC * ptl))

    base = np.zeros((ptl, D), np.float32)
    base[:n] = pos_emb[start:start + n] + hashes[b, start:start + n]
    baseT = np.ascontiguousarray(
        base.reshape(ptl, DC, P).transpose(2, 1, 0).reshape(P, DC * ptl))

    pidp = np.empty(ptl, np.int64)
    pidp[:n] = pid
    pidp[n:] = -np.arange(1, ptl - n + 1)

    m = np.zeros((nt, P, 384), np.float32)
    for j in range(nt):
        w0 = np.clip(j - 1, 0, nt - 3) * P
        kk = pidp[j * P:(j + 1) * P]
        qq = pidp[w0:w0 + 384]
        m[j] = (kk[:, None] == qq[None, :]).astype(np.float32)
    masks = b16(m.transpose(1, 0, 2).reshape(P, nt * 384))
    return {"oht": oht, "baseT": baseT, "masks": masks}


def kernel(**inputs):
    pid_all = np.asarray(inputs["patch_ids"])

    shards = []
    for b in range(B):
        pid = np.asarray(pid_all[b])
        bnd = np.nonzero(pid[1:] != pid[:-1])[0] + 1
        cand = bnd[(bnd >= S - 1152) & (bnd <= 1152)]
        if len(cand) == 0:
            raise RuntimeError("no patch boundary near S/2; cannot shard")
        s = int(cand[np.argmin(np.abs(cand - S // 2))])
        shards.append((b, 0, s))
        shards.append((b, s, S - s))

    lt = max(n for _, _, n in shards)
    lt = max(lt, 1026)  # floor so chunk 3 isn't degenerate-tiny
    nt = (lt + P - 1) // P

    use_lng = not (
        all(np.all(np.asarray(inputs[k]) == 1.0)
            for k in ("ln0_g", "ln1_g", "ln2_g")) and
        all(np.all(np.asarray(inputs[k]) == 0.0)
            for k in ("ln0_b", "ln1_b", "ln2_b")))
    if use_lng:
        raise NotImplementedError(
            "non-identity LN affine not supported in fast path")

    shared = _pack_shared(inputs, lt, nt, use_lng)
    in_maps = []
    for b, start, n in shards:
        mcore = dict(shared)
        mcore.update(_prep_core(inputs, b, start, n, lt, nt))
        in_maps.append(mcore)

    nc = _get_nc(lt, nt, use_lng)
    res = bass_utils.run_bass_kernel_spmd(nc, in_maps,
                                          core_ids=list(range(NCORES)))

    ptl = nt * P
    out = np.zeros((B, S, D), np.float32)
    for i, (b, start, n) in enumerate(shards):
        ht = res.results[i]["houtT"]
        hfull = ht.reshape(P, DC, ptl).transpose(2, 1, 0).reshape(ptl, D)
        out[b, start:start + n] = hfull[:n]
    return out


if __name__ == "__main__":
    import sys
    lt = int(sys.argv[1]) if len(sys.argv) > 1 else 1032
    _get_nc(lt, (lt + P - 1) // P, False)
    print("built ok")


# revision 19
# speedup vs baseline: 1.2864x; 1.0277x over previous
"""BLT local encoder (2-layer transformer, patch-equality block-diagonal attention)
on 8 Trainium2 NeuronCores.

v2. Sharding: each of the 4 sequences splits at a patch-run boundary nearest
S/2 -> 8 independent shards, one per core, zero cross-core communication.

Kernel design (per core, L_tok = max shard length ~1032):
- Residual hT kept float32 feature-major [P, 8dc x PTL]; everything else bf16.
- Weights prepacked host-side into SBUF-ready bf16 col/row blocks, streamed
  once per layer (no restreaming), double-buffered.
- One LayerNorm per sublayer, output xh bf16 reused by Q, K and V.
- Full-shard attention: per (head, key-tile j) one score matmul with moving
  dim >= 256; softmax denominator via a ones-column appended to V (row 64 of
  the ctx psum); per-head normalize fused into the psum->SBUF copy.
- Engine split: PE matmuls; DVE normalize/copies/masks; Act square/exp/gelu;
  Pool partition-broadcasts + residual adds.
"""

import numpy as np

import concourse.bass as bass
import concourse.tile as tile
from concourse import bacc, bass_utils, mybir

F32 = mybir.dt.float32
F32R = mybir.dt.float32r
BF16 = mybir.dt.bfloat16
AF = mybir.ActivationFunctionType
OP = mybir.AluOpType

B, S, D, H, F, L = 4, 2048, 1024, 16, 4096, 2
DH = D // H      # 64
DC = D // 128    # 8
FC = F // 128    # 32
EPS = 1e-5
SCALE = 1.0 / np.sqrt(DH)
P = 128
VP = 384         # vocab 260 padded
VC = VP // 128   # 3
NCORES = 8


def _chunks(lt):
    out = []
    o = 0
    while o < lt:
        c = min(512, lt - o)
        out.append((o, c))
        o += c
    return out


def _build(lt, nt, use_lng, w64):
    """lt: tokens; nt: tiles; use_lng: ln affine ops; w64: +-64-token window."""
    ptl = nt * P
    EW = 256 if w64 else 384
    chs = _chunks(lt)
    nc = bacc.Bacc("TRN2", target_bir_lowering=False, debug=False,
                   num_devices=NCORES)

    def din(name, shape, dt=BF16):
        return nc.dram_tensor(name, shape, dt, kind="ExternalInput").ap()

    oht = din("oht", [P, VC * ptl])
    tokemb_d = din("tokemb", [P, VC * D])
    baseT = din("baseT", [P, DC * ptl], F32R)
    masks_d = din("masks", [P, nt * EW])
    # prepacked weights
    kcb_d, qcb_d, ocb_d, vrb_d, w1cb_d, w2cb_d = [], [], [], [], [], []
    for l in range(L):
        kcb_d.append(din(f"kcb{l}", [P, DC * DC * 128]))
        qcb_d.append(din(f"qcb{l}", [P, DC * DC * 128]))
        ocb_d.append(din(f"ocb{l}", [P, DC * DC * 128]))
        vrb_d.append(din(f"vrb{l}", [P, DC * D]))
        w1cb_d.append(din(f"w1cb{l}", [P, 8 * DC * 512]))
        w2cb_d.append(din(f"w2cb{l}", [P, DC * FC * 128]))
    # packed per-feature consts: [P, col] layout, 8 cols per D-vector
    # cols: 0 ones | 1 eps(row0) | then per layer l at 2+64*l:
    #   bq 0:8 bk 8:16 bv 16:24 bo 24:32 b2 32:40 b1 40:72 (unused gap)
    # ln g/b (if use_lng): separate tensor lngb
    cb_d = din("cb", [P, 2 + 96 * L], F32)
    lngb_d = din("lngb", [P, 8 * (2 + 4 * L)], F32) if use_lng else None
    houtT = nc.dram_tensor("houtT", [P, DC * ptl], F32R,
                           kind="ExternalOutput").ap()

    with tile.TileContext(nc) as tc:
        with (
            nc.allow_low_precision(
                reason="bf16 softmax/LN staging validated vs reference"),
            tc.tile_pool(name="pers", bufs=1) as pers,
            tc.tile_pool(name="big", bufs=4) as big,
            tc.tile_pool(name="xhp", bufs=1) as xhp,
            tc.tile_pool(name="wcb", bufs=3) as wcb,
            tc.tile_pool(name="est", bufs=2) as estp,
            tc.tile_pool(name="lnt", bufs=3) as lnp,
            tc.tile_pool(name="sm", bufs=2) as smp,
            tc.tile_pool(name="dv", bufs=2) as dvp,
            tc.tile_pool(name="pp", bufs=8, space="PSUM") as pp,
        ):
            ohsb = wcb.tile([P, VC * ptl], BF16, tag="w", name="ohsb")
            nc.sync.dma_start(out=ohsb, in_=oht)
            tesb = wcb.tile([P, VC * D], BF16, tag="w", name="tesb")
            nc.sync.dma_start(out=tesb, in_=tokemb_d)
            cb = pers.tile([P, 2 + 96 * L], F32, tag="cb")
            nc.sync.dma_start(out=cb, in_=cb_d)
            eps_t = cb[0:1, 1:2]
            ones_r = pers.tile([P, 1], F32R, tag="ones_r")
            nc.vector.tensor_copy(ones_r, cb[:, 0:1])
            ones_b = pers.tile([P, 1], BF16, tag="ones_b")
            nc.vector.tensor_copy(ones_b, cb[:, 0:1])
            if use_lng:
                lngb = pers.tile([P, 8 * (2 + 4 * L)], F32, tag="lngb")
                nc.sync.dma_start(out=lngb, in_=lngb_d)

            masks = pers.tile([P, nt * EW], BF16, tag="masks")
            nc.sync.dma_start(out=masks, in_=masks_d)

            hT = pers.tile([P, DC * ptl], F32R, tag="hT")

            def bcol(l, i):  # bias col i (in 8-col groups) for layer l
                c0 = 2 + 96 * l + 8 * i
                return cb[:, c0:c0 + 8]

            def ln_stats(rms, ci, t0, cl):
                ps1 = pp.tile([1, 512], F32, tag="mm", name="lns1")
                ps2 = pp.tile([1, 512], F32, tag="mm", name="lns2")
                for dc in range(DC):
                    hsl = hT[:, dc * ptl + t0:dc * ptl + t0 + cl]
                    sq = lnp.tile([P, 512], BF16, tag="sq", name=f"sq{dc}")
                    nc.scalar.activation(sq[:, 0:cl], hsl, AF.Square)
                    nc.tensor.matmul(ps1[:, 0:cl], lhsT=ones_r, rhs=hsl,
                                     start=(dc == 0), stop=(dc == DC - 1))
                    nc.tensor.matmul(ps2[:, 0:cl], lhsT=ones_b,
                                     rhs=sq[:, 0:cl],
                                     start=(dc == 0), stop=(dc == DC - 1))
                st = smp.tile([P, 2 * 512], F32, tag="st", name="st")
                stb = smp.tile([P, 2 * 512], BF16, tag="stb", name="stb")
                mean = st[0:1, 0:cl]
                var = st[0:1, 512:512 + cl]
                rstd = stb[0:1, 0:cl]
                mr = stb[0:1, 512:512 + cl]
                nc.vector.tensor_scalar_mul(mean, ps1[:, 0:cl], 1.0 / D)
                nc.vector.tensor_mul(var, mean, mean)
                nc.vector.scalar_tensor_tensor(
                    var, ps2[:, 0:cl], 1.0 / D, var,
                    op0=OP.mult, op1=OP.subtract)
                nc.scalar.activation(var, var, AF.Sqrt, bias=eps_t)
                nc.vector.reciprocal(rstd, var)
                nc.vector.tensor_mul(mr, mean, rstd)
                RM = dvp.tile([P, 2 * 512], BF16, tag="rm", name="RM", bufs=3)
                nc.gpsimd.partition_broadcast(RM[:, 0:cl], rstd)
                nc.gpsimd.partition_broadcast(RM[:, 512:512 + cl], mr)
                rms[ci] = RM

            def ln_norm(rms, gi, out_tile, ci, t0, cl):
                RM = rms[ci]
                for dc in range(DC):
                    hsl = hT[:, dc * ptl + t0:dc * ptl + t0 + cl]
                    d1 = lnp.tile([P, 512], BF16, tag="d1", name=f"d1_{dc}")
                    nc.vector.tensor_mul(d1[:, 0:cl], hsl, RM[:, 0:cl])
                    osl = out_tile[:, dc * ptl + t0:dc * ptl + t0 + cl]
                    if use_lng and gi is not None:
                        d2 = lnp.tile([P, 512], BF16, tag="d2",
                                      name=f"d2_{dc}")
                        nc.vector.tensor_sub(d2[:, 0:cl], d1[:, 0:cl],
                                             RM[:, 512:512 + cl])
                        g0 = 8 * (2 * gi)
                        nc.vector.tensor_scalar(
                            osl, d2[:, 0:cl],
                            lngb[:, g0 + dc:g0 + dc + 1],
                            lngb[:, g0 + 8 + dc:g0 + 8 + dc + 1],
                            op0=OP.mult, op1=OP.add)
                    else:
                        nc.vector.tensor_sub(osl, d1[:, 0:cl],
                                             RM[:, 512:512 + cl])

            def ln_chunk(gi, out_tile, t0, cl):
                rms = {}
                ln_stats(rms, 0, t0, cl)
                ln_norm(rms, gi, out_tile, 0, t0, cl)

            def ln_pass(gi, out_tile):
                for (t0, cl) in chs:
                    ln_chunk(gi, out_tile, t0, cl)

            # ---------- embeddings ----------
            for dc in range(DC):
                nc.sync.dma_start(out=hT[:, dc * ptl:(dc + 1) * ptl],
                                  in_=baseT[:, dc * ptl:(dc + 1) * ptl])
            rms0 = {}
            for ci, (t0, cl) in enumerate(chs):
                for dc in range(DC):
                    pse = pp.tile([P, 512], F32, tag="mm", name="pse")
                    for vc in range(VC):
                        nc.tensor.matmul(
                            pse[:, 0:cl],
                            lhsT=tesb[:, vc * D + dc * 128:vc * D + dc * 128 + 128],
                            rhs=ohsb[:, vc * ptl + t0:vc * ptl + t0 + cl],
                            start=(vc == 0), stop=(vc == VC - 1))
                    hsl = hT[:, dc * ptl + t0:dc * ptl + t0 + cl]
                    nc.vector.tensor_add(hsl, pse[:, 0:cl], hsl)
                ln_stats(rms0, ci, t0, cl)
            for ci, (t0, cl) in enumerate(chs):
                ln_norm(rms0, None, hT, ci, t0, cl)   # LN0 in place

            # ---------- layers ----------
            for l in range(L):
                xh = xhp.tile([P, DC * ptl], BF16, tag="xh", name=f"xh{l}a")

                # ---- K/Q/V + attention, interleaved ----
                KT = big.tile([P, DC * ptl], BF16, tag="b18", name=f"KT{l}")
                Vsb = big.tile([P, nt * H * 65], BF16, tag="b18", name=f"Vsb{l}")
                QT = big.tile([P, DC * ptl], BF16, tag="b18", name=f"QT{l}")
                ctxc = big.tile([P, DC * ptl], BF16, tag="b18", name=f"ctx{l}")
                if lt < ptl:
                    nc.vector.memset(
                        Vsb[:, (nt - 1) * H * 65:nt * H * 65], 0.0)
                ones_v = Vsb.rearrange("p (g x) -> p g x", x=65)[:, :, 64:65]
                nc.vector.memset(ones_v, 1.0)

                def v_tg(nh, tg):
                    if True:
                        tts = [t for t in range(4 * tg, min(4 * tg + 4, nt))
                               if lt - t * P > 0]
                        pvs = {}
                        for tt in tts:
                            pvs[tt] = pp.tile([P, 512], F32, tag="mm",
                                              name=f"psv{tt}_{nh}")
                        for dc in range(DC):
                            vrb = wcb.tile([P, 512], BF16, tag="w",
                                           name=f"vrb{nh}_{tg}_{dc}")
                            nc.sync.dma_start(
                                out=vrb,
                                in_=vrb_d[l][:, (nh * DC + dc) * 512:
                                             (nh * DC + dc + 1) * 512])
                            for tt in tts:
                                tl = min(P, lt - tt * P)
                                nc.tensor.matmul(
                                    pvs[tt][0:tl, :],
                                    lhsT=xh[:, dc * ptl + tt * P:dc * ptl + tt * P + tl],
                                    rhs=vrb,
                                    start=(dc == 0), stop=(dc == DC - 1))
                        for tt in tts:
                            tl = min(P, lt - tt * P)
                            pv = pvs[tt][0:tl, :].rearrange(
                                "p (h x) -> p h x", h=8)
                            ov = Vsb[0:tl, (tt * H + nh * 8) * 65:
                                     (tt * H + nh * 8 + 8) * 65].rearrange(
                                "p (h x) -> p h x", x=65)[:, :, 0:64]
                            nc.scalar.copy(ov, pv)

                def kq_block(oc):
                    kcb = wcb.tile([P, DC * 128], BF16, tag="w",
                                   name=f"kcb{oc}")
                    nc.sync.dma_start(
                        out=kcb, in_=kcb_d[l][:, oc * D:(oc + 1) * D])
                    for (t0, cl) in chs:
                        ps = pp.tile([P, 512], F32, tag="mm", name=f"psk{oc}")
                        for dc in range(DC):
                            nc.tensor.matmul(
                                ps[:, 0:cl],
                                lhsT=kcb[:, dc * 128:dc * 128 + 128],
                                rhs=xh[:, dc * ptl + t0:dc * ptl + t0 + cl],
                                start=(dc == 0), stop=(dc == DC - 1))
                        nc.vector.tensor_scalar_add(
                            KT[:, oc * ptl + t0:oc * ptl + t0 + cl],
                            ps[:, 0:cl], bcol(l, 1)[:, oc:oc + 1])
                    qcb = wcb.tile([P, DC * 128], BF16, tag="w",
                                   name=f"qcb{oc}")
                    nc.sync.dma_start(
                        out=qcb, in_=qcb_d[l][:, oc * D:(oc + 1) * D])
                    for (t0, cl) in chs:
                        ps = pp.tile([P, 512], F32, tag="mm", name=f"psq{oc}")
                        for dc in range(DC):
                            nc.tensor.matmul(
                                ps[:, 0:cl],
                                lhsT=qcb[:, dc * 128:dc * 128 + 128],
                                rhs=xh[:, dc * ptl + t0:dc * ptl + t0 + cl],
                                start=(dc == 0), stop=(dc == DC - 1))
                        nc.vector.tensor_scalar_add(
                            QT[:, oc * ptl + t0:oc * ptl + t0 + cl],
                            ps[:, 0:cl], bcol(l, 0)[:, oc:oc + 1])
                    if lt < ptl:
                        nc.vector.memset(KT[:, oc * ptl + lt:(oc + 1) * ptl],
                                         0.0)
                        nc.vector.memset(QT[:, oc * ptl + lt:(oc + 1) * ptl],
                                         0.0)

                def head_scores(h):
                    dch, po = h // 2, (h % 2) * 64
                    est = estp.tile([P, nt * EW], BF16, tag="est",
                                    name=f"est{h}")
                    ests[h] = est
                    for j in range(nt):
                        if w64:
                            w0 = min(max(j * P - 64, 0), ptl - EW)
                            nq = EW
                            lo = w0
                        else:
                            loj = max(j - 1, 0)
                            hi = min(j + 1, nt - 1)
                            nq = (hi - loj + 1) * P
                            w0 = min(max(j - 1, 0), nt - 3) * P
                            lo = loj * P
                        pst = pp.tile([P, 384], F32, tag="mm", name=f"pst{j}")
                        nc.tensor.matmul(
                            pst[:, 0:nq],
                            lhsT=KT[po:po + 64, dch * ptl + j * P:dch * ptl + j * P + P],
                            rhs=QT[po:po + 64, dch * ptl + lo:dch * ptl + lo + nq],
                            start=True, stop=True)
                        esl = est[:, j * EW + (lo - w0):j * EW + (lo - w0) + nq]
                        nc.scalar.activation(esl, pst[:, 0:nq], AF.Exp,
                                             scale=float(SCALE))
                    nc.vector.tensor_mul(est, est, masks)

                def head_ctx(h):
                    dch, po = h // 2, (h % 2) * 64
                    est = ests[h]
                    for qg in range((nt + 3) // 4):
                        qts = [q for q in range(4 * qg, min(4 * qg + 4, nt))]
                        gw = len(qts) * P
                        psc = pp.tile([65, 512], F32, tag="mm", name=f"psc{qg}")
                        for qi, qt in enumerate(qts):
                            if w64:
                                # left half [qt*128, +64): j=qt-1 then qt
                                # right half [qt*128+64, +64): j=qt then qt+1
                                for half in range(2):
                                    qlo = qt * P + 64 * half
                                    oc_ = psc[:, qi * P + 64 * half:
                                              qi * P + 64 * half + 64]
                                    js = ([qt - 1, qt] if half == 0
                                          else [qt, qt + 1])
                                    js = [j for j in js if 0 <= j < nt]
                                    for kk, j in enumerate(js):
                                        w0 = min(max(j * P - 64, 0), ptl - EW)
                                        rsl = est[:, j * EW + qlo - w0:
                                                  j * EW + qlo - w0 + 64]
                                        nc.tensor.matmul(
                                            oc_,
                                            lhsT=Vsb[:, (j * H + h) * 65:
                                                     (j * H + h) * 65 + 65],
                                            rhs=rsl,
                                            start=(kk == 0),
                                            stop=(kk == len(js) - 1))
                            else:
                                js = [j for j in (qt - 1, qt, qt + 1)
                                      if 0 <= j < nt]
                                for kk, j in enumerate(js):
                                    w0 = min(max(j - 1, 0), nt - 3) * P
                                    rsl = est[:, j * EW + qt * P - w0:
                                              j * EW + qt * P - w0 + P]
                                    nc.tensor.matmul(
                                        psc[:, qi * P:(qi + 1) * P],
                                        lhsT=Vsb[:, (j * H + h) * 65:
                                                 (j * H + h) * 65 + 65],
                                        rhs=rsl,
                                        start=(kk == 0), stop=(kk == len(js) - 1))
                        dinv = dvp.tile([1, 512], BF16, tag="dinv",
                                        name=f"dinv{qg}")
                        nc.vector.reciprocal(dinv[:, 0:gw], psc[64:65, 0:gw])
                        dnb = dvp.tile([P, 512], BF16, tag="dnb",
                                       name=f"dnb{qg}")
                        nc.gpsimd.partition_broadcast(dnb[0:64, 0:gw],
                                                      dinv[:, 0:gw])
                        nc.vector.tensor_mul(
                            ctxc[po:po + 64,
                                 dch * ptl + qg * 512:dch * ptl + qg * 512 + gw],
                            psc[0:64, 0:gw], dnb[0:64, 0:gw])

                ests = {}
                gi1 = 2 * l if use_lng else None
                rms1 = {}
                for ci, (t0, cl) in enumerate(chs):
                    ln_stats(rms1, ci, t0, cl)
                for ci, (t0, cl) in enumerate(chs):
                    ln_norm(rms1, gi1, xh, ci, t0, cl)
                    v_tg(0, ci)
                kq_block(0)
                for oc in range(1, DC):
                    if oc == 5:
                        for ci in range(len(chs)):
                            v_tg(1, ci)
                    head_scores(2 * oc - 2)
                    head_scores(2 * oc - 1)
                    kq_block(oc)
                    head_ctx(2 * oc - 2)
                    head_ctx(2 * oc - 1)
                head_scores(14)
                head_scores(15)
                head_ctx(14)
                head_ctx(15)

                # ---- O-projection (chunk-outer) + residual + LN2 ----
                xh = xhp.tile([P, DC * ptl], BF16, tag="xh", name=f"xh{l}b")
                gi2 = 2 * l + 1 if use_lng else None
                rms2 = {}
                for ci, (t0, cl) in enumerate(chs):
                    for do_ in range(DC):
                        ocb = wcb.tile([P, DC * 128], BF16, tag="w",
                                       name=f"ocb{ci}_{do_}")
                        nc.sync.dma_start(
                            out=ocb, in_=ocb_d[l][:, do_ * D:(do_ + 1) * D])
                        ps = pp.tile([P, 512], F32, tag="mm", name=f"pso{do_}")
                        for dc in range(DC):
                            nc.tensor.matmul(
                                ps[:, 0:cl], lhsT=ocb[:, dc * 128:dc * 128 + 128],
                                rhs=ctxc[:, dc * ptl + t0:dc * ptl + t0 + cl],
                                start=(dc == 0), stop=(dc == DC - 1))
                        hsl = hT[:, do_ * ptl + t0:do_ * ptl + t0 + cl]
                        nc.vector.scalar_tensor_tensor(
                            hsl, ps[:, 0:cl], bcol(l, 3)[:, do_:do_ + 1], hsl,
                            op0=OP.add, op1=OP.add)
                    ln_stats(rms2, ci, t0, cl)
                for ci, (t0, cl) in enumerate(chs):
                    ln_norm(rms2, gi2, xh, ci, t0, cl)

                # ---- FFN ----
                Us = [big.tile([P, 8 * ptl], BF16, tag="b18", name=f"U{l}_{i}")
                      for i in range(4)]

                def usl(fc, t0, cl):
                    t = Us[fc // 8]
                    k = fc % 8
                    return t[:, k * ptl + t0:k * ptl + t0 + cl]

                for fcb in range(8):
                    w1cb = wcb.tile([P, DC * 512], BF16, tag="w",
                                    name=f"w1cb{fcb}")
                    nc.sync.dma_start(
                        out=w1cb,
                        in_=w1cb_d[l][:, fcb * DC * 512:(fcb + 1) * DC * 512])
                    for fc2 in range(4):
                        fc = fcb * 4 + fc2
                        for (t0, cl) in chs:
                            ps = pp.tile([P, 512], F32, tag="mm",
                                         name=f"psf{fc2}")
                            for dc in range(DC):
                                nc.tensor.matmul(
                                    ps[:, 0:cl],
                                    lhsT=w1cb[:, dc * 512 + fc2 * 128:
                                              dc * 512 + fc2 * 128 + 128],
                                    rhs=xh[:, dc * ptl + t0:dc * ptl + t0 + cl],
                                    start=(dc == 0), stop=(dc == DC - 1))
                            bidx = 5 + fc // 8
                            nc.scalar.activation(
                                usl(fc, t0, cl), ps[:, 0:cl], AF.Gelu,
                                bias=bcol(l, bidx)[:, fc % 8:fc % 8 + 1])
                for do_ in range(DC):
                    w2cb = wcb.tile([P, FC * 128], BF16, tag="w",
                                    name=f"w2cb{do_}")
                    nc.sync.dma_start(
                        out=w2cb,
                        in_=w2cb_d[l][:, do_ * FC * 128:(do_ + 1) * FC * 128])
                    for (t0, cl) in chs:
                        ps = pp.tile([P, 512], F32, tag="mm", name=f"psh{do_}")
                        for fc in range(FC):
                            nc.tensor.matmul(
                                ps[:, 0:cl],
                                lhsT=w2cb[:, fc * 128:fc * 128 + 128],
                                rhs=usl(fc, t0, cl),
                                start=(fc == 0), stop=(fc == FC - 1))
                        hsl = hT[:, do_ * ptl + t0:do_ * ptl + t0 + cl]
                        nc.vector.scalar_tensor_tensor(
                            hsl, ps[:, 0:cl], bcol(l, 4)[:, do_:do_ + 1], hsl,
                            op0=OP.add, op1=OP.add)
                    if l == L - 1:
                        nc.sync.dma_start(
                            out=houtT[:, do_ * ptl:(do_ + 1) * ptl],
                            in_=hT[:, do_ * ptl:(do_ + 1) * ptl])

    nc.compile()
    return nc


_NC_CACHE = {}


def _get_nc(lt=1032, nt=9, use_lng=False, w64=True):
    key = (lt, nt, use_lng, w64)
    if key not in _NC_CACHE:
        _NC_CACHE[key] = _build(lt, nt, use_lng, w64)
    return _NC_CACHE[key]


def _pack_shared(inputs, lt, nt, use_lng):
    bf = np.dtype("bfloat16") if hasattr(np, "bfloat16") else None
    import ml_dtypes
    BFD = ml_dtypes.bfloat16

    def b16(x):
        return np.ascontiguousarray(np.asarray(x, np.float32).astype(BFD))

    tok = np.asarray(inputs["tok_emb"], np.float32)
    tokp = np.zeros((VP, D), np.float32)
    tokp[:tok.shape[0]] = tok
    tokemb = b16(tokp.reshape(VC, P, D).transpose(1, 0, 2).reshape(P, VC * D))

    shared = {"tokemb": tokemb}
    for l in range(L):
        Wq = np.asarray(inputs["Wq"][l], np.float32)
        Wk = np.asarray(inputs["Wk"][l], np.float32)
        Wv = np.asarray(inputs["Wv"][l], np.float32)
        Wo = np.asarray(inputs["Wo"][l], np.float32)
        W1 = np.asarray(inputs["W1"][l], np.float32)
        W2 = np.asarray(inputs["W2"][l], np.float32)

        def colblocks(W, ocn):  # [D, D] -> [P, ocn*DC*128]
            # block (oc): [p, dc, c] = W[dc*128+p, oc*128+c]
            Wr = W.reshape(DC, P, ocn, 128)  # [dc, p, oc, c]
            return np.ascontiguousarray(
                Wr.transpose(1, 2, 0, 3).reshape(P, ocn * DC * 128))

        shared[f"kcb{l}"] = b16(colblocks(Wk, DC))
        shared[f"qcb{l}"] = b16(colblocks(Wq, DC))
        shared[f"ocb{l}"] = b16(colblocks(Wo, DC))
        # vrb: [p, nh, dc, c] = Wv[dc*128+p, nh*512+c]
        Wvr = Wv.reshape(DC, P, 2, 512)
        shared[f"vrb{l}"] = b16(
            Wvr.transpose(1, 2, 0, 3).reshape(P, 2 * DC * 512))
        # w1cb: [p, fcb, dc, c] = W1[dc*128+p, fcb*512+c]
        W1r = W1.reshape(DC, P, 8, 512)
        shared[f"w1cb{l}"] = b16(
            W1r.transpose(1, 2, 0, 3).reshape(P, 8 * DC * 512))
        # w2cb: [p, do, fc, c] = W2[fc*128+p, do*128+c]
        W2r = W2.reshape(FC, P, DC, 128)
        shared[f"w2cb{l}"] = b16(
            W2r.transpose(1, 2, 0, 3).reshape(P, DC * FC * 128))

    cbw = np.zeros((P, 2 + 96 * L), np.float32)
    cbw[:, 0] = 1.0
    cbw[0, 1] = EPS
    for l in range(L):
        c0 = 2 + 96 * l
        # bv is folded into bo: probs sum to 1, so ctx@Wo + bo with V+bv
        # equals (ctx from plain V)@Wo + (bo + bv@Wo).
        bo_eff = (np.asarray(inputs["bo"][l], np.float32)
                  + np.asarray(inputs["bv"][l], np.float32)
                  @ np.asarray(inputs["Wo"][l], np.float32))
        vals = {"bq": np.asarray(inputs["bq"][l], np.float32),
                "bk": np.asarray(inputs["bk"][l], np.float32),
                "bv": np.zeros(D, np.float32),
                "bo": bo_eff,
                "b2": np.asarray(inputs["b2"][l], np.float32)}
        for i, key in enumerate(("bq", "bk", "bv", "bo", "b2")):
            cbw[:, c0 + 8 * i:c0 + 8 * i + 8] = vals[key].reshape(DC, P).T
        b1v = np.asarray(inputs["b1"][l], np.float32)
        cbw[:, c0 + 40:c0 + 72] = b1v.reshape(FC, P).T
    shared["cb"] = np.ascontiguousarray(cbw)

    if use_lng:
        gb = np.zeros((P, 8 * (2 + 4 * L)), np.float32)
        # group 0: ln0 (handled as gi=None in build... keep identity)
        idx = 0
        for l in range(L):
            for which in range(2):
                gi = 2 * l + which
                g = np.asarray(inputs["ln1_g" if which == 0 else "ln2_g"][l],
                               np.float32)
                bb = np.asarray(inputs["ln1_b" if which == 0 else "ln2_b"][l],
                                np.float32)
                gb[:, 8 * (2 * gi):8 * (2 * gi) + 8] = g.reshape(DC, P).T
                gb[:, 8 * (2 * gi + 1):8 * (2 * gi + 1) + 8] = bb.reshape(DC, P).T
        shared["lngb"] = np.ascontiguousarray(gb)
    return shared


def _prep_core(inputs, b, start, n, lt, nt, w64):
    import ml_dtypes
    BFD = ml_dtypes.bfloat16
    ptl = nt * P

    def b16(x):
        return np.ascontiguousarray(np.asarray(x, np.float32).astype(BFD))

    ids = np.asarray(inputs["input_ids"][b, start:start + n])
    pid = np.asarray(inputs["patch_ids"][b, start:start + n]).astype(np.int64)
    pos_emb = np.asarray(inputs["pos_emb"], np.float32)
    hashes = np.asarray(inputs["hash_embeddings"], np.float32)

    oh = np.zeros((VP, ptl), np.float32)
    oh[ids, np.arange(n)] = 1.0
    oht = b16(oh.reshape(VC, P, ptl).transpose(1, 0, 2).reshape(P, VC * ptl))

    base = np.zeros((ptl, D), np.float32)
    base[:n] = pos_emb[start:start + n] + hashes[b, start:start + n]
    baseT = np.ascontiguousarray(
        base.reshape(ptl, DC, P).transpose(2, 1, 0).reshape(P, DC * ptl))

    pidp = np.empty(ptl, np.int64)
    pidp[:n] = pid
    pidp[n:] = -np.arange(1, ptl - n + 1)

    ew = 256 if w64 else 384
    m = np.zeros((nt, P, ew), np.float32)
    for j in range(nt):
        if w64:
            w0 = int(np.clip(j * P - 64, 0, ptl - ew))
        else:
            w0 = int(np.clip(j - 1, 0, nt - 3)) * P
        kk = pidp[j * P:(j + 1) * P]
        qq = pidp[w0:w0 + ew]
        m[j] = (kk[:, None] == qq[None, :]).astype(np.float32)
    masks = b16(m.transpose(1, 0, 2).reshape(P, nt * ew))
    return {"oht": oht, "baseT": baseT, "masks": masks}


def kernel(**inputs):
    pid_all = np.asarray(inputs["patch_ids"])

    shards = []
    for b in range(B):
        pid = np.asarray(pid_all[b])
        bnd = np.nonzero(pid[1:] != pid[:-1])[0] + 1
        cand = bnd[(bnd >= S - 1152) & (bnd <= 1152)]
        if len(cand) == 0:
            raise RuntimeError("no patch boundary near S/2; cannot shard")
        s = int(cand[np.argmin(np.abs(cand - S // 2))])
        shards.append((b, 0, s))
        shards.append((b, s, S - s))

    lt = max(n for _, _, n in shards)
    lt = max(lt, 1026)  # floor so chunk 3 isn't degenerate-tiny
    nt = (lt + P - 1) // P

    maxrun = 0
    for b in range(B):
        p = np.asarray(pid_all[b])
        bnd = np.nonzero(p[1:] != p[:-1])[0] + 1
        edges = np.concatenate([[0], bnd, [len(p)]])
        maxrun = max(maxrun, int(np.diff(edges).max()))
    w64 = maxrun <= 64

    use_lng = not (
        all(np.all(np.asarray(inputs[k]) == 1.0)
            for k in ("ln0_g", "ln1_g", "ln2_g")) and
        all(np.all(np.asarray(inputs[k]) == 0.0)
            for k in ("ln0_b", "ln1_b", "ln2_b")))
    if use_lng:
        raise NotImplementedError(
            "non-identity LN affine not supported in fast path")

    shared = _pack_shared(inputs, lt, nt, use_lng)
    in_maps = []
    for b, start, n in shards:
        mcore = dict(shared)
        mcore.update(_prep_core(inputs, b, start, n, lt, nt, w64))
        in_maps.append(mcore)

    nc = _get_nc(lt, nt, use_lng, w64)
    res = bass_utils.run_bass_kernel_spmd(nc, in_maps,
                                          core_ids=list(range(NCORES)))

    ptl = nt * P
    out = np.zeros((B, S, D), np.float32)
    for i, (b, start, n) in enumerate(shards):
        ht = res.results[i]["houtT"]
        hfull = ht.reshape(P, DC, ptl).transpose(2, 1, 0).reshape(ptl, D)
        out[b, start:start + n] = hfull[:n]
    return out


if __name__ == "__main__":
    import sys
    lt = int(sys.argv[1]) if len(sys.argv) > 1 else 1032
    _get_nc(lt, (lt + P - 1) // P, False)
    print("built ok")


# revision 20
# speedup vs baseline: 1.3284x; 1.0327x over previous
"""BLT local encoder (2-layer transformer, patch-equality block-diagonal attention)
on 8 Trainium2 NeuronCores.

v2. Sharding: each of the 4 sequences splits at a patch-run boundary nearest
S/2 -> 8 independent shards, one per core, zero cross-core communication.

Kernel design (per core, L_tok = max shard length ~1032):
- Residual hT kept float32 feature-major [P, 8dc x PTL]; everything else bf16.
- Weights prepacked host-side into SBUF-ready bf16 col/row blocks, streamed
  once per layer (no restreaming), double-buffered.
- One LayerNorm per sublayer, output xh bf16 reused by Q, K and V.
- Full-shard attention: per (head, key-tile j) one score matmul with moving
  dim >= 256; softmax denominator via a ones-column appended to V (row 64 of
  the ctx psum); per-head normalize fused into the psum->SBUF copy.
- Engine split: PE matmuls; DVE normalize/copies/masks; Act square/exp/gelu;
  Pool partition-broadcasts + residual adds.
"""

import numpy as np

import concourse.bass as bass
import concourse.tile as tile
from concourse import bacc, bass_utils, mybir

F32 = mybir.dt.float32
F32R = mybir.dt.float32r
BF16 = mybir.dt.bfloat16
AF = mybir.ActivationFunctionType
OP = mybir.AluOpType

B, S, D, H, F, L = 4, 2048, 1024, 16, 4096, 2
DH = D // H      # 64
DC = D // 128    # 8
FC = F // 128    # 32
EPS = 1e-5
SCALE = 1.0 / np.sqrt(DH)
P = 128
VP = 384         # vocab 260 padded
VC = VP // 128   # 3
NCORES = 8


def _chunks(lt):
    out = []
    o = 0
    while o < lt:
        c = min(512, lt - o)
        out.append((o, c))
        o += c
    return out


def _build(lt, nt, use_lng, w64):
    """lt: tokens; nt: tiles; use_lng: ln affine ops; w64: +-64-token window."""
    ptl = nt * P
    EW = 256 if w64 else 384
    chs = _chunks(lt)
    nc = bacc.Bacc("TRN2", target_bir_lowering=False, debug=False,
                   num_devices=NCORES)

    def din(name, shape, dt=BF16):
        return nc.dram_tensor(name, shape, dt, kind="ExternalInput").ap()

    oht = din("oht", [P, VC * ptl])
    tokemb_d = din("tokemb", [P, VC * D])
    baseT = din("baseT", [P, DC * ptl], F32R)
    masks_d = din("masks", [P, nt * EW])
    # prepacked weights
    kcb_d, qcb_d, ocb_d, vrb_d, w1cb_d, w2cb_d = [], [], [], [], [], []
    for l in range(L):
        kcb_d.append(din(f"kcb{l}", [P, DC * DC * 128]))
        qcb_d.append(din(f"qcb{l}", [P, DC * DC * 128]))
        ocb_d.append(din(f"ocb{l}", [P, DC * DC * 128]))
        vrb_d.append(din(f"vrb{l}", [P, DC * D]))
        w1cb_d.append(din(f"w1cb{l}", [P, 8 * DC * 512]))
        w2cb_d.append(din(f"w2cb{l}", [P, DC * FC * 128]))
    # packed per-feature consts: [P, col] layout, 8 cols per D-vector
    # cols: 0 ones | 1 eps(row0) | then per layer l at 2+64*l:
    #   bq 0:8 bk 8:16 bv 16:24 bo 24:32 b2 32:40 b1 40:72 (unused gap)
    # ln g/b (if use_lng): separate tensor lngb
    cb_d = din("cb", [P, 2 + 96 * L], F32)
    lngb_d = din("lngb", [P, 8 * (2 + 4 * L)], F32) if use_lng else None
    houtT = nc.dram_tensor("houtT", [P, DC * ptl], F32R,
                           kind="ExternalOutput").ap()

    with tile.TileContext(nc) as tc:
        with (
            nc.allow_low_precision(
                reason="bf16 softmax/LN staging validated vs reference"),
            tc.tile_pool(name="pers", bufs=1) as pers,
            tc.tile_pool(name="big", bufs=4) as big,
            tc.tile_pool(name="xhp", bufs=1) as xhp,
            tc.tile_pool(name="wcb", bufs=4) as wcb,
            tc.tile_pool(name="est", bufs=2) as estp,
            tc.tile_pool(name="lnt", bufs=3) as lnp,
            tc.tile_pool(name="sm", bufs=2) as smp,
            tc.tile_pool(name="dv", bufs=2) as dvp,
            tc.tile_pool(name="pp", bufs=8, space="PSUM") as pp,
        ):
            ohsb = wcb.tile([P, VC * ptl], BF16, tag="w", name="ohsb")
            for vc in range(VC):
                nc.sync.dma_start(out=ohsb[:, vc * ptl:(vc + 1) * ptl],
                                  in_=oht[:, vc * ptl:(vc + 1) * ptl])
            tesb = wcb.tile([P, VC * D], BF16, tag="w", name="tesb")
            nc.sync.dma_start(out=tesb, in_=tokemb_d)
            cb = pers.tile([P, 2 + 96 * L], F32, tag="cb")
            nc.sync.dma_start(out=cb, in_=cb_d)
            eps_t = cb[0:1, 1:2]
            ones_r = pers.tile([P, 1], F32R, tag="ones_r")
            nc.vector.tensor_copy(ones_r, cb[:, 0:1])
            ones_b = pers.tile([P, 1], BF16, tag="ones_b")
            nc.vector.tensor_copy(ones_b, cb[:, 0:1])
            if use_lng:
                lngb = pers.tile([P, 8 * (2 + 4 * L)], F32, tag="lngb")
                nc.sync.dma_start(out=lngb, in_=lngb_d)

            masks = pers.tile([P, nt * EW], BF16, tag="masks")
            nc.sync.dma_start(out=masks, in_=masks_d)

            hT = pers.tile([P, DC * ptl], F32R, tag="hT")

            def bcol(l, i):  # bias col i (in 8-col groups) for layer l
                c0 = 2 + 96 * l + 8 * i
                return cb[:, c0:c0 + 8]

            def ln_stats(rms, ci, t0, cl):
                ps1 = pp.tile([1, 512], F32, tag="mm", name="lns1")
                ps2 = pp.tile([1, 512], F32, tag="mm", name="lns2")
                for dc in range(DC):
                    hsl = hT[:, dc * ptl + t0:dc * ptl + t0 + cl]
                    sq = lnp.tile([P, 512], BF16, tag="sq", name=f"sq{dc}")
                    nc.scalar.activation(sq[:, 0:cl], hsl, AF.Square)
                    nc.tensor.matmul(ps1[:, 0:cl], lhsT=ones_r, rhs=hsl,
                                     start=(dc == 0), stop=(dc == DC - 1))
                    nc.tensor.matmul(ps2[:, 0:cl], lhsT=ones_b,
                                     rhs=sq[:, 0:cl],
                                     start=(dc == 0), stop=(dc == DC - 1))
                st = smp.tile([P, 2 * 512], F32, tag="st", name="st")
                stb = smp.tile([P, 2 * 512], BF16, tag="stb", name="stb")
                mean = st[0:1, 0:cl]
                var = st[0:1, 512:512 + cl]
                rstd = stb[0:1, 0:cl]
                mr = stb[0:1, 512:512 + cl]
                nc.vector.tensor_scalar_mul(mean, ps1[:, 0:cl], 1.0 / D)
                nc.vector.tensor_mul(var, mean, mean)
                nc.vector.scalar_tensor_tensor(
                    var, ps2[:, 0:cl], 1.0 / D, var,
                    op0=OP.mult, op1=OP.subtract)
                nc.scalar.activation(var, var, AF.Sqrt, bias=eps_t)
                nc.vector.reciprocal(rstd, var)
                nc.vector.tensor_mul(mr, mean, rstd)
                RM = dvp.tile([P, 2 * 512], BF16, tag="rm", name="RM", bufs=3)
                nc.gpsimd.partition_broadcast(RM[:, 0:cl], rstd)
                nc.gpsimd.partition_broadcast(RM[:, 512:512 + cl], mr)
                rms[ci] = RM

            def ln_norm(rms, gi, out_tile, ci, t0, cl):
                RM = rms[ci]
                for dc in range(DC):
                    hsl = hT[:, dc * ptl + t0:dc * ptl + t0 + cl]
                    d1 = lnp.tile([P, 512], BF16, tag="d1", name=f"d1_{dc}")
                    eng = nc.gpsimd if dc >= 6 else nc.vector
                    eng.tensor_mul(d1[:, 0:cl], hsl, RM[:, 0:cl])
                    osl = out_tile[:, dc * ptl + t0:dc * ptl + t0 + cl]
                    if use_lng and gi is not None:
                        d2 = lnp.tile([P, 512], BF16, tag="d2",
                                      name=f"d2_{dc}")
                        nc.vector.tensor_sub(d2[:, 0:cl], d1[:, 0:cl],
                                             RM[:, 512:512 + cl])
                        g0 = 8 * (2 * gi)
                        nc.vector.tensor_scalar(
                            osl, d2[:, 0:cl],
                            lngb[:, g0 + dc:g0 + dc + 1],
                            lngb[:, g0 + 8 + dc:g0 + 8 + dc + 1],
                            op0=OP.mult, op1=OP.add)
                    else:
                        nc.vector.tensor_sub(osl, d1[:, 0:cl],
                                             RM[:, 512:512 + cl])

            def ln_chunk(gi, out_tile, t0, cl):
                rms = {}
                ln_stats(rms, 0, t0, cl)
                ln_norm(rms, gi, out_tile, 0, t0, cl)

            def ln_pass(gi, out_tile):
                for (t0, cl) in chs:
                    ln_chunk(gi, out_tile, t0, cl)

            # ---------- embeddings ----------
            for dc in range(DC):
                nc.sync.dma_start(out=hT[:, dc * ptl:(dc + 1) * ptl],
                                  in_=baseT[:, dc * ptl:(dc + 1) * ptl])
            rms0 = {}
            for ci, (t0, cl) in enumerate(chs):
                for dc in range(DC):
                    pse = pp.tile([P, 512], F32, tag="mm", name="pse")
                    for vc in range(VC):
                        nc.tensor.matmul(
                            pse[:, 0:cl],
                            lhsT=tesb[:, vc * D + dc * 128:vc * D + dc * 128 + 128],
                            rhs=ohsb[:, vc * ptl + t0:vc * ptl + t0 + cl],
                            start=(vc == 0), stop=(vc == VC - 1))
                    hsl = hT[:, dc * ptl + t0:dc * ptl + t0 + cl]
                    nc.vector.tensor_add(hsl, pse[:, 0:cl], hsl)
                ln_stats(rms0, ci, t0, cl)
            for ci, (t0, cl) in enumerate(chs):
                ln_norm(rms0, None, hT, ci, t0, cl)   # LN0 in place

            # ---------- layers ----------
            for l in range(L):
                xh = xhp.tile([P, DC * ptl], BF16, tag="xh", name=f"xh{l}a")

                # ---- K/Q/V + attention, interleaved ----
                KT = big.tile([P, DC * ptl], BF16, tag="b18", name=f"KT{l}")
                Vsb = big.tile([P, nt * H * 65], BF16, tag="b18", name=f"Vsb{l}")
                QT = big.tile([P, DC * ptl], BF16, tag="b18", name=f"QT{l}")
                ctxc = big.tile([P, DC * ptl], BF16, tag="b18", name=f"ctx{l}")
                if lt < ptl:
                    nc.vector.memset(
                        Vsb[:, (nt - 1) * H * 65:nt * H * 65], 0.0)
                ones_v = Vsb.rearrange("p (g x) -> p g x", x=65)[:, :, 64:65]
                nc.vector.memset(ones_v, 1.0)

                def v_tg(nh, tg):
                    if True:
                        tts = [t for t in range(4 * tg, min(4 * tg + 4, nt))
                               if lt - t * P > 0]
                        pvs = {}
                        for tt in tts:
                            pvs[tt] = pp.tile([P, 512], F32, tag="mm",
                                              name=f"psv{tt}_{nh}")
                        for dc in range(DC):
                            vrb = wcb.tile([P, 512], BF16, tag="w",
                                           name=f"vrb{nh}_{tg}_{dc}")
                            nc.sync.dma_start(
                                out=vrb,
                                in_=vrb_d[l][:, (nh * DC + dc) * 512:
                                             (nh * DC + dc + 1) * 512])
                            for tt in tts:
                                tl = min(P, lt - tt * P)
                                nc.tensor.matmul(
                                    pvs[tt][0:tl, :],
                                    lhsT=xh[:, dc * ptl + tt * P:dc * ptl + tt * P + tl],
                                    rhs=vrb,
                                    start=(dc == 0), stop=(dc == DC - 1))
                        for tt in tts:
                            tl = min(P, lt - tt * P)
                            pv = pvs[tt][0:tl, :].rearrange(
                                "p (h x) -> p h x", h=8)
                            ov = Vsb[0:tl, (tt * H + nh * 8) * 65:
                                     (tt * H + nh * 8 + 8) * 65].rearrange(
                                "p (h x) -> p h x", x=65)[:, :, 0:64]
                            nc.scalar.copy(ov, pv)

                def kq_block(oc):
                    kcb = wcb.tile([P, DC * 128], BF16, tag="w",
                                   name=f"kcb{oc}")
                    nc.sync.dma_start(
                        out=kcb, in_=kcb_d[l][:, oc * D:(oc + 1) * D])
                    for (t0, cl) in chs:
                        ps = pp.tile([P, 512], F32, tag="mm", name=f"psk{oc}")
                        for dc in range(DC):
                            nc.tensor.matmul(
                                ps[:, 0:cl],
                                lhsT=kcb[:, dc * 128:dc * 128 + 128],
                                rhs=xh[:, dc * ptl + t0:dc * ptl + t0 + cl],
                                start=(dc == 0), stop=(dc == DC - 1))
                        nc.vector.tensor_scalar_add(
                            KT[:, oc * ptl + t0:oc * ptl + t0 + cl],
                            ps[:, 0:cl], bcol(l, 1)[:, oc:oc + 1])
                    qcb = wcb.tile([P, DC * 128], BF16, tag="w",
                                   name=f"qcb{oc}")
                    nc.sync.dma_start(
                        out=qcb, in_=qcb_d[l][:, oc * D:(oc + 1) * D])
                    for (t0, cl) in chs:
                        ps = pp.tile([P, 512], F32, tag="mm", name=f"psq{oc}")
                        for dc in range(DC):
                            nc.tensor.matmul(
                                ps[:, 0:cl],
                                lhsT=qcb[:, dc * 128:dc * 128 + 128],
                                rhs=xh[:, dc * ptl + t0:dc * ptl + t0 + cl],
                                start=(dc == 0), stop=(dc == DC - 1))
                        nc.vector.tensor_scalar_add(
                            QT[:, oc * ptl + t0:oc * ptl + t0 + cl],
                            ps[:, 0:cl], bcol(l, 0)[:, oc:oc + 1])
                    if lt < ptl:
                        nc.vector.memset(KT[:, oc * ptl + lt:(oc + 1) * ptl],
                                         0.0)
                        nc.vector.memset(QT[:, oc * ptl + lt:(oc + 1) * ptl],
                                         0.0)

                def head_scores(h):
                    dch, po = h // 2, (h % 2) * 64
                    est = estp.tile([P, nt * EW], BF16, tag="est",
                                    name=f"est{h}")
                    ests[h] = est
                    for j in range(nt):
                        if w64:
                            w0 = min(max(j * P - 64, 0), ptl - EW)
                            nq = EW
                            lo = w0
                        else:
                            loj = max(j - 1, 0)
                            hi = min(j + 1, nt - 1)
                            nq = (hi - loj + 1) * P
                            w0 = min(max(j - 1, 0), nt - 3) * P
                            lo = loj * P
                        pst = pp.tile([P, 384], F32, tag="mm", name=f"pst{j}")
                        nc.tensor.matmul(
                            pst[:, 0:nq],
                            lhsT=KT[po:po + 64, dch * ptl + j * P:dch * ptl + j * P + P],
                            rhs=QT[po:po + 64, dch * ptl + lo:dch * ptl + lo + nq],
                            start=True, stop=True)
                        esl = est[:, j * EW + (lo - w0):j * EW + (lo - w0) + nq]
                        nc.scalar.activation(esl, pst[:, 0:nq], AF.Exp,
                                             scale=float(SCALE))
                    nc.vector.tensor_mul(est, est, masks)

                def head_ctx(h):
                    dch, po = h // 2, (h % 2) * 64
                    est = ests[h]
                    for qg in range((nt + 3) // 4):
                        qts = [q for q in range(4 * qg, min(4 * qg + 4, nt))]
                        gw = len(qts) * P
                        psc = pp.tile([65, 512], F32, tag="mm", name=f"psc{qg}")
                        for qi, qt in enumerate(qts):
                            if w64:
                                # left half [qt*128, +64): j=qt-1 then qt
                                # right half [qt*128+64, +64): j=qt then qt+1
                                for half in range(2):
                                    qlo = qt * P + 64 * half
                                    oc_ = psc[:, qi * P + 64 * half:
                                              qi * P + 64 * half + 64]
                                    js = ([qt - 1, qt] if half == 0
                                          else [qt, qt + 1])
                                    js = [j for j in js if 0 <= j < nt]
                                    for kk, j in enumerate(js):
                                        w0 = min(max(j * P - 64, 0), ptl - EW)
                                        rsl = est[:, j * EW + qlo - w0:
                                                  j * EW + qlo - w0 + 64]
                                        nc.tensor.matmul(
                                            oc_,
                                            lhsT=Vsb[:, (j * H + h) * 65:
                                                     (j * H + h) * 65 + 65],
                                            rhs=rsl,
                                            start=(kk == 0),
                                            stop=(kk == len(js) - 1))
                            else:
                                js = [j for j in (qt - 1, qt, qt + 1)
                                      if 0 <= j < nt]
                                for kk, j in enumerate(js):
                                    w0 = min(max(j - 1, 0), nt - 3) * P
                                    rsl = est[:, j * EW + qt * P - w0:
                                              j * EW + qt * P - w0 + P]
                                    nc.tensor.matmul(
                                        psc[:, qi * P:(qi + 1) * P],
                                        lhsT=Vsb[:, (j * H + h) * 65:
                                                 (j * H + h) * 65 + 65],
                                        rhs=rsl,
                                        start=(kk == 0), stop=(kk == len(js) - 1))
                        dinv = dvp.tile([1, 512], BF16, tag="dinv",
                                        name=f"dinv{qg}")
                        nc.vector.reciprocal(dinv[:, 0:gw], psc[64:65, 0:gw])
                        dnb = dvp.tile([P, 512], BF16, tag="dnb",
                                       name=f"dnb{qg}")
                        nc.gpsimd.partition_broadcast(dnb[0:64, 0:gw],
                                                      dinv[:, 0:gw])
                        nc.vector.tensor_mul(
                            ctxc[po:po + 64,
                                 dch * ptl + qg * 512:dch * ptl + qg * 512 + gw],
                            psc[0:64, 0:gw], dnb[0:64, 0:gw])

                ests = {}
                gi1 = 2 * l if use_lng else None
                rms1 = {}
                for ci, (t0, cl) in enumerate(chs):
                    ln_stats(rms1, ci, t0, cl)
                for ci, (t0, cl) in enumerate(chs):
                    ln_norm(rms1, gi1, xh, ci, t0, cl)
                    v_tg(0, ci)
                kq_block(0)
                for oc in range(1, DC):
                    if oc == 5:
                        for ci in range(len(chs)):
                            v_tg(1, ci)
                    head_scores(2 * oc - 2)
                    head_scores(2 * oc - 1)
                    kq_block(oc)
                    head_ctx(2 * oc - 2)
                    head_ctx(2 * oc - 1)
                head_scores(14)
                head_scores(15)
                head_ctx(14)
                head_ctx(15)

                # ---- O-projection (chunk-outer) + residual + LN2 ----
                xh = xhp.tile([P, DC * ptl], BF16, tag="xh", name=f"xh{l}b")
                gi2 = 2 * l + 1 if use_lng else None
                rms2 = {}
                for ci, (t0, cl) in enumerate(chs):
                    for do_ in range(DC):
                        ocb = wcb.tile([P, DC * 128], BF16, tag="w",
                                       name=f"ocb{ci}_{do_}")
                        nc.sync.dma_start(
                            out=ocb, in_=ocb_d[l][:, do_ * D:(do_ + 1) * D])
                        ps = pp.tile([P, 512], F32, tag="mm", name=f"pso{do_}")
                        for dc in range(DC):
                            nc.tensor.matmul(
                                ps[:, 0:cl], lhsT=ocb[:, dc * 128:dc * 128 + 128],
                                rhs=ctxc[:, dc * ptl + t0:dc * ptl + t0 + cl],
                                start=(dc == 0), stop=(dc == DC - 1))
                        hsl = hT[:, do_ * ptl + t0:do_ * ptl + t0 + cl]
                        nc.vector.scalar_tensor_tensor(
                            hsl, ps[:, 0:cl], bcol(l, 3)[:, do_:do_ + 1], hsl,
                            op0=OP.add, op1=OP.add)
                    ln_stats(rms2, ci, t0, cl)
                for ci, (t0, cl) in enumerate(chs):
                    ln_norm(rms2, gi2, xh, ci, t0, cl)

                # ---- FFN ----
                Us = [big.tile([P, 8 * ptl], BF16, tag="b18", name=f"U{l}_{i}")
                      for i in range(4)]

                def usl(fc, t0, cl):
                    t = Us[fc // 8]
                    k = fc % 8
                    return t[:, k * ptl + t0:k * ptl + t0 + cl]

                for fcb in range(8):
                    w1cb = wcb.tile([P, DC * 512], BF16, tag="w",
                                    name=f"w1cb{fcb}")
                    nc.sync.dma_start(
                        out=w1cb,
                        in_=w1cb_d[l][:, fcb * DC * 512:(fcb + 1) * DC * 512])
                    for fc2 in range(4):
                        fc = fcb * 4 + fc2
                        for (t0, cl) in chs:
                            ps = pp.tile([P, 512], F32, tag="mm",
                                         name=f"psf{fc2}")
                            for dc in range(DC):
                                nc.tensor.matmul(
                                    ps[:, 0:cl],
                                    lhsT=w1cb[:, dc * 512 + fc2 * 128:
                                              dc * 512 + fc2 * 128 + 128],
                                    rhs=xh[:, dc * ptl + t0:dc * ptl + t0 + cl],
                                    start=(dc == 0), stop=(dc == DC - 1))
                            bidx = 5 + fc // 8
                            nc.scalar.activation(
                                usl(fc, t0, cl), ps[:, 0:cl], AF.Gelu,
                                bias=bcol(l, bidx)[:, fc % 8:fc % 8 + 1])
                for do_ in range(DC):
                    w2cb = wcb.tile([P, FC * 128], BF16, tag="w",
                                    name=f"w2cb{do_}")
                    nc.sync.dma_start(
                        out=w2cb,
                        in_=w2cb_d[l][:, do_ * FC * 128:(do_ + 1) * FC * 128])
                    for (t0, cl) in chs:
                        ps = pp.tile([P, 512], F32, tag="mm", name=f"psh{do_}")
                        for fc in range(FC):
                            nc.tensor.matmul(
                                ps[:, 0:cl],
                                lhsT=w2cb[:, fc * 128:fc * 128 + 128],
                                rhs=usl(fc, t0, cl),
                                start=(fc == 0), stop=(fc == FC - 1))
                        hsl = hT[:, do_ * ptl + t0:do_ * ptl + t0 + cl]
                        nc.vector.scalar_tensor_tensor(
                            hsl, ps[:, 0:cl], bcol(l, 4)[:, do_:do_ + 1], hsl,
                            op0=OP.add, op1=OP.add)
                    if l == L - 1:
                        for (t0o, clo) in chs:
                            nc.sync.dma_start(
                                out=houtT[:, do_ * ptl + t0o:do_ * ptl + t0o + clo],
                                in_=hT[:, do_ * ptl + t0o:do_ * ptl + t0o + clo])

    nc.compile()
    return nc


_NC_CACHE = {}


def _get_nc(lt=1032, nt=9, use_lng=False, w64=True):
    key = (lt, nt, use_lng, w64)
    if key not in _NC_CACHE:
        _NC_CACHE[key] = _build(lt, nt, use_lng, w64)
    return _NC_CACHE[key]


def _pack_shared(inputs, lt, nt, use_lng):
    bf = np.dtype("bfloat16") if hasattr(np, "bfloat16") else None
    import ml_dtypes
    BFD = ml_dtypes.bfloat16

    def b16(x):
        return np.ascontiguousarray(np.asarray(x, np.float32).astype(BFD))

    tok = np.asarray(inputs["tok_emb"], np.float32)
    tokp = np.zeros((VP, D), np.float32)
    tokp[:tok.shape[0]] = tok
    tokemb = b16(tokp.reshape(VC, P, D).transpose(1, 0, 2).reshape(P, VC * D))

    shared = {"tokemb": tokemb}
    for l in range(L):
        Wq = np.asarray(inputs["Wq"][l], np.float32)
        Wk = np.asarray(inputs["Wk"][l], np.float32)
        Wv = np.asarray(inputs["Wv"][l], np.float32)
        Wo = np.asarray(inputs["Wo"][l], np.float32)
        W1 = np.asarray(inputs["W1"][l], np.float32)
        W2 = np.asarray(inputs["W2"][l], np.float32)

        def colblocks(W, ocn):  # [D, D] -> [P, ocn*DC*128]
            # block (oc): [p, dc, c] = W[dc*128+p, oc*128+c]
            Wr = W.reshape(DC, P, ocn, 128)  # [dc, p, oc, c]
            return np.ascontiguousarray(
                Wr.transpose(1, 2, 0, 3).reshape(P, ocn * DC * 128))

        shared[f"kcb{l}"] = b16(colblocks(Wk, DC))
        shared[f"qcb{l}"] = b16(colblocks(Wq, DC))
        shared[f"ocb{l}"] = b16(colblocks(Wo, DC))
        # vrb: [p, nh, dc, c] = Wv[dc*128+p, nh*512+c]
        Wvr = Wv.reshape(DC, P, 2, 512)
        shared[f"vrb{l}"] = b16(
            Wvr.transpose(1, 2, 0, 3).reshape(P, 2 * DC * 512))
        # w1cb: [p, fcb, dc, c] = W1[dc*128+p, fcb*512+c]
        W1r = W1.reshape(DC, P, 8, 512)
        shared[f"w1cb{l}"] = b16(
            W1r.transpose(1, 2, 0, 3).reshape(P, 8 * DC * 512))
        # w2cb: [p, do, fc, c] = W2[fc*128+p, do*128+c]
        W2r = W2.reshape(FC, P, DC, 128)
        shared[f"w2cb{l}"] = b16(
            W2r.transpose(1, 2, 0, 3).reshape(P, DC * FC * 128))

    cbw = np.zeros((P, 2 + 96 * L), np.float32)
    cbw[:, 0] = 1.0
    cbw[0, 1] = EPS
    for l in range(L):
        c0 = 2 + 96 * l
        # bv is folded into bo: probs sum to 1, so ctx@Wo + bo with V+bv
        # equals (ctx from plain V)@Wo + (bo + bv@Wo).
        bo_eff = (np.asarray(inputs["bo"][l], np.float32)
                  + np.asarray(inputs["bv"][l], np.float32)
                  @ np.asarray(inputs["Wo"][l], np.float32))
        vals = {"bq": np.asarray(inputs["bq"][l], np.float32),
                "bk": np.asarray(inputs["bk"][l], np.float32),
                "bv": np.zeros(D, np.float32),
                "bo": bo_eff,
                "b2": np.asarray(inputs["b2"][l], np.float32)}
        for i, key in enumerate(("bq", "bk", "bv", "bo", "b2")):
            cbw[:, c0 + 8 * i:c0 + 8 * i + 8] = vals[key].reshape(DC, P).T
        b1v = np.asarray(inputs["b1"][l], np.float32)
        cbw[:, c0 + 40:c0 + 72] = b1v.reshape(FC, P).T
    shared["cb"] = np.ascontiguousarray(cbw)

    if use_lng:
        gb = np.zeros((P, 8 * (2 + 4 * L)), np.float32)
        # group 0: ln0 (handled as gi=None in build... keep identity)
        idx = 0
        for l in range(L):
            for which in range(2):
                gi = 2 * l + which
                g = np.asarray(inputs["ln1_g" if which == 0 else "ln2_g"][l],
                               np.float32)
                bb = np.asarray(inputs["ln1_b" if which == 0 else "ln2_b"][l],
                                np.float32)
                gb[:, 8 * (2 * gi):8 * (2 * gi) + 8] = g.reshape(DC, P).T
                gb[:, 8 * (2 * gi + 1):8 * (2 * gi + 1) + 8] = bb.reshape(DC, P).T
        shared["lngb"] = np.ascontiguousarray(gb)
    return shared


def _prep_core(inputs, b, start, n, lt, nt, w64):
    import ml_dtypes
    BFD = ml_dtypes.bfloat16
    ptl = nt * P

    def b16(x):
        return np.ascontiguousarray(np.asarray(x, np.float32).astype(BFD))

    ids = np.asarray(inputs["input_ids"][b, start:start + n])
    pid = np.asarray(inputs["patch_ids"][b, start:start + n]).astype(np.int64)
    pos_emb = np.asarray(inputs["pos_emb"], np.float32)
    hashes = np.asarray(inputs["hash_embeddings"], np.float32)

    oh = np.zeros((VP, ptl), np.float32)
    oh[ids, np.arange(n)] = 1.0
    oht = b16(oh.reshape(VC, P, ptl).transpose(1, 0, 2).reshape(P, VC * ptl))

    base = np.zeros((ptl, D), np.float32)
    base[:n] = pos_emb[start:start + n] + hashes[b, start:start + n]
    baseT = np.ascontiguousarray(
        base.reshape(ptl, DC, P).transpose(2, 1, 0).reshape(P, DC * ptl))

    pidp = np.empty(ptl, np.int64)
    pidp[:n] = pid
    pidp[n:] = -np.arange(1, ptl - n + 1)

    ew = 256 if w64 else 384
    m = np.zeros((nt, P, ew), np.float32)
    for j in range(nt):
        if w64:
            w0 = int(np.clip(j * P - 64, 0, ptl - ew))
        else:
            w0 = int(np.clip(j - 1, 0, nt - 3)) * P
        kk = pidp[j * P:(j + 1) * P]
        qq = pidp[w0:w0 + ew]
        m[j] = (kk[:, None] == qq[None, :]).astype(np.float32)
    masks = b16(m.transpose(1, 0, 2).reshape(P, nt * ew))
    return {"oht": oht, "baseT": baseT, "masks": masks}


def kernel(**inputs):
    pid_all = np.asarray(inputs["patch_ids"])

    shards = []
    for b in range(B):
        pid = np.asarray(pid_all[b])
        bnd = np.nonzero(pid[1:] != pid[:-1])[0] + 1
        cand = bnd[(bnd >= S - 1152) & (bnd <= 1152)]
        if len(cand) == 0:
            raise RuntimeError("no patch boundary near S/2; cannot shard")
        s = int(cand[np.argmin(np.abs(cand - S // 2))])
        shards.append((b, 0, s))
        shards.append((b, s, S - s))

    lt = max(n for _, _, n in shards)
    lt = max(lt, 1026)  # floor so chunk 3 isn't degenerate-tiny
    nt = (lt + P - 1) // P

    maxrun = 0
    for b in range(B):
        p = np.asarray(pid_all[b])
        bnd = np.nonzero(p[1:] != p[:-1])[0] + 1
        edges = np.concatenate([[0], bnd, [len(p)]])
        maxrun = max(maxrun, int(np.diff(edges).max()))
    w64 = maxrun <= 64

    use_lng = not (
        all(np.all(np.asarray(inputs[k]) == 1.0)
            for k in ("ln0_g", "ln1_g", "ln2_g")) and
        all(np.all(np.asarray(inputs[k]) == 0.0)
            for k in ("ln0_b", "ln1_b", "ln2_b")))
    if use_lng:
        raise NotImplementedError(
            "non-identity LN affine not supported in fast path")

    shared = _pack_shared(inputs, lt, nt, use_lng)
    in_maps = []
    for b, start, n in shards:
        mcore = dict(shared)
        mcore.update(_prep_core(inputs, b, start, n, lt, nt, w64))
        in_maps.append(mcore)

    nc = _get_nc(lt, nt, use_lng, w64)
    res = bass_utils.run_bass_kernel_spmd(nc, in_maps,
                                          core_ids=list(range(NCORES)))

    ptl = nt * P
    out = np.zeros((B, S, D), np.float32)
    for i, (b, start, n) in enumerate(shards):
        ht = res.results[i]["houtT"]
        hfull = ht.reshape(P, DC, ptl).transpose(2, 1, 0).reshape(ptl, D)
        out[b, start:start + n] = hfull[:n]
    return out


if __name__ == "__main__":
    import sys
    lt = int(sys.argv[1]) if len(sys.argv) > 1 else 1032
    _get_nc(lt, (lt + P - 1) // P, False)
    print("built ok")


# revision 21
# speedup vs baseline: 1.3296x; 1.0009x over previous
"""BLT local encoder (2-layer transformer, patch-equality block-diagonal attention)
on 8 Trainium2 NeuronCores.

v2. Sharding: each of the 4 sequences splits at a patch-run boundary nearest
S/2 -> 8 independent shards, one per core, zero cross-core communication.

Kernel design (per core, L_tok = max shard length ~1032):
- Residual hT kept float32 feature-major [P, 8dc x PTL]; everything else bf16.
- Weights prepacked host-side into SBUF-ready bf16 col/row blocks, streamed
  once per layer (no restreaming), double-buffered.
- One LayerNorm per sublayer, output xh bf16 reused by Q, K and V.
- Full-shard attention: per (head, key-tile j) one score matmul with moving
  dim >= 256; softmax denominator via a ones-column appended to V (row 64 of
  the ctx psum); per-head normalize fused into the psum->SBUF copy.
- Engine split: PE matmuls; DVE normalize/copies/masks; Act square/exp/gelu;
  Pool partition-broadcasts + residual adds.
"""

import numpy as np

import concourse.bass as bass
import concourse.tile as tile
from concourse import bacc, bass_utils, mybir

F32 = mybir.dt.float32
F32R = mybir.dt.float32r
BF16 = mybir.dt.bfloat16
AF = mybir.ActivationFunctionType
OP = mybir.AluOpType

B, S, D, H, F, L = 4, 2048, 1024, 16, 4096, 2
DH = D // H      # 64
DC = D // 128    # 8
FC = F // 128    # 32
EPS = 1e-5
SCALE = 1.0 / np.sqrt(DH)
P = 128
VP = 384         # vocab 260 padded
VC = VP // 128   # 3
NCORES = 8


def _chunks(lt):
    out = []
    o = 0
    while o < lt:
        c = min(512, lt - o)
        out.append((o, c))
        o += c
    return out


def _build(lt, nt, use_lng, w64):
    """lt: tokens; nt: tiles; use_lng: ln affine ops; w64: +-64-token window."""
    ptl = nt * P
    EW = 256 if w64 else 384
    chs = _chunks(lt)
    nc = bacc.Bacc("TRN2", target_bir_lowering=False, debug=False,
                   num_devices=NCORES)

    def din(name, shape, dt=BF16):
        return nc.dram_tensor(name, shape, dt, kind="ExternalInput").ap()

    oht = din("oht", [P, VC * ptl])
    tokemb_d = din("tokemb", [P, VC * D])
    baseT = din("baseT", [P, DC * ptl], F32R)
    masks_d = din("masks", [P, nt * EW])
    # prepacked weights
    kcb_d, qcb_d, ocb_d, vrb_d, w1cb_d, w2cb_d = [], [], [], [], [], []
    for l in range(L):
        kcb_d.append(din(f"kcb{l}", [P, DC * DC * 128]))
        qcb_d.append(din(f"qcb{l}", [P, DC * DC * 128]))
        ocb_d.append(din(f"ocb{l}", [P, DC * DC * 128]))
        vrb_d.append(din(f"vrb{l}", [P, DC * D]))
        w1cb_d.append(din(f"w1cb{l}", [P, 8 * DC * 512]))
        w2cb_d.append(din(f"w2cb{l}", [P, DC * FC * 128]))
    # packed per-feature consts: [P, col] layout, 8 cols per D-vector
    # cols: 0 ones | 1 eps(row0) | then per layer l at 2+64*l:
    #   bq 0:8 bk 8:16 bv 16:24 bo 24:32 b2 32:40 b1 40:72 (unused gap)
    # ln g/b (if use_lng): separate tensor lngb
    cb_d = din("cb", [P, 2 + 96 * L], F32)
    lngb_d = din("lngb", [P, 8 * (2 + 4 * L)], F32) if use_lng else None
    houtT = nc.dram_tensor("houtT", [P, DC * ptl], F32R,
                           kind="ExternalOutput").ap()

    with tile.TileContext(nc) as tc:
        with (
            nc.allow_low_precision(
                reason="bf16 softmax/LN staging validated vs reference"),
            tc.tile_pool(name="pers", bufs=1) as pers,
            tc.tile_pool(name="big", bufs=4) as big,
            tc.tile_pool(name="xhp", bufs=1) as xhp,
            tc.tile_pool(name="wcb", bufs=4) as wcb,
            tc.tile_pool(name="est", bufs=2) as estp,
            tc.tile_pool(name="lnt", bufs=3) as lnp,
            tc.tile_pool(name="sm", bufs=2) as smp,
            tc.tile_pool(name="dv", bufs=2) as dvp,
            tc.tile_pool(name="pp", bufs=8, space="PSUM") as pp,
        ):
            ohsb = wcb.tile([P, VC * ptl], BF16, tag="w", name="ohsb")
            tesb = wcb.tile([P, VC * D], BF16, tag="w", name="tesb")
            for vc in range(VC):
                nc.sync.dma_start(out=ohsb[:, vc * ptl:(vc + 1) * ptl],
                                  in_=oht[:, vc * ptl:(vc + 1) * ptl])
                nc.sync.dma_start(out=tesb[:, vc * D:(vc + 1) * D],
                                  in_=tokemb_d[:, vc * D:(vc + 1) * D])
            cb = pers.tile([P, 2 + 96 * L], F32, tag="cb")
            nc.sync.dma_start(out=cb, in_=cb_d)
            eps_t = cb[0:1, 1:2]
            ones_r = pers.tile([P, 1], F32R, tag="ones_r")
            nc.vector.tensor_copy(ones_r, cb[:, 0:1])
            ones_b = pers.tile([P, 1], BF16, tag="ones_b")
            nc.vector.tensor_copy(ones_b, cb[:, 0:1])
            if use_lng:
                lngb = pers.tile([P, 8 * (2 + 4 * L)], F32, tag="lngb")
                nc.sync.dma_start(out=lngb, in_=lngb_d)

            masks = pers.tile([P, nt * EW], BF16, tag="masks")
            nc.sync.dma_start(out=masks, in_=masks_d)

            hT = pers.tile([P, DC * ptl], F32R, tag="hT")

            def bcol(l, i):  # bias col i (in 8-col groups) for layer l
                c0 = 2 + 96 * l + 8 * i
                return cb[:, c0:c0 + 8]

            def ln_stats(rms, ci, t0, cl):
                ps1 = pp.tile([1, 512], F32, tag="mm", name="lns1")
                ps2 = pp.tile([1, 512], F32, tag="mm", name="lns2")
                for dc in range(DC):
                    hsl = hT[:, dc * ptl + t0:dc * ptl + t0 + cl]
                    sq = lnp.tile([P, 512], BF16, tag="sq", name=f"sq{dc}")
                    if dc < 4:
                        nc.scalar.activation(sq[:, 0:cl], hsl, AF.Square)
                    elif dc < 7:
                        nc.vector.tensor_mul(sq[:, 0:cl], hsl, hsl)
                    else:
                        nc.gpsimd.tensor_mul(sq[:, 0:cl], hsl, hsl)
                    nc.tensor.matmul(ps1[:, 0:cl], lhsT=ones_r, rhs=hsl,
                                     start=(dc == 0), stop=(dc == DC - 1))
                    nc.tensor.matmul(ps2[:, 0:cl], lhsT=ones_b,
                                     rhs=sq[:, 0:cl],
                                     start=(dc == 0), stop=(dc == DC - 1))
                st = smp.tile([P, 2 * 512], F32, tag="st", name="st")
                stb = smp.tile([P, 2 * 512], BF16, tag="stb", name="stb")
                mean = st[0:1, 0:cl]
                var = st[0:1, 512:512 + cl]
                rstd = stb[0:1, 0:cl]
                mr = stb[0:1, 512:512 + cl]
                nc.vector.tensor_scalar_mul(mean, ps1[:, 0:cl], 1.0 / D)
                nc.vector.tensor_mul(var, mean, mean)
                nc.vector.scalar_tensor_tensor(
                    var, ps2[:, 0:cl], 1.0 / D, var,
                    op0=OP.mult, op1=OP.subtract)
                nc.scalar.activation(var, var, AF.Sqrt, bias=eps_t)
                nc.vector.reciprocal(rstd, var)
                nc.vector.tensor_mul(mr, mean, rstd)
                RM = dvp.tile([P, 2 * 512], BF16, tag="rm", name="RM", bufs=3)
                nc.gpsimd.partition_broadcast(RM[:, 0:cl], rstd)
                nc.gpsimd.partition_broadcast(RM[:, 512:512 + cl], mr)
                rms[ci] = RM

            def ln_norm(rms, gi, out_tile, ci, t0, cl):
                RM = rms[ci]
                for dc in range(DC):
                    hsl = hT[:, dc * ptl + t0:dc * ptl + t0 + cl]
                    d1 = lnp.tile([P, 512], BF16, tag="d1", name=f"d1_{dc}")
                    eng = nc.gpsimd if dc >= 6 else nc.vector
                    eng.tensor_mul(d1[:, 0:cl], hsl, RM[:, 0:cl])
                    osl = out_tile[:, dc * ptl + t0:dc * ptl + t0 + cl]
                    if use_lng and gi is not None:
                        d2 = lnp.tile([P, 512], BF16, tag="d2",
                                      name=f"d2_{dc}")
                        nc.vector.tensor_sub(d2[:, 0:cl], d1[:, 0:cl],
                                             RM[:, 512:512 + cl])
                        g0 = 8 * (2 * gi)
                        nc.vector.tensor_scalar(
                            osl, d2[:, 0:cl],
                            lngb[:, g0 + dc:g0 + dc + 1],
                            lngb[:, g0 + 8 + dc:g0 + 8 + dc + 1],
                            op0=OP.mult, op1=OP.add)
                    else:
                        nc.vector.tensor_sub(osl, d1[:, 0:cl],
                                             RM[:, 512:512 + cl])

            def ln_chunk(gi, out_tile, t0, cl):
                rms = {}
                ln_stats(rms, 0, t0, cl)
                ln_norm(rms, gi, out_tile, 0, t0, cl)

            def ln_pass(gi, out_tile):
                for (t0, cl) in chs:
                    ln_chunk(gi, out_tile, t0, cl)

            # ---------- embeddings ----------
            for dc in range(DC):
                nc.sync.dma_start(out=hT[:, dc * ptl:(dc + 1) * ptl],
                                  in_=baseT[:, dc * ptl:(dc + 1) * ptl])
            rms0 = {}
            for ci, (t0, cl) in enumerate(chs):
                for dc in range(DC):
                    pse = pp.tile([P, 512], F32, tag="mm", name="pse")
                    for vc in range(VC):
                        nc.tensor.matmul(
                            pse[:, 0:cl],
                            lhsT=tesb[:, vc * D + dc * 128:vc * D + dc * 128 + 128],
                            rhs=ohsb[:, vc * ptl + t0:vc * ptl + t0 + cl],
                            start=(vc == 0), stop=(vc == VC - 1))
                    hsl = hT[:, dc * ptl + t0:dc * ptl + t0 + cl]
                    nc.vector.tensor_add(hsl, pse[:, 0:cl], hsl)
                ln_stats(rms0, ci, t0, cl)
            for ci, (t0, cl) in enumerate(chs):
                ln_norm(rms0, None, hT, ci, t0, cl)   # LN0 in place

            # ---------- layers ----------
            for l in range(L):
                xh = xhp.tile([P, DC * ptl], BF16, tag="xh", name=f"xh{l}a")

                # ---- K/Q/V + attention, interleaved ----
                KT = big.tile([P, DC * ptl], BF16, tag="b18", name=f"KT{l}")
                Vsb = big.tile([P, nt * H * 65], BF16, tag="b18", name=f"Vsb{l}")
                QT = big.tile([P, DC * ptl], BF16, tag="b18", name=f"QT{l}")
                ctxc = big.tile([P, DC * ptl], BF16, tag="b18", name=f"ctx{l}")
                if lt < ptl:
                    nc.vector.memset(
                        Vsb[:, (nt - 1) * H * 65:nt * H * 65], 0.0)
                ones_v = Vsb.rearrange("p (g x) -> p g x", x=65)[:, :, 64:65]
                nc.vector.memset(ones_v, 1.0)

                def v_tg(nh, tg, norm=None):
                    if norm is not None:
                        rms_, gi_, t0_, cl_ = norm
                        RM = rms_
                    if True:
                        tts = [t for t in range(4 * tg, min(4 * tg + 4, nt))
                               if lt - t * P > 0]
                        pvs = {}
                        for tt in tts:
                            pvs[tt] = pp.tile([P, 512], F32, tag="mm",
                                              name=f"psv{tt}_{nh}")
                        for dc in range(DC):
                            if norm is not None:
                                hsl = hT[:, dc * ptl + t0_:dc * ptl + t0_ + cl_]
                                d1 = lnp.tile([P, 512], BF16, tag="d1",
                                              name=f"d1v{dc}")
                                eng = nc.gpsimd if dc >= 6 else nc.vector
                                eng.tensor_mul(d1[:, 0:cl_], hsl, RM[:, 0:cl_])
                                nc.vector.tensor_sub(
                                    xh[:, dc * ptl + t0_:dc * ptl + t0_ + cl_],
                                    d1[:, 0:cl_], RM[:, 512:512 + cl_])
                            vrb = wcb.tile([P, 512], BF16, tag="w",
                                           name=f"vrb{nh}_{tg}_{dc}")
                            nc.sync.dma_start(
                                out=vrb,
                                in_=vrb_d[l][:, (nh * DC + dc) * 512:
                                             (nh * DC + dc + 1) * 512])
                            for tt in tts:
                                tl = min(P, lt - tt * P)
                                nc.tensor.matmul(
                                    pvs[tt][0:tl, :],
                                    lhsT=xh[:, dc * ptl + tt * P:dc * ptl + tt * P + tl],
                                    rhs=vrb,
                                    start=(dc == 0), stop=(dc == DC - 1))
                        for tt in tts:
                            tl = min(P, lt - tt * P)
                            pv = pvs[tt][0:tl, :].rearrange(
                                "p (h x) -> p h x", h=8)
                            ov = Vsb[0:tl, (tt * H + nh * 8) * 65:
                                     (tt * H + nh * 8 + 8) * 65].rearrange(
                                "p (h x) -> p h x", x=65)[:, :, 0:64]
                            nc.scalar.copy(ov, pv)

                def kq_block(oc):
                    kcb = wcb.tile([P, DC * 128], BF16, tag="w",
                                   name=f"kcb{oc}")
                    nc.sync.dma_start(
                        out=kcb, in_=kcb_d[l][:, oc * D:(oc + 1) * D])
                    for (t0, cl) in chs:
                        ps = pp.tile([P, 512], F32, tag="mm", name=f"psk{oc}")
                        for dc in range(DC):
                            nc.tensor.matmul(
                                ps[:, 0:cl],
                                lhsT=kcb[:, dc * 128:dc * 128 + 128],
                                rhs=xh[:, dc * ptl + t0:dc * ptl + t0 + cl],
                                start=(dc == 0), stop=(dc == DC - 1))
                        nc.vector.tensor_scalar_add(
                            KT[:, oc * ptl + t0:oc * ptl + t0 + cl],
                            ps[:, 0:cl], bcol(l, 1)[:, oc:oc + 1])
                    qcb = wcb.tile([P, DC * 128], BF16, tag="w",
                                   name=f"qcb{oc}")
                    nc.sync.dma_start(
                        out=qcb, in_=qcb_d[l][:, oc * D:(oc + 1) * D])
                    for (t0, cl) in chs:
                        ps = pp.tile([P, 512], F32, tag="mm", name=f"psq{oc}")
                        for dc in range(DC):
                            nc.tensor.matmul(
                                ps[:, 0:cl],
                                lhsT=qcb[:, dc * 128:dc * 128 + 128],
                                rhs=xh[:, dc * ptl + t0:dc * ptl + t0 + cl],
                                start=(dc == 0), stop=(dc == DC - 1))
                        nc.scalar.activation(
                            QT[:, oc * ptl + t0:oc * ptl + t0 + cl],
                            ps[:, 0:cl], AF.Identity,
                            bias=bcol(l, 0)[:, oc:oc + 1])
                    if lt < ptl:
                        nc.vector.memset(KT[:, oc * ptl + lt:(oc + 1) * ptl],
                                         0.0)
                        nc.vector.memset(QT[:, oc * ptl + lt:(oc + 1) * ptl],
                                         0.0)

                def head_scores(h):
                    dch, po = h // 2, (h % 2) * 64
                    est = estp.tile([P, nt * EW], BF16, tag="est",
                                    name=f"est{h}")
                    ests[h] = est
                    for j in range(nt):
                        if w64:
                            w0 = min(max(j * P - 64, 0), ptl - EW)
                            nq = EW
                            lo = w0
                        else:
                            loj = max(j - 1, 0)
                            hi = min(j + 1, nt - 1)
                            nq = (hi - loj + 1) * P
                            w0 = min(max(j - 1, 0), nt - 3) * P
                            lo = loj * P
                        pst = pp.tile([P, 384], F32, tag="mm", name=f"pst{j}")
                        nc.tensor.matmul(
                            pst[:, 0:nq],
                            lhsT=KT[po:po + 64, dch * ptl + j * P:dch * ptl + j * P + P],
                            rhs=QT[po:po + 64, dch * ptl + lo:dch * ptl + lo + nq],
                            start=True, stop=True)
                        esl = est[:, j * EW + (lo - w0):j * EW + (lo - w0) + nq]
                        nc.scalar.activation(esl, pst[:, 0:nq], AF.Exp,
                                             scale=float(SCALE))
                    nc.vector.tensor_mul(est, est, masks)

                def head_ctx(h):
                    dch, po = h // 2, (h % 2) * 64
                    est = ests[h]
                    for qg in range((nt + 3) // 4):
                        qts = [q for q in range(4 * qg, min(4 * qg + 4, nt))]
                        gw = len(qts) * P
                        psc = pp.tile([65, 512], F32, tag="mm", name=f"psc{qg}")
                        for qi, qt in enumerate(qts):
                            if w64:
                                # left half [qt*128, +64): j=qt-1 then qt
                                # right half [qt*128+64, +64): j=qt then qt+1
                                for half in range(2):
                                    qlo = qt * P + 64 * half
                                    oc_ = psc[:, qi * P + 64 * half:
                                              qi * P + 64 * half + 64]
                                    js = ([qt - 1, qt] if half == 0
                                          else [qt, qt + 1])
                                    js = [j for j in js if 0 <= j < nt]
                                    for kk, j in enumerate(js):
                                        w0 = min(max(j * P - 64, 0), ptl - EW)
                                        rsl = est[:, j * EW + qlo - w0:
                                                  j * EW + qlo - w0 + 64]
                                        nc.tensor.matmul(
                                            oc_,
                                            lhsT=Vsb[:, (j * H + h) * 65:
                                                     (j * H + h) * 65 + 65],
                                            rhs=rsl,
                                            start=(kk == 0),
                                            stop=(kk == len(js) - 1))
                            else:
                                js = [j for j in (qt - 1, qt, qt + 1)
                                      if 0 <= j < nt]
                                for kk, j in enumerate(js):
                                    w0 = min(max(j - 1, 0), nt - 3) * P
                                    rsl = est[:, j * EW + qt * P - w0:
                                              j * EW + qt * P - w0 + P]
                                    nc.tensor.matmul(
                                        psc[:, qi * P:(qi + 1) * P],
                                        lhsT=Vsb[:, (j * H + h) * 65:
                                                 (j * H + h) * 65 + 65],
                                        rhs=rsl,
                                        start=(kk == 0), stop=(kk == len(js) - 1))
                        dinv = dvp.tile([1, 512], BF16, tag="dinv",
                                        name=f"dinv{qg}")
                        nc.vector.reciprocal(dinv[:, 0:gw], psc[64:65, 0:gw])
                        dnb = dvp.tile([P, 512], BF16, tag="dnb",
                                       name=f"dnb{qg}")
                        nc.gpsimd.partition_broadcast(dnb[0:64, 0:gw],
                                                      dinv[:, 0:gw])
                        nc.vector.tensor_mul(
                            ctxc[po:po + 64,
                                 dch * ptl + qg * 512:dch * ptl + qg * 512 + gw],
                            psc[0:64, 0:gw], dnb[0:64, 0:gw])

                ests = {}
                gi1 = 2 * l if use_lng else None
                rms1 = {}
                for ci, (t0, cl) in enumerate(chs):
                    ln_stats(rms1, ci, t0, cl)
                for ci, (t0, cl) in enumerate(chs):
                    if use_lng:
                        ln_norm(rms1, gi1, xh, ci, t0, cl)
                        v_tg(0, ci)
                    else:
                        v_tg(0, ci, norm=(rms1[ci], gi1, t0, cl))
                kq_block(0)
                for oc in range(1, DC):
                    if oc == 5:
                        for ci in range(len(chs)):
                            v_tg(1, ci)
                    head_scores(2 * oc - 2)
                    head_scores(2 * oc - 1)
                    kq_block(oc)
                    head_ctx(2 * oc - 2)
                    head_ctx(2 * oc - 1)
                head_scores(14)
                head_scores(15)
                head_ctx(14)
                head_ctx(15)

                # ---- O-projection (chunk-outer) + residual + LN2 ----
                xh = xhp.tile([P, DC * ptl], BF16, tag="xh", name=f"xh{l}b")
                gi2 = 2 * l + 1 if use_lng else None
                rms2 = {}
                for ci, (t0, cl) in enumerate(chs):
                    for do_ in range(DC):
                        ocb = wcb.tile([P, DC * 128], BF16, tag="w",
                                       name=f"ocb{ci}_{do_}")
                        nc.sync.dma_start(
                            out=ocb, in_=ocb_d[l][:, do_ * D:(do_ + 1) * D])
                        ps = pp.tile([P, 512], F32, tag="mm", name=f"pso{do_}")
                        for dc in range(DC):
                            nc.tensor.matmul(
                                ps[:, 0:cl], lhsT=ocb[:, dc * 128:dc * 128 + 128],
                                rhs=ctxc[:, dc * ptl + t0:dc * ptl + t0 + cl],
                                start=(dc == 0), stop=(dc == DC - 1))
                        hsl = hT[:, do_ * ptl + t0:do_ * ptl + t0 + cl]
                        nc.vector.scalar_tensor_tensor(
                            hsl, ps[:, 0:cl], bcol(l, 3)[:, do_:do_ + 1], hsl,
                            op0=OP.add, op1=OP.add)
                    ln_stats(rms2, ci, t0, cl)
                for ci, (t0, cl) in enumerate(chs):
                    ln_norm(rms2, gi2, xh, ci, t0, cl)

                # ---- FFN ----
                Us = [big.tile([P, 8 * ptl], BF16, tag="b18", name=f"U{l}_{i}")
                      for i in range(4)]

                def usl(fc, t0, cl):
                    t = Us[fc // 8]
                    k = fc % 8
                    return t[:, k * ptl + t0:k * ptl + t0 + cl]

                for fcb in range(8):
                    w1cb = wcb.tile([P, DC * 512], BF16, tag="w",
                                    name=f"w1cb{fcb}")
                    nc.sync.dma_start(
                        out=w1cb,
                        in_=w1cb_d[l][:, fcb * DC * 512:(fcb + 1) * DC * 512])
                    for fc2 in range(4):
                        fc = fcb * 4 + fc2
                        for (t0, cl) in chs:
                            ps = pp.tile([P, 512], F32, tag="mm",
                                         name=f"psf{fc2}")
                            for dc in range(DC):
                                nc.tensor.matmul(
                                    ps[:, 0:cl],
                                    lhsT=w1cb[:, dc * 512 + fc2 * 128:
                                              dc * 512 + fc2 * 128 + 128],
                                    rhs=xh[:, dc * ptl + t0:dc * ptl + t0 + cl],
                                    start=(dc == 0), stop=(dc == DC - 1))
                            bidx = 5 + fc // 8
                            nc.scalar.activation(
                                usl(fc, t0, cl), ps[:, 0:cl], AF.Gelu,
                                bias=bcol(l, bidx)[:, fc % 8:fc % 8 + 1])
                for do_ in range(DC):
                    w2cb = wcb.tile([P, FC * 128], BF16, tag="w",
                                    name=f"w2cb{do_}")
                    nc.sync.dma_start(
                        out=w2cb,
                        in_=w2cb_d[l][:, do_ * FC * 128:(do_ + 1) * FC * 128])
                    for (t0, cl) in chs:
                        ps = pp.tile([P, 512], F32, tag="mm", name=f"psh{do_}")
                        for fc in range(FC):
                            nc.tensor.matmul(
                                ps[:, 0:cl],
                                lhsT=w2cb[:, fc * 128:fc * 128 + 128],
                                rhs=usl(fc, t0, cl),
                                start=(fc == 0), stop=(fc == FC - 1))
                        hsl = hT[:, do_ * ptl + t0:do_ * ptl + t0 + cl]
                        nc.vector.scalar_tensor_tensor(
                            hsl, ps[:, 0:cl], bcol(l, 4)[:, do_:do_ + 1], hsl,
                            op0=OP.add, op1=OP.add)
                    if l == L - 1:
                        for (t0o, clo) in chs:
                            nc.sync.dma_start(
                                out=houtT[:, do_ * ptl + t0o:do_ * ptl + t0o + clo],
                                in_=hT[:, do_ * ptl + t0o:do_ * ptl + t0o + clo])

    nc.compile()
    return nc


_NC_CACHE = {}


def _get_nc(lt=1032, nt=9, use_lng=False, w64=True):
    key = (lt, nt, use_lng, w64)
    if key not in _NC_CACHE:
        _NC_CACHE[key] = _build(lt, nt, use_lng, w64)
    return _NC_CACHE[key]


def _pack_shared(inputs, lt, nt, use_lng):
    bf = np.dtype("bfloat16") if hasattr(np, "bfloat16") else None
    import ml_dtypes
    BFD = ml_dtypes.bfloat16

    def b16(x):
        return np.ascontiguousarray(np.asarray(x, np.float32).astype(BFD))

    tok = np.asarray(inputs["tok_emb"], np.float32)
    tokp = np.zeros((VP, D), np.float32)
    tokp[:tok.shape[0]] = tok
    tokemb = b16(tokp.reshape(VC, P, D).transpose(1, 0, 2).reshape(P, VC * D))

    shared = {"tokemb": tokemb}
    for l in range(L):
        Wq = np.asarray(inputs["Wq"][l], np.float32)
        Wk = np.asarray(inputs["Wk"][l], np.float32)
        Wv = np.asarray(inputs["Wv"][l], np.float32)
        Wo = np.asarray(inputs["Wo"][l], np.float32)
        W1 = np.asarray(inputs["W1"][l], np.float32)
        W2 = np.asarray(inputs["W2"][l], np.float32)

        def colblocks(W, ocn):  # [D, D] -> [P, ocn*DC*128]
            # block (oc): [p, dc, c] = W[dc*128+p, oc*128+c]
            Wr = W.reshape(DC, P, ocn, 128)  # [dc, p, oc, c]
            return np.ascontiguousarray(
                Wr.transpose(1, 2, 0, 3).reshape(P, ocn * DC * 128))

        shared[f"kcb{l}"] = b16(colblocks(Wk, DC))
        shared[f"qcb{l}"] = b16(colblocks(Wq, DC))
        shared[f"ocb{l}"] = b16(colblocks(Wo, DC))
        # vrb: [p, nh, dc, c] = Wv[dc*128+p, nh*512+c]
        Wvr = Wv.reshape(DC, P, 2, 512)
        shared[f"vrb{l}"] = b16(
            Wvr.transpose(1, 2, 0, 3).reshape(P, 2 * DC * 512))
        # w1cb: [p, fcb, dc, c] = W1[dc*128+p, fcb*512+c]
        W1r = W1.reshape(DC, P, 8, 512)
        shared[f"w1cb{l}"] = b16(
            W1r.transpose(1, 2, 0, 3).reshape(P, 8 * DC * 512))
        # w2cb: [p, do, fc, c] = W2[fc*128+p, do*128+c]
        W2r = W2.reshape(FC, P, DC, 128)
        shared[f"w2cb{l}"] = b16(
            W2r.transpose(1, 2, 0, 3).reshape(P, DC * FC * 128))

    cbw = np.zeros((P, 2 + 96 * L), np.float32)
    cbw[:, 0] = 1.0
    cbw[0, 1] = EPS
    for l in range(L):
        c0 = 2 + 96 * l
        # bv is folded into bo: probs sum to 1, so ctx@Wo + bo with V+bv
        # equals (ctx from plain V)@Wo + (bo + bv@Wo).
        bo_eff = (np.asarray(inputs["bo"][l], np.float32)
                  + np.asarray(inputs["bv"][l], np.float32)
                  @ np.asarray(inputs["Wo"][l], np.float32))
        vals = {"bq": np.asarray(inputs["bq"][l], np.float32),
                "bk": np.asarray(inputs["bk"][l], np.float32),
                "bv": np.zeros(D, np.float32),
                "bo": bo_eff,
                "b2": np.asarray(inputs["b2"][l], np.float32)}
        for i, key in enumerate(("bq", "bk", "bv", "bo", "b2")):
            cbw[:, c0 + 8 * i:c0 + 8 * i + 8] = vals[key].reshape(DC, P).T
        b1v = np.asarray(inputs["b1"][l], np.float32)
        cbw[:, c0 + 40:c0 + 72] = b1v.reshape(FC, P).T
    shared["cb"] = np.ascontiguousarray(cbw)

    if use_lng:
        gb = np.zeros((P, 8 * (2 + 4 * L)), np.float32)
        # group 0: ln0 (handled as gi=None in build... keep identity)
        idx = 0
        for l in range(L):
            for which in range(2):
                gi = 2 * l + which
                g = np.asarray(inputs["ln1_g" if which == 0 else "ln2_g"][l],
                               np.float32)
                bb = np.asarray(inputs["ln1_b" if which == 0 else "ln2_b"][l],
                                np.float32)
                gb[:, 8 * (2 * gi):8 * (2 * gi) + 8] = g.reshape(DC, P).T
                gb[:, 8 * (2 * gi + 1):8 * (2 * gi + 1) + 8] = bb.reshape(DC, P).T
        shared["lngb"] = np.ascontiguousarray(gb)
    return shared


def _prep_core(inputs, b, start, n, lt, nt, w64):
    import ml_dtypes
    BFD = ml_dtypes.bfloat16
    ptl = nt * P

    def b16(x):
        return np.ascontiguousarray(np.asarray(x, np.float32).astype(BFD))

    ids = np.asarray(inputs["input_ids"][b, start:start + n])
    pid = np.asarray(inputs["patch_ids"][b, start:start + n]).astype(np.int64)
    pos_emb = np.asarray(inputs["pos_emb"], np.float32)
    hashes = np.asarray(inputs["hash_embeddings"], np.float32)

    oh = np.zeros((VP, ptl), np.float32)
    oh[ids, np.arange(n)] = 1.0
    oht = b16(oh.reshape(VC, P, ptl).transpose(1, 0, 2).reshape(P, VC * ptl))

    base = np.zeros((ptl, D), np.float32)
    base[:n] = pos_emb[start:start + n] + hashes[b, start:start + n]
    baseT = np.ascontiguousarray(
        base.reshape(ptl, DC, P).transpose(2, 1, 0).reshape(P, DC * ptl))

    pidp = np.empty(ptl, np.int64)
    pidp[:n] = pid
    pidp[n:] = -np.arange(1, ptl - n + 1)

    ew = 256 if w64 else 384
    m = np.zeros((nt, P, ew), np.float32)
    for j in range(nt):
        if w64:
            w0 = int(np.clip(j * P - 64, 0, ptl - ew))
        else:
            w0 = int(np.clip(j - 1, 0, nt - 3)) * P
        kk = pidp[j * P:(j + 1) * P]
        qq = pidp[w0:w0 + ew]
        m[j] = (kk[:, None] == qq[None, :]).astype(np.float32)
    masks = b16(m.transpose(1, 0, 2).reshape(P, nt * ew))
    return {"oht": oht, "baseT": baseT, "masks": masks}


def kernel(**inputs):
    pid_all = np.asarray(inputs["patch_ids"])

    shards = []
    for b in range(B):
        pid = np.asarray(pid_all[b])
        bnd = np.nonzero(pid[1:] != pid[:-1])[0] + 1
        cand = bnd[(bnd >= S - 1152) & (bnd <= 1152)]
        if len(cand) == 0:
            raise RuntimeError("no patch boundary near S/2; cannot shard")
        s = int(cand[np.argmin(np.abs(cand - S // 2))])
        shards.append((b, 0, s))
        shards.append((b, s, S - s))

    lt = max(n for _, _, n in shards)
    lt = max(lt, 1026)  # floor so chunk 3 isn't degenerate-tiny
    nt = (lt + P - 1) // P

    maxrun = 0
    for b in range(B):
        p = np.asarray(pid_all[b])
        bnd = np.nonzero(p[1:] != p[:-1])[0] + 1
        edges = np.concatenate([[0], bnd, [len(p)]])
        maxrun = max(maxrun, int(np.diff(edges).max()))
    w64 = maxrun <= 64

    use_lng = not (
        all(np.all(np.asarray(inputs[k]) == 1.0)
            for k in ("ln0_g", "ln1_g", "ln2_g")) and
        all(np.all(np.asarray(inputs[k]) == 0.0)
            for k in ("ln0_b", "ln1_b", "ln2_b")))
    if use_lng:
        raise NotImplementedError(
            "non-identity LN affine not supported in fast path")

    shared = _pack_shared(inputs, lt, nt, use_lng)
    in_maps = []
    for b, start, n in shards:
        mcore = dict(shared)
        mcore.update(_prep_core(inputs, b, start, n, lt, nt, w64))
        in_maps.append(mcore)

    nc = _get_nc(lt, nt, use_lng, w64)
    res = bass_utils.run_bass_kernel_spmd(nc, in_maps,
                                          core_ids=list(range(NCORES)))

    ptl = nt * P
    out = np.zeros((B, S, D), np.float32)
    for i, (b, start, n) in enumerate(shards):
        ht = res.results[i]["houtT"]
        hfull = ht.reshape(P, DC, ptl).transpose(2, 1, 0).reshape(ptl, D)
        out[b, start:start + n] = hfull[:n]
    return out


if __name__ == "__main__":
    import sys
    lt = int(sys.argv[1]) if len(sys.argv) > 1 else 1032
    _get_nc(lt, (lt + P - 1) // P, False)
    print("built ok")


# revision 24
# speedup vs baseline: 1.3876x; 1.0436x over previous
"""BLT local encoder (2-layer transformer, patch-equality block-diagonal attention)
on 8 Trainium2 NeuronCores.

v2. Sharding: each of the 4 sequences splits at a patch-run boundary nearest
S/2 -> 8 independent shards, one per core, zero cross-core communication.

Kernel design (per core, L_tok = max shard length ~1032):
- Residual hT kept float32 feature-major [P, 8dc x PTL]; everything else bf16.
- Weights prepacked host-side into SBUF-ready bf16 col/row blocks, streamed
  once per layer (no restreaming), double-buffered.
- One LayerNorm per sublayer, output xh bf16 reused by Q, K and V.
- Full-shard attention: per (head, key-tile j) one score matmul with moving
  dim >= 256; softmax denominator via a ones-column appended to V (row 64 of
  the ctx psum); per-head normalize fused into the psum->SBUF copy.
- Engine split: PE matmuls; DVE normalize/copies/masks; Act square/exp/gelu;
  Pool partition-broadcasts + residual adds.
"""

import numpy as np

import concourse.bass as bass
import concourse.tile as tile
from concourse import bacc, bass_utils, mybir

F32 = mybir.dt.float32
F32R = mybir.dt.float32r
BF16 = mybir.dt.bfloat16
AF = mybir.ActivationFunctionType
OP = mybir.AluOpType

B, S, D, H, F, L = 4, 2048, 1024, 16, 4096, 2
DH = D // H      # 64
DC = D // 128    # 8
FC = F // 128    # 32
EPS = 1e-5
SCALE = 1.0 / np.sqrt(DH)
P = 128
VP = 384         # vocab 260 padded
VC = VP // 128   # 3
NCORES = 8


def _chunks(lt):
    out = []
    o = 0
    while o < lt:
        c = min(512, lt - o)
        out.append((o, c))
        o += c
    return out


def _build(lt, nt, use_lng, w64):
    """lt: tokens; nt: tiles; use_lng: ln affine ops; w64: +-64-token window."""
    ptl = nt * P
    EW = 256 if w64 else 384
    chs = _chunks(lt)
    nc = bacc.Bacc("TRN2", target_bir_lowering=False, debug=False,
                   num_devices=NCORES)

    def din(name, shape, dt=BF16):
        return nc.dram_tensor(name, shape, dt, kind="ExternalInput").ap()

    baseT = din("baseT", [P, DC * ptl], F32R)
    masks_d = din("masks", [P, nt * EW])
    # prepacked weights
    kcb_d, qcb_d, ocb_d, vrb_d, w1cb_d, w2cb_d = [], [], [], [], [], []
    for l in range(L):
        kcb_d.append(din(f"kcb{l}", [P, DC * DC * 128]))
        qcb_d.append(din(f"qcb{l}", [P, DC * DC * 128]))
        ocb_d.append(din(f"ocb{l}", [P, DC * DC * 128]))
        vrb_d.append(din(f"vrb{l}", [P, DC * D]))
        w1cb_d.append(din(f"w1cb{l}", [P, 8 * DC * 512]))
        w2cb_d.append(din(f"w2cb{l}", [P, DC * FC * 128]))
    # packed per-feature consts: [P, col] layout, 8 cols per D-vector
    # cols: 0 ones | 1 eps(row0) | then per layer l at 2+64*l:
    #   bq 0:8 bk 8:16 bv 16:24 bo 24:32 b2 32:40 b1 40:72 (unused gap)
    # ln g/b (if use_lng): separate tensor lngb
    cb_d = din("cb", [P, 2 + 96 * L], F32)
    lngb_d = din("lngb", [P, 8 * (2 + 4 * L)], F32) if use_lng else None
    houtT = nc.dram_tensor("houtT", [P, DC * ptl], F32R,
                           kind="ExternalOutput").ap()

    with tile.TileContext(nc) as tc:
        with (
            nc.allow_low_precision(
                reason="bf16 softmax/LN staging validated vs reference"),
            tc.tile_pool(name="pers", bufs=1) as pers,
            tc.tile_pool(name="big", bufs=4) as big,
            tc.tile_pool(name="xhp", bufs=1) as xhp,
            tc.tile_pool(name="wcb", bufs=4) as wcb,
            tc.tile_pool(name="est", bufs=2) as estp,
            tc.tile_pool(name="lnt", bufs=3) as lnp,
            tc.tile_pool(name="sm", bufs=2) as smp,
            tc.tile_pool(name="dv", bufs=2) as dvp,
            tc.tile_pool(name="pp", bufs=8, space="PSUM") as pp,
        ):
            cb = pers.tile([P, 2 + 96 * L], F32, tag="cb")
            nc.sync.dma_start(out=cb, in_=cb_d)
            eps_t = cb[0:1, 1:2]
            ones_r = pers.tile([P, 1], F32R, tag="ones_r")
            nc.vector.tensor_copy(ones_r, cb[:, 0:1])
            ones_b = pers.tile([P, 1], BF16, tag="ones_b")
            nc.vector.tensor_copy(ones_b, cb[:, 0:1])
            if use_lng:
                lngb = pers.tile([P, 8 * (2 + 4 * L)], F32, tag="lngb")
                nc.sync.dma_start(out=lngb, in_=lngb_d)

            masks = pers.tile([P, nt * EW], BF16, tag="masks")
            nc.sync.dma_start(out=masks, in_=masks_d)

            hT = pers.tile([P, DC * ptl], F32R, tag="hT")

            def bcol(l, i):  # bias col i (in 8-col groups) for layer l
                c0 = 2 + 96 * l + 8 * i
                return cb[:, c0:c0 + 8]

            def ln_stats(rms, ci, t0, cl):
                ps1 = pp.tile([1, 512], F32, tag="mm", name="lns1")
                ps2 = pp.tile([1, 512], F32, tag="mm", name="lns2")
                for dc in range(DC):
                    hsl = hT[:, dc * ptl + t0:dc * ptl + t0 + cl]
                    sq = lnp.tile([P, 512], BF16, tag="sq", name=f"sq{dc}")
                    if dc < 4:
                        nc.scalar.activation(sq[:, 0:cl], hsl, AF.Square)
                    elif dc < 7:
                        nc.vector.tensor_mul(sq[:, 0:cl], hsl, hsl)
                    else:
                        nc.gpsimd.tensor_mul(sq[:, 0:cl], hsl, hsl)
                    nc.tensor.matmul(ps1[:, 0:cl], lhsT=ones_r, rhs=hsl,
                                     start=(dc == 0), stop=(dc == DC - 1))
                    nc.tensor.matmul(ps2[:, 0:cl], lhsT=ones_b,
                                     rhs=sq[:, 0:cl],
                                     start=(dc == 0), stop=(dc == DC - 1))
                st = smp.tile([P, 2 * 512], F32, tag="st", name="st")
                stb = smp.tile([P, 2 * 512], BF16, tag="stb", name="stb")
                mean = st[0:1, 0:cl]
                var = st[0:1, 512:512 + cl]
                rstd = stb[0:1, 0:cl]
                mr = stb[0:1, 512:512 + cl]
                nc.vector.tensor_scalar_mul(mean, ps1[:, 0:cl], 1.0 / D)
                nc.vector.tensor_mul(var, mean, mean)
                nc.vector.scalar_tensor_tensor(
                    var, ps2[:, 0:cl], 1.0 / D, var,
                    op0=OP.mult, op1=OP.subtract)
                nc.scalar.activation(var, var, AF.Sqrt, bias=eps_t)
                nc.vector.reciprocal(rstd, var)
                nc.vector.tensor_mul(mr, mean, rstd)
                RM = dvp.tile([P, 2 * 512], BF16, tag="rm", name="RM", bufs=3)
                nc.gpsimd.partition_broadcast(RM[:, 0:cl], rstd)
                nc.gpsimd.partition_broadcast(RM[:, 512:512 + cl], mr)
                rms[ci] = RM

            def ln_norm(rms, gi, out_tile, ci, t0, cl):
                RM = rms[ci]
                for dc in range(DC):
                    hsl = hT[:, dc * ptl + t0:dc * ptl + t0 + cl]
                    d1 = lnp.tile([P, 512], BF16, tag="d1", name=f"d1_{dc}")
                    eng = nc.gpsimd if dc >= 6 else nc.vector
                    eng.tensor_mul(d1[:, 0:cl], hsl, RM[:, 0:cl])
                    osl = out_tile[:, dc * ptl + t0:dc * ptl + t0 + cl]
                    if use_lng and gi is not None:
                        d2 = lnp.tile([P, 512], BF16, tag="d2",
                                      name=f"d2_{dc}")
                        nc.vector.tensor_sub(d2[:, 0:cl], d1[:, 0:cl],
                                             RM[:, 512:512 + cl])
                        g0 = 8 * (2 * gi)
                        nc.vector.tensor_scalar(
                            osl, d2[:, 0:cl],
                            lngb[:, g0 + dc:g0 + dc + 1],
                            lngb[:, g0 + 8 + dc:g0 + 8 + dc + 1],
                            op0=OP.mult, op1=OP.add)
                    else:
                        nc.vector.tensor_sub(osl, d1[:, 0:cl],
                                             RM[:, 512:512 + cl])

            # ---------- initial residual (host LN0(emb)) ----------
            for dc in range(DC):
                nc.sync.dma_start(out=hT[:, dc * ptl:(dc + 1) * ptl],
                                  in_=baseT[:, dc * ptl:(dc + 1) * ptl])

            # ---------- layers ----------
            for l in range(L):
                xh = xhp.tile([P, DC * ptl], BF16, tag="xh", name=f"xh{l}a")

                # ---- K/Q/V + attention, interleaved ----
                KT = big.tile([P, DC * ptl], BF16, tag="b18", name=f"KT{l}")
                Vsb = big.tile([P, nt * H * 65], BF16, tag="b18", name=f"Vsb{l}")
                QT = big.tile([P, DC * ptl], BF16, tag="b18", name=f"QT{l}")
                ctxc = big.tile([P, DC * ptl], BF16, tag="b18", name=f"ctx{l}")
                if lt < ptl:
                    nc.vector.memset(
                        Vsb[:, (nt - 1) * H * 65:nt * H * 65], 0.0)
                ones_v = Vsb.rearrange("p (g x) -> p g x", x=65)[:, :, 64:65]
                nc.vector.memset(ones_v, 1.0)

                def v_tg(nh, tg, norm=None):
                    if norm is not None:
                        rms_, gi_, t0_, cl_ = norm
                        RM = rms_
                    if True:
                        tts = [t for t in range(4 * tg, min(4 * tg + 4, nt))
                               if lt - t * P > 0]
                        pvs = {}
                        for tt in tts:
                            pvs[tt] = pp.tile([P, 512], F32, tag="mm",
                                              name=f"psv{tt}_{nh}")
                        for dc in range(DC):
                            if norm is not None:
                                hsl = hT[:, dc * ptl + t0_:dc * ptl + t0_ + cl_]
                                d1 = lnp.tile([P, 512], BF16, tag="d1",
                                              name=f"d1v{dc}")
                                eng = nc.gpsimd if dc >= 6 else nc.vector
                                eng.tensor_mul(d1[:, 0:cl_], hsl, RM[:, 0:cl_])
                                nc.vector.tensor_sub(
                                    xh[:, dc * ptl + t0_:dc * ptl + t0_ + cl_],
                                    d1[:, 0:cl_], RM[:, 512:512 + cl_])
                            vrb = wcb.tile([P, 512], BF16, tag="w",
                                           name=f"vrb{nh}_{tg}_{dc}")
                            nc.sync.dma_start(
                                out=vrb,
                                in_=vrb_d[l][:, (nh * DC + dc) * 512:
                                             (nh * DC + dc + 1) * 512])
                            for tt in tts:
                                tl = min(P, lt - tt * P)
                                nc.tensor.matmul(
                                    pvs[tt][0:tl, :],
                                    lhsT=xh[:, dc * ptl + tt * P:dc * ptl + tt * P + tl],
                                    rhs=vrb,
                                    start=(dc == 0), stop=(dc == DC - 1))
                        for tt in tts:
                            tl = min(P, lt - tt * P)
                            pv = pvs[tt][0:tl, :].rearrange(
                                "p (h x) -> p h x", h=8)
                            ov = Vsb[0:tl, (tt * H + nh * 8) * 65:
                                     (tt * H + nh * 8 + 8) * 65].rearrange(
                                "p (h x) -> p h x", x=65)[:, :, 0:64]
                            nc.scalar.copy(ov, pv)

                def kq_block(oc):
                    kcb = wcb.tile([P, DC * 128], BF16, tag="w",
                                   name=f"kcb{oc}")
                    nc.sync.dma_start(
                        out=kcb, in_=kcb_d[l][:, oc * D:(oc + 1) * D])
                    for (t0, cl) in chs:
                        ps = pp.tile([P, 512], F32, tag="mm", name=f"psk{oc}")
                        for dc in range(DC):
                            nc.tensor.matmul(
                                ps[:, 0:cl],
                                lhsT=kcb[:, dc * 128:dc * 128 + 128],
                                rhs=xh[:, dc * ptl + t0:dc * ptl + t0 + cl],
                                start=(dc == 0), stop=(dc == DC - 1))
                        nc.vector.tensor_scalar_add(
                            KT[:, oc * ptl + t0:oc * ptl + t0 + cl],
                            ps[:, 0:cl], bcol(l, 1)[:, oc:oc + 1])
                    qcb = wcb.tile([P, DC * 128], BF16, tag="w",
                                   name=f"qcb{oc}")
                    nc.sync.dma_start(
                        out=qcb, in_=qcb_d[l][:, oc * D:(oc + 1) * D])
                    for (t0, cl) in chs:
                        ps = pp.tile([P, 512], F32, tag="mm", name=f"psq{oc}")
                        for dc in range(DC):
                            nc.tensor.matmul(
                                ps[:, 0:cl],
                                lhsT=qcb[:, dc * 128:dc * 128 + 128],
                                rhs=xh[:, dc * ptl + t0:dc * ptl + t0 + cl],
                                start=(dc == 0), stop=(dc == DC - 1))
                        nc.scalar.activation(
                            QT[:, oc * ptl + t0:oc * ptl + t0 + cl],
                            ps[:, 0:cl], AF.Identity,
                            bias=bcol(l, 0)[:, oc:oc + 1])
                    if lt < ptl:
                        nc.vector.memset(KT[:, oc * ptl + lt:(oc + 1) * ptl],
                                         0.0)
                        nc.vector.memset(QT[:, oc * ptl + lt:(oc + 1) * ptl],
                                         0.0)

                def head_scores(h):
                    dch, po = h // 2, (h % 2) * 64
                    est = estp.tile([P, nt * EW], BF16, tag="est",
                                    name=f"est{h}")
                    ests[h] = est
                    for j in range(nt):
                        if w64:
                            w0 = min(max(j * P - 64, 0), ptl - EW)
                            nq = EW
                            lo = w0
                        else:
                            loj = max(j - 1, 0)
                            hi = min(j + 1, nt - 1)
                            nq = (hi - loj + 1) * P
                            w0 = min(max(j - 1, 0), nt - 3) * P
                            lo = loj * P
                        pst = pp.tile([P, 384], F32, tag="mm", name=f"pst{j}")
                        nc.tensor.matmul(
                            pst[:, 0:nq],
                            lhsT=KT[po:po + 64, dch * ptl + j * P:dch * ptl + j * P + P],
                            rhs=QT[po:po + 64, dch * ptl + lo:dch * ptl + lo + nq],
                            start=True, stop=True)
                        esl = est[:, j * EW + (lo - w0):j * EW + (lo - w0) + nq]
                        nc.scalar.activation(esl, pst[:, 0:nq], AF.Exp,
                                             scale=float(SCALE))
                    nc.vector.tensor_mul(est, est, masks)

                def head_ctx(h):
                    dch, po = h // 2, (h % 2) * 64
                    est = ests[h]
                    for qg in range((nt + 3) // 4):
                        qts = [q for q in range(4 * qg, min(4 * qg + 4, nt))]
                        gw = len(qts) * P
                        psc = pp.tile([65, 512], F32, tag="mm", name=f"psc{qg}")
                        for qi, qt in enumerate(qts):
                            if w64:
                                # left half [qt*128, +64): j=qt-1 then qt
                                # right half [qt*128+64, +64): j=qt then qt+1
                                for half in range(2):
                                    qlo = qt * P + 64 * half
                                    oc_ = psc[:, qi * P + 64 * half:
                                              qi * P + 64 * half + 64]
                                    js = ([qt - 1, qt] if half == 0
                                          else [qt, qt + 1])
                                    js = [j for j in js if 0 <= j < nt]
                                    for kk, j in enumerate(js):
                                        w0 = min(max(j * P - 64, 0), ptl - EW)
                                        rsl = est[:, j * EW + qlo - w0:
                                                  j * EW + qlo - w0 + 64]
                                        nc.tensor.matmul(
                                            oc_,
                                            lhsT=Vsb[:, (j * H + h) * 65:
                                                     (j * H + h) * 65 + 65],
                                            rhs=rsl,
                                            start=(kk == 0),
                                            stop=(kk == len(js) - 1))
                            else:
                                js = [j for j in (qt - 1, qt, qt + 1)
                                      if 0 <= j < nt]
                                for kk, j in enumerate(js):
                                    w0 = min(max(j - 1, 0), nt - 3) * P
                                    rsl = est[:, j * EW + qt * P - w0:
                                              j * EW + qt * P - w0 + P]
                                    nc.tensor.matmul(
                                        psc[:, qi * P:(qi + 1) * P],
                                        lhsT=Vsb[:, (j * H + h) * 65:
                                                 (j * H + h) * 65 + 65],
                                        rhs=rsl,
                                        start=(kk == 0), stop=(kk == len(js) - 1))
                        dinv = dvp.tile([1, 512], BF16, tag="dinv",
                                        name=f"dinv{qg}")
                        nc.vector.reciprocal(dinv[:, 0:gw], psc[64:65, 0:gw])
                        dnb = dvp.tile([P, 512], BF16, tag="dnb",
                                       name=f"dnb{qg}")
                        nc.gpsimd.partition_broadcast(dnb[0:64, 0:gw],
                                                      dinv[:, 0:gw])
                        nc.vector.tensor_mul(
                            ctxc[po:po + 64,
                                 dch * ptl + qg * 512:dch * ptl + qg * 512 + gw],
                            psc[0:64, 0:gw], dnb[0:64, 0:gw])

                ests = {}
                gi1 = 2 * l if use_lng else None
                rms1 = {}
                for ci, (t0, cl) in enumerate(chs):
                    ln_stats(rms1, ci, t0, cl)
                for ci, (t0, cl) in enumerate(chs):
                    if use_lng:
                        ln_norm(rms1, gi1, xh, ci, t0, cl)
                        v_tg(0, ci)
                    else:
                        v_tg(0, ci, norm=(rms1[ci], gi1, t0, cl))
                kq_block(0)
                for oc in range(1, DC):
                    if oc == 5:
                        for ci in range(len(chs)):
                            v_tg(1, ci)
                    head_scores(2 * oc - 2)
                    head_scores(2 * oc - 1)
                    kq_block(oc)
                    head_ctx(2 * oc - 2)
                    head_ctx(2 * oc - 1)
                head_scores(14)
                head_scores(15)
                head_ctx(14)
                head_ctx(15)

                # ---- O-projection (chunk-outer) + residual + LN2 ----
                xh = xhp.tile([P, DC * ptl], BF16, tag="xh", name=f"xh{l}b")
                gi2 = 2 * l + 1 if use_lng else None
                rms2 = {}
                for ci, (t0, cl) in enumerate(chs):
                    for do_ in range(DC):
                        ocb = wcb.tile([P, DC * 128], BF16, tag="w",
                                       name=f"ocb{ci}_{do_}")
                        nc.sync.dma_start(
                            out=ocb, in_=ocb_d[l][:, do_ * D:(do_ + 1) * D])
                        ps = pp.tile([P, 512], F32, tag="mm", name=f"pso{do_}")
                        for dc in range(DC):
                            nc.tensor.matmul(
                                ps[:, 0:cl], lhsT=ocb[:, dc * 128:dc * 128 + 128],
                                rhs=ctxc[:, dc * ptl + t0:dc * ptl + t0 + cl],
                                start=(dc == 0), stop=(dc == DC - 1))
                        hsl = hT[:, do_ * ptl + t0:do_ * ptl + t0 + cl]
                        nc.vector.scalar_tensor_tensor(
                            hsl, ps[:, 0:cl], bcol(l, 3)[:, do_:do_ + 1], hsl,
                            op0=OP.add, op1=OP.add)
                    ln_stats(rms2, ci, t0, cl)
                for ci, (t0, cl) in enumerate(chs):
                    ln_norm(rms2, gi2, xh, ci, t0, cl)

                # ---- FFN ----
                Us = [big.tile([P, 8 * ptl], BF16, tag="b18", name=f"U{l}_{i}")
                      for i in range(4)]

                def usl(fc, t0, cl):
                    t = Us[fc // 8]
                    k = fc % 8
                    return t[:, k * ptl + t0:k * ptl + t0 + cl]

                for fcb in range(8):
                    w1cb = wcb.tile([P, DC * 512], BF16, tag="w",
                                    name=f"w1cb{fcb}")
                    nc.sync.dma_start(
                        out=w1cb,
                        in_=w1cb_d[l][:, fcb * DC * 512:(fcb + 1) * DC * 512])
                    for fc2 in range(4):
                        fc = fcb * 4 + fc2
                        for (t0, cl) in chs:
                            ps = pp.tile([P, 512], F32, tag="mm",
                                         name=f"psf{fc2}")
                            for dc in range(DC):
                                nc.tensor.matmul(
                                    ps[:, 0:cl],
                                    lhsT=w1cb[:, dc * 512 + fc2 * 128:
                                              dc * 512 + fc2 * 128 + 128],
                                    rhs=xh[:, dc * ptl + t0:dc * ptl + t0 + cl],
                                    start=(dc == 0), stop=(dc == DC - 1))
                            bidx = 5 + fc // 8
                            nc.scalar.activation(
                                usl(fc, t0, cl), ps[:, 0:cl], AF.Gelu,
                                bias=bcol(l, bidx)[:, fc % 8:fc % 8 + 1])
                for do_ in range(DC):
                    w2cb = wcb.tile([P, FC * 128], BF16, tag="w",
                                    name=f"w2cb{do_}")
                    nc.sync.dma_start(
                        out=w2cb,
                        in_=w2cb_d[l][:, do_ * FC * 128:(do_ + 1) * FC * 128])
                    for (t0, cl) in chs:
                        ps = pp.tile([P, 512], F32, tag="mm", name=f"psh{do_}")
                        for fc in range(FC):
                            nc.tensor.matmul(
                                ps[:, 0:cl],
                                lhsT=w2cb[:, fc * 128:fc * 128 + 128],
                                rhs=usl(fc, t0, cl),
                                start=(fc == 0), stop=(fc == FC - 1))
                        hsl = hT[:, do_ * ptl + t0:do_ * ptl + t0 + cl]
                        nc.vector.scalar_tensor_tensor(
                            hsl, ps[:, 0:cl], bcol(l, 4)[:, do_:do_ + 1], hsl,
                            op0=OP.add, op1=OP.add)
                    if l == L - 1:
                        for (t0o, clo) in chs:
                            nc.sync.dma_start(
                                out=houtT[:, do_ * ptl + t0o:do_ * ptl + t0o + clo],
                                in_=hT[:, do_ * ptl + t0o:do_ * ptl + t0o + clo])

    nc.compile()
    return nc


_NC_CACHE = {}


def _get_nc(lt=1032, nt=9, use_lng=False, w64=True):
    key = (lt, nt, use_lng, w64)
    if key not in _NC_CACHE:
        _NC_CACHE[key] = _build(lt, nt, use_lng, w64)
    return _NC_CACHE[key]


def _pack_shared(inputs, lt, nt, use_lng):
    bf = np.dtype("bfloat16") if hasattr(np, "bfloat16") else None
    import ml_dtypes
    BFD = ml_dtypes.bfloat16

    def b16(x):
        return np.ascontiguousarray(np.asarray(x, np.float32).astype(BFD))

    shared = {}
    for l in range(L):
        Wq = np.asarray(inputs["Wq"][l], np.float32)
        Wk = np.asarray(inputs["Wk"][l], np.float32)
        Wv = np.asarray(inputs["Wv"][l], np.float32)
        Wo = np.asarray(inputs["Wo"][l], np.float32)
        W1 = np.asarray(inputs["W1"][l], np.float32)
        W2 = np.asarray(inputs["W2"][l], np.float32)

        def colblocks(W, ocn):  # [D, D] -> [P, ocn*DC*128]
            # block (oc): [p, dc, c] = W[dc*128+p, oc*128+c]
            Wr = W.reshape(DC, P, ocn, 128)  # [dc, p, oc, c]
            return np.ascontiguousarray(
                Wr.transpose(1, 2, 0, 3).reshape(P, ocn * DC * 128))

        shared[f"kcb{l}"] = b16(colblocks(Wk, DC))
        shared[f"qcb{l}"] = b16(colblocks(Wq, DC))
        shared[f"ocb{l}"] = b16(colblocks(Wo, DC))
        # vrb: [p, nh, dc, c] = Wv[dc*128+p, nh*512+c]
        Wvr = Wv.reshape(DC, P, 2, 512)
        shared[f"vrb{l}"] = b16(
            Wvr.transpose(1, 2, 0, 3).reshape(P, 2 * DC * 512))
        # w1cb: [p, fcb, dc, c] = W1[dc*128+p, fcb*512+c]
        W1r = W1.reshape(DC, P, 8, 512)
        shared[f"w1cb{l}"] = b16(
            W1r.transpose(1, 2, 0, 3).reshape(P, 8 * DC * 512))
        # w2cb: [p, do, fc, c] = W2[fc*128+p, do*128+c]
        W2r = W2.reshape(FC, P, DC, 128)
        shared[f"w2cb{l}"] = b16(
            W2r.transpose(1, 2, 0, 3).reshape(P, DC * FC * 128))

    cbw = np.zeros((P, 2 + 96 * L), np.float32)
    cbw[:, 0] = 1.0
    cbw[0, 1] = EPS
    for l in range(L):
        c0 = 2 + 96 * l
        # bv is folded into bo: probs sum to 1, so ctx@Wo + bo with V+bv
        # equals (ctx from plain V)@Wo + (bo + bv@Wo).
        bo_eff = (np.asarray(inputs["bo"][l], np.float32)
                  + np.asarray(inputs["bv"][l], np.float32)
                  @ np.asarray(inputs["Wo"][l], np.float32))
        vals = {"bq": np.asarray(inputs["bq"][l], np.float32),
                "bk": np.asarray(inputs["bk"][l], np.float32),
                "bv": np.zeros(D, np.float32),
                "bo": bo_eff,
                "b2": np.asarray(inputs["b2"][l], np.float32)}
        for i, key in enumerate(("bq", "bk", "bv", "bo", "b2")):
            cbw[:, c0 + 8 * i:c0 + 8 * i + 8] = vals[key].reshape(DC, P).T
        b1v = np.asarray(inputs["b1"][l], np.float32)
        cbw[:, c0 + 40:c0 + 72] = b1v.reshape(FC, P).T
    shared["cb"] = np.ascontiguousarray(cbw)

    if use_lng:
        gb = np.zeros((P, 8 * (2 + 4 * L)), np.float32)
        # group 0: ln0 (handled as gi=None in build... keep identity)
        idx = 0
        for l in range(L):
            for which in range(2):
                gi = 2 * l + which
                g = np.asarray(inputs["ln1_g" if which == 0 else "ln2_g"][l],
                               np.float32)
                bb = np.asarray(inputs["ln1_b" if which == 0 else "ln2_b"][l],
                                np.float32)
                gb[:, 8 * (2 * gi):8 * (2 * gi) + 8] = g.reshape(DC, P).T
                gb[:, 8 * (2 * gi + 1):8 * (2 * gi + 1) + 8] = bb.reshape(DC, P).T
        shared["lngb"] = np.ascontiguousarray(gb)
    return shared


def _prep_core(inputs, b, start, n, lt, nt, w64):
    import ml_dtypes
    BFD = ml_dtypes.bfloat16
    ptl = nt * P

    def b16(x):
        return np.ascontiguousarray(np.asarray(x, np.float32).astype(BFD))

    ids = np.asarray(inputs["input_ids"][b, start:start + n])
    pid = np.asarray(inputs["patch_ids"][b, start:start + n]).astype(np.int64)
    pos_emb = np.asarray(inputs["pos_emb"], np.float32)
    hashes = np.asarray(inputs["hash_embeddings"], np.float32)
    tok = np.asarray(inputs["tok_emb"], np.float32)

    base = np.zeros((ptl, D), np.float32)
    emb = (tok[ids] + pos_emb[start:start + n]
           + hashes[b, start:start + n]).astype(np.float32)
    mu = emb.mean(-1, keepdims=True)
    var = ((emb - mu) ** 2).mean(-1, keepdims=True)
    g0 = np.asarray(inputs["ln0_g"], np.float32)
    b0 = np.asarray(inputs["ln0_b"], np.float32)
    base[:n] = (emb - mu) / np.sqrt(var + EPS) * g0 + b0
    baseT = np.ascontiguousarray(
        base.reshape(ptl, DC, P).transpose(2, 1, 0).reshape(P, DC * ptl))

    pidp = np.empty(ptl, np.int64)
    pidp[:n] = pid
    pidp[n:] = -np.arange(1, ptl - n + 1)

    ew = 256 if w64 else 384
    m = np.zeros((nt, P, ew), np.float32)
    for j in range(nt):
        if w64:
            w0 = int(np.clip(j * P - 64, 0, ptl - ew))
        else:
            w0 = int(np.clip(j - 1, 0, nt - 3)) * P
        kk = pidp[j * P:(j + 1) * P]
        qq = pidp[w0:w0 + ew]
        m[j] = (kk[:, None] == qq[None, :]).astype(np.float32)
    masks = b16(m.transpose(1, 0, 2).reshape(P, nt * ew))
    return {"baseT": baseT, "masks": masks}


def kernel(**inputs):
    pid_all = np.asarray(inputs["patch_ids"])

    shards = []
    for b in range(B):
        pid = np.asarray(pid_all[b])
        bnd = np.nonzero(pid[1:] != pid[:-1])[0] + 1
        cand = bnd[(bnd >= S - 1152) & (bnd <= 1152)]
        if len(cand) == 0:
            raise RuntimeError("no patch boundary near S/2; cannot shard")
        s = int(cand[np.argmin(np.abs(cand - S // 2))])
        shards.append((b, 0, s))
        shards.append((b, s, S - s))

    lt = max(n for _, _, n in shards)
    lt = max(lt, 1026)  # floor so chunk 3 isn't degenerate-tiny
    nt = (lt + P - 1) // P

    maxrun = 0
    for b in range(B):
        p = np.asarray(pid_all[b])
        bnd = np.nonzero(p[1:] != p[:-1])[0] + 1
        edges = np.concatenate([[0], bnd, [len(p)]])
        maxrun = max(maxrun, int(np.diff(edges).max()))
    w64 = maxrun <= 64

    use_lng = not (
        all(np.all(np.asarray(inputs[k]) == 1.0)
            for k in ("ln1_g", "ln2_g")) and
        all(np.all(np.asarray(inputs[k]) == 0.0)
            for k in ("ln1_b", "ln2_b")))
    if use_lng:
        raise NotImplementedError(
            "non-identity LN affine not supported in fast path")

    shared = _pack_shared(inputs, lt, nt, use_lng)
    in_maps = []
    for b, start, n in shards:
        mcore = dict(shared)
        mcore.update(_prep_core(inputs, b, start, n, lt, nt, w64))
        in_maps.append(mcore)

    nc = _get_nc(lt, nt, use_lng, w64)
    res = bass_utils.run_bass_kernel_spmd(nc, in_maps,
                                          core_ids=list(range(NCORES)))

    ptl = nt * P
    out = np.zeros((B, S, D), np.float32)
    for i, (b, start, n) in enumerate(shards):
        ht = res.results[i]["houtT"]
        hfull = ht.reshape(P, DC, ptl).transpose(2, 1, 0).reshape(ptl, D)
        out[b, start:start + n] = hfull[:n]
    return out


if __name__ == "__main__":
    import sys
    lt = int(sys.argv[1]) if len(sys.argv) > 1 else 1032
    _get_nc(lt, (lt + P - 1) // P, False)
    print("built ok")


# revision 25
# speedup vs baseline: 1.4179x; 1.0218x over previous
"""BLT local encoder (2-layer transformer, patch-equality block-diagonal attention)
on 8 Trainium2 NeuronCores.

v2. Sharding: each of the 4 sequences splits at a patch-run boundary nearest
S/2 -> 8 independent shards, one per core, zero cross-core communication.

Kernel design (per core, L_tok = max shard length ~1032):
- Residual hT kept float32 feature-major [P, 8dc x PTL]; everything else bf16.
- Weights prepacked host-side into SBUF-ready bf16 col/row blocks, streamed
  once per layer (no restreaming), double-buffered.
- One LayerNorm per sublayer, output xh bf16 reused by Q, K and V.
- Full-shard attention: per (head, key-tile j) one score matmul with moving
  dim >= 256; softmax denominator via a ones-column appended to V (row 64 of
  the ctx psum); per-head normalize fused into the psum->SBUF copy.
- Engine split: PE matmuls; DVE normalize/copies/masks; Act square/exp/gelu;
  Pool partition-broadcasts + residual adds.
"""

import numpy as np

import concourse.bass as bass
import concourse.tile as tile
from concourse import bacc, bass_utils, mybir

F32 = mybir.dt.float32
F32R = mybir.dt.float32r
BF16 = mybir.dt.bfloat16
AF = mybir.ActivationFunctionType
OP = mybir.AluOpType

B, S, D, H, F, L = 4, 2048, 1024, 16, 4096, 2
DH = D // H      # 64
DC = D // 128    # 8
FC = F // 128    # 32
EPS = 1e-5
SCALE = 1.0 / np.sqrt(DH)
P = 128
VP = 384         # vocab 260 padded
VC = VP // 128   # 3
NCORES = 8


def _chunks(lt):
    out = []
    o = 0
    while o < lt:
        c = min(512, lt - o)
        out.append((o, c))
        o += c
    return out


def _build(lt, nt, use_lng, wov):
    """lt: tokens; nt: tiles; use_lng: ln affine ops; wov: +-wov-token window."""
    ptl = nt * P
    EW = (128 + 2 * wov) if wov else 384
    chs = _chunks(lt)
    nc = bacc.Bacc("TRN2", target_bir_lowering=False, debug=False,
                   num_devices=NCORES)

    def din(name, shape, dt=BF16):
        return nc.dram_tensor(name, shape, dt, kind="ExternalInput").ap()

    baseT = din("baseT", [P, DC * ptl], F32R)
    masks_d = din("masks", [P, nt * EW])
    # prepacked weights
    kcb_d, qcb_d, ocb_d, vrb_d, w1cb_d, w2cb_d = [], [], [], [], [], []
    for l in range(L):
        kcb_d.append(din(f"kcb{l}", [P, DC * DC * 128]))
        qcb_d.append(din(f"qcb{l}", [P, DC * DC * 128]))
        ocb_d.append(din(f"ocb{l}", [P, DC * DC * 128]))
        vrb_d.append(din(f"vrb{l}", [P, DC * D]))
        w1cb_d.append(din(f"w1cb{l}", [P, 8 * DC * 512]))
        w2cb_d.append(din(f"w2cb{l}", [P, DC * FC * 128]))
    # packed per-feature consts: [P, col] layout, 8 cols per D-vector
    # cols: 0 ones | 1 eps(row0) | then per layer l at 2+64*l:
    #   bq 0:8 bk 8:16 bv 16:24 bo 24:32 b2 32:40 b1 40:72 (unused gap)
    # ln g/b (if use_lng): separate tensor lngb
    cb_d = din("cb", [P, 2 + 96 * L], F32)
    lngb_d = din("lngb", [P, 8 * (2 + 4 * L)], F32) if use_lng else None
    houtT = nc.dram_tensor("houtT", [P, DC * ptl], F32R,
                           kind="ExternalOutput").ap()

    with tile.TileContext(nc) as tc:
        with (
            nc.allow_low_precision(
                reason="bf16 softmax/LN staging validated vs reference"),
            tc.tile_pool(name="pers", bufs=1) as pers,
            tc.tile_pool(name="big", bufs=4) as big,
            tc.tile_pool(name="xhp", bufs=1) as xhp,
            tc.tile_pool(name="wcb", bufs=4) as wcb,
            tc.tile_pool(name="est", bufs=2) as estp,
            tc.tile_pool(name="lnt", bufs=3) as lnp,
            tc.tile_pool(name="sm", bufs=2) as smp,
            tc.tile_pool(name="dv", bufs=2) as dvp,
            tc.tile_pool(name="pp", bufs=8, space="PSUM") as pp,
        ):
            cb = pers.tile([P, 2 + 96 * L], F32, tag="cb")
            nc.sync.dma_start(out=cb, in_=cb_d)
            eps_t = cb[0:1, 1:2]
            ones_r = pers.tile([P, 1], F32R, tag="ones_r")
            nc.vector.tensor_copy(ones_r, cb[:, 0:1])
            ones_b = pers.tile([P, 1], BF16, tag="ones_b")
            nc.vector.tensor_copy(ones_b, cb[:, 0:1])
            if use_lng:
                lngb = pers.tile([P, 8 * (2 + 4 * L)], F32, tag="lngb")
                nc.sync.dma_start(out=lngb, in_=lngb_d)

            masks = pers.tile([P, nt * EW], BF16, tag="masks")
            nc.sync.dma_start(out=masks, in_=masks_d)

            hT = pers.tile([P, DC * ptl], F32R, tag="hT")

            def bcol(l, i):  # bias col i (in 8-col groups) for layer l
                c0 = 2 + 96 * l + 8 * i
                return cb[:, c0:c0 + 8]

            def ln_stats(rms, ci, t0, cl):
                ps1 = pp.tile([1, 512], F32, tag="mm", name="lns1")
                ps2 = pp.tile([1, 512], F32, tag="mm", name="lns2")
                for dc in range(DC):
                    hsl = hT[:, dc * ptl + t0:dc * ptl + t0 + cl]
                    sq = lnp.tile([P, 512], BF16, tag="sq", name=f"sq{dc}")
                    if dc < 4:
                        nc.scalar.activation(sq[:, 0:cl], hsl, AF.Square)
                    elif dc < 7:
                        nc.vector.tensor_mul(sq[:, 0:cl], hsl, hsl)
                    else:
                        nc.gpsimd.tensor_mul(sq[:, 0:cl], hsl, hsl)
                    nc.tensor.matmul(ps1[:, 0:cl], lhsT=ones_r, rhs=hsl,
                                     start=(dc == 0), stop=(dc == DC - 1))
                    nc.tensor.matmul(ps2[:, 0:cl], lhsT=ones_b,
                                     rhs=sq[:, 0:cl],
                                     start=(dc == 0), stop=(dc == DC - 1))
                st = smp.tile([P, 2 * 512], F32, tag="st", name="st")
                stb = smp.tile([P, 2 * 512], BF16, tag="stb", name="stb")
                mean = st[0:1, 0:cl]
                var = st[0:1, 512:512 + cl]
                rstd = stb[0:1, 0:cl]
                mr = stb[0:1, 512:512 + cl]
                nc.vector.tensor_scalar_mul(mean, ps1[:, 0:cl], 1.0 / D)
                nc.vector.tensor_mul(var, mean, mean)
                nc.vector.scalar_tensor_tensor(
                    var, ps2[:, 0:cl], 1.0 / D, var,
                    op0=OP.mult, op1=OP.subtract)
                nc.scalar.activation(var, var, AF.Sqrt, bias=eps_t)
                nc.vector.reciprocal(rstd, var)
                nc.vector.tensor_mul(mr, mean, rstd)
                RM = dvp.tile([P, 2 * 512], BF16, tag="rm", name="RM", bufs=3)
                nc.gpsimd.partition_broadcast(RM[:, 0:cl], rstd)
                nc.gpsimd.partition_broadcast(RM[:, 512:512 + cl], mr)
                rms[ci] = RM

            def ln_norm(rms, gi, out_tile, ci, t0, cl):
                RM = rms[ci]
                for dc in range(DC):
                    hsl = hT[:, dc * ptl + t0:dc * ptl + t0 + cl]
                    d1 = lnp.tile([P, 512], BF16, tag="d1", name=f"d1_{dc}")
                    eng = nc.gpsimd if dc >= 6 else nc.vector
                    eng.tensor_mul(d1[:, 0:cl], hsl, RM[:, 0:cl])
                    osl = out_tile[:, dc * ptl + t0:dc * ptl + t0 + cl]
                    if use_lng and gi is not None:
                        d2 = lnp.tile([P, 512], BF16, tag="d2",
                                      name=f"d2_{dc}")
                        nc.vector.tensor_sub(d2[:, 0:cl], d1[:, 0:cl],
                                             RM[:, 512:512 + cl])
                        g0 = 8 * (2 * gi)
                        nc.vector.tensor_scalar(
                            osl, d2[:, 0:cl],
                            lngb[:, g0 + dc:g0 + dc + 1],
                            lngb[:, g0 + 8 + dc:g0 + 8 + dc + 1],
                            op0=OP.mult, op1=OP.add)
                    else:
                        nc.vector.tensor_sub(osl, d1[:, 0:cl],
                                             RM[:, 512:512 + cl])

            # ---------- initial residual (host LN0(emb)) ----------
            for dc in range(DC):
                nc.sync.dma_start(out=hT[:, dc * ptl:(dc + 1) * ptl],
                                  in_=baseT[:, dc * ptl:(dc + 1) * ptl])

            # ---------- layers ----------
            for l in range(L):
                xh = xhp.tile([P, DC * ptl], BF16, tag="xh", name=f"xh{l}a")

                # ---- K/Q/V + attention, interleaved ----
                KT = big.tile([P, DC * ptl], BF16, tag="b18", name=f"KT{l}")
                Vsb = big.tile([P, nt * H * 65], BF16, tag="b18", name=f"Vsb{l}")
                QT = big.tile([P, DC * ptl], BF16, tag="b18", name=f"QT{l}")
                ctxc = big.tile([P, DC * ptl], BF16, tag="b18", name=f"ctx{l}")
                if lt < ptl:
                    nc.vector.memset(
                        Vsb[:, (nt - 1) * H * 65:nt * H * 65], 0.0)
                ones_v = Vsb.rearrange("p (g x) -> p g x", x=65)[:, :, 64:65]
                nc.vector.memset(ones_v, 1.0)

                def v_tg(nh, tg, norm=None):
                    if norm is not None:
                        rms_, gi_, t0_, cl_ = norm
                        RM = rms_
                    if True:
                        tts = [t for t in range(4 * tg, min(4 * tg + 4, nt))
                               if lt - t * P > 0]
                        pvs = {}
                        for tt in tts:
                            pvs[tt] = pp.tile([P, 512], F32, tag="mm",
                                              name=f"psv{tt}_{nh}")
                        for dc in range(DC):
                            if norm is not None:
                                hsl = hT[:, dc * ptl + t0_:dc * ptl + t0_ + cl_]
                                d1 = lnp.tile([P, 512], BF16, tag="d1",
                                              name=f"d1v{dc}")
                                eng = nc.gpsimd if dc >= 6 else nc.vector
                                eng.tensor_mul(d1[:, 0:cl_], hsl, RM[:, 0:cl_])
                                nc.vector.tensor_sub(
                                    xh[:, dc * ptl + t0_:dc * ptl + t0_ + cl_],
                                    d1[:, 0:cl_], RM[:, 512:512 + cl_])
                            vrb = wcb.tile([P, 512], BF16, tag="w",
                                           name=f"vrb{nh}_{tg}_{dc}")
                            nc.sync.dma_start(
                                out=vrb,
                                in_=vrb_d[l][:, (nh * DC + dc) * 512:
                                             (nh * DC + dc + 1) * 512])
                            for tt in tts:
                                tl = min(P, lt - tt * P)
                                nc.tensor.matmul(
                                    pvs[tt][0:tl, :],
                                    lhsT=xh[:, dc * ptl + tt * P:dc * ptl + tt * P + tl],
                                    rhs=vrb,
                                    start=(dc == 0), stop=(dc == DC - 1))
                        for tt in tts:
                            tl = min(P, lt - tt * P)
                            pv = pvs[tt][0:tl, :].rearrange(
                                "p (h x) -> p h x", h=8)
                            ov = Vsb[0:tl, (tt * H + nh * 8) * 65:
                                     (tt * H + nh * 8 + 8) * 65].rearrange(
                                "p (h x) -> p h x", x=65)[:, :, 0:64]
                            nc.scalar.copy(ov, pv)

                def kq_block(oc):
                    kcb = wcb.tile([P, DC * 128], BF16, tag="w",
                                   name=f"kcb{oc}")
                    nc.sync.dma_start(
                        out=kcb, in_=kcb_d[l][:, oc * D:(oc + 1) * D])
                    for (t0, cl) in chs:
                        ps = pp.tile([P, 512], F32, tag="mm", name=f"psk{oc}")
                        for dc in range(DC):
                            nc.tensor.matmul(
                                ps[:, 0:cl],
                                lhsT=kcb[:, dc * 128:dc * 128 + 128],
                                rhs=xh[:, dc * ptl + t0:dc * ptl + t0 + cl],
                                start=(dc == 0), stop=(dc == DC - 1))
                        nc.vector.tensor_scalar_add(
                            KT[:, oc * ptl + t0:oc * ptl + t0 + cl],
                            ps[:, 0:cl], bcol(l, 1)[:, oc:oc + 1])
                    qcb = wcb.tile([P, DC * 128], BF16, tag="w",
                                   name=f"qcb{oc}")
                    nc.sync.dma_start(
                        out=qcb, in_=qcb_d[l][:, oc * D:(oc + 1) * D])
                    for (t0, cl) in chs:
                        ps = pp.tile([P, 512], F32, tag="mm", name=f"psq{oc}")
                        for dc in range(DC):
                            nc.tensor.matmul(
                                ps[:, 0:cl],
                                lhsT=qcb[:, dc * 128:dc * 128 + 128],
                                rhs=xh[:, dc * ptl + t0:dc * ptl + t0 + cl],
                                start=(dc == 0), stop=(dc == DC - 1))
                        nc.scalar.activation(
                            QT[:, oc * ptl + t0:oc * ptl + t0 + cl],
                            ps[:, 0:cl], AF.Identity,
                            bias=bcol(l, 0)[:, oc:oc + 1])
                    if lt < ptl:
                        nc.vector.memset(KT[:, oc * ptl + lt:(oc + 1) * ptl],
                                         0.0)
                        nc.vector.memset(QT[:, oc * ptl + lt:(oc + 1) * ptl],
                                         0.0)

                def head_scores(h):
                    dch, po = h // 2, (h % 2) * 64
                    est = estp.tile([P, nt * EW], BF16, tag="est",
                                    name=f"est{h}")
                    ests[h] = est
                    for j in range(nt):
                        if wov:
                            w0 = min(max(j * P - wov, 0), ptl - EW)
                            nq = EW
                            lo = w0
                        else:
                            loj = max(j - 1, 0)
                            hi = min(j + 1, nt - 1)
                            nq = (hi - loj + 1) * P
                            w0 = min(max(j - 1, 0), nt - 3) * P
                            lo = loj * P
                        pst = pp.tile([P, 384], F32, tag="mm", name=f"pst{j}")
                        nc.tensor.matmul(
                            pst[:, 0:nq],
                            lhsT=KT[po:po + 64, dch * ptl + j * P:dch * ptl + j * P + P],
                            rhs=QT[po:po + 64, dch * ptl + lo:dch * ptl + lo + nq],
                            start=True, stop=True)
                        esl = est[:, j * EW + (lo - w0):j * EW + (lo - w0) + nq]
                        nc.scalar.activation(esl, pst[:, 0:nq], AF.Exp,
                                             scale=float(SCALE))
                    nc.vector.tensor_mul(est, est, masks)

                def head_ctx(h):
                    dch, po = h // 2, (h % 2) * 64
                    est = ests[h]
                    for qg in range((nt + 3) // 4):
                        qts = [q for q in range(4 * qg, min(4 * qg + 4, nt))]
                        gw = len(qts) * P
                        psc = pp.tile([65, 512], F32, tag="mm", name=f"psc{qg}")
                        for qi, qt in enumerate(qts):
                            if wov:
                                regions = [(0, wov, [qt, qt - 1]),
                                           (wov, P - wov, [qt]),
                                           (P - wov, P, [qt, qt + 1])]
                                for (a, b, js0) in regions:
                                    if b <= a:
                                        continue
                                    js = [j for j in js0 if 0 <= j < nt]
                                    oc_ = psc[:, qi * P + a:qi * P + b]
                                    for kk, j in enumerate(js):
                                        w0 = min(max(j * P - wov, 0),
                                                 ptl - EW)
                                        qa = qt * P + a - w0
                                        rsl = est[:, j * EW + qa:
                                                  j * EW + qa + (b - a)]
                                        nc.tensor.matmul(
                                            oc_,
                                            lhsT=Vsb[:, (j * H + h) * 65:
                                                     (j * H + h) * 65 + 65],
                                            rhs=rsl,
                                            start=(kk == 0),
                                            stop=(kk == len(js) - 1))
                            else:
                                js = [j for j in (qt - 1, qt, qt + 1)
                                      if 0 <= j < nt]
                                for kk, j in enumerate(js):
                                    w0 = min(max(j - 1, 0), nt - 3) * P
                                    rsl = est[:, j * EW + qt * P - w0:
                                              j * EW + qt * P - w0 + P]
                                    nc.tensor.matmul(
                                        psc[:, qi * P:(qi + 1) * P],
                                        lhsT=Vsb[:, (j * H + h) * 65:
                                                 (j * H + h) * 65 + 65],
                                        rhs=rsl,
                                        start=(kk == 0), stop=(kk == len(js) - 1))
                        dinv = dvp.tile([1, 512], BF16, tag="dinv",
                                        name=f"dinv{qg}")
                        nc.vector.reciprocal(dinv[:, 0:gw], psc[64:65, 0:gw])
                        dnb = dvp.tile([P, 512], BF16, tag="dnb",
                                       name=f"dnb{qg}")
                        nc.gpsimd.partition_broadcast(dnb[0:64, 0:gw],
                                                      dinv[:, 0:gw])
                        nc.vector.tensor_mul(
                            ctxc[po:po + 64,
                                 dch * ptl + qg * 512:dch * ptl + qg * 512 + gw],
                            psc[0:64, 0:gw], dnb[0:64, 0:gw])

                ests = {}
                gi1 = 2 * l if use_lng else None
                rms1 = {}
                for ci, (t0, cl) in enumerate(chs):
                    ln_stats(rms1, ci, t0, cl)
                for ci, (t0, cl) in enumerate(chs):
                    if use_lng:
                        ln_norm(rms1, gi1, xh, ci, t0, cl)
                        v_tg(0, ci)
                    else:
                        v_tg(0, ci, norm=(rms1[ci], gi1, t0, cl))
                kq_block(0)
                for oc in range(1, DC):
                    if oc == 5:
                        for ci in range(len(chs)):
                            v_tg(1, ci)
                    head_scores(2 * oc - 2)
                    head_scores(2 * oc - 1)
                    kq_block(oc)
                    head_ctx(2 * oc - 2)
                    head_ctx(2 * oc - 1)
                head_scores(14)
                head_scores(15)
                head_ctx(14)
                head_ctx(15)

                # ---- O-projection (chunk-outer) + residual + LN2 ----
                xh = xhp.tile([P, DC * ptl], BF16, tag="xh", name=f"xh{l}b")
                gi2 = 2 * l + 1 if use_lng else None
                rms2 = {}
                for ci, (t0, cl) in enumerate(chs):
                    for do_ in range(DC):
                        ocb = wcb.tile([P, DC * 128], BF16, tag="w",
                                       name=f"ocb{ci}_{do_}")
                        nc.sync.dma_start(
                            out=ocb, in_=ocb_d[l][:, do_ * D:(do_ + 1) * D])
                        ps = pp.tile([P, 512], F32, tag="mm", name=f"pso{do_}")
                        for dc in range(DC):
                            nc.tensor.matmul(
                                ps[:, 0:cl], lhsT=ocb[:, dc * 128:dc * 128 + 128],
                                rhs=ctxc[:, dc * ptl + t0:dc * ptl + t0 + cl],
                                start=(dc == 0), stop=(dc == DC - 1))
                        hsl = hT[:, do_ * ptl + t0:do_ * ptl + t0 + cl]
                        nc.vector.scalar_tensor_tensor(
                            hsl, ps[:, 0:cl], bcol(l, 3)[:, do_:do_ + 1], hsl,
                            op0=OP.add, op1=OP.add)
                    ln_stats(rms2, ci, t0, cl)
                for ci, (t0, cl) in enumerate(chs):
                    ln_norm(rms2, gi2, xh, ci, t0, cl)

                # ---- FFN ----
                Us = [big.tile([P, 8 * ptl], BF16, tag="b18", name=f"U{l}_{i}")
                      for i in range(4)]

                def usl(fc, t0, cl):
                    t = Us[fc // 8]
                    k = fc % 8
                    return t[:, k * ptl + t0:k * ptl + t0 + cl]

                for fcb in range(8):
                    w1cb = wcb.tile([P, DC * 512], BF16, tag="w",
                                    name=f"w1cb{fcb}")
                    nc.sync.dma_start(
                        out=w1cb,
                        in_=w1cb_d[l][:, fcb * DC * 512:(fcb + 1) * DC * 512])
                    for fc2 in range(4):
                        fc = fcb * 4 + fc2
                        for (t0, cl) in chs:
                            ps = pp.tile([P, 512], F32, tag="mm",
                                         name=f"psf{fc2}")
                            for dc in range(DC):
                                nc.tensor.matmul(
                                    ps[:, 0:cl],
                                    lhsT=w1cb[:, dc * 512 + fc2 * 128:
                                              dc * 512 + fc2 * 128 + 128],
                                    rhs=xh[:, dc * ptl + t0:dc * ptl + t0 + cl],
                                    start=(dc == 0), stop=(dc == DC - 1))
                            bidx = 5 + fc // 8
                            nc.scalar.activation(
                                usl(fc, t0, cl), ps[:, 0:cl], AF.Gelu,
                                bias=bcol(l, bidx)[:, fc % 8:fc % 8 + 1])
                for do_ in range(DC):
                    w2cb = wcb.tile([P, FC * 128], BF16, tag="w",
                                    name=f"w2cb{do_}")
                    nc.sync.dma_start(
                        out=w2cb,
                        in_=w2cb_d[l][:, do_ * FC * 128:(do_ + 1) * FC * 128])
                    for (t0, cl) in chs:
                        ps = pp.tile([P, 512], F32, tag="mm", name=f"psh{do_}")
                        for fc in range(FC):
                            nc.tensor.matmul(
                                ps[:, 0:cl],
                                lhsT=w2cb[:, fc * 128:fc * 128 + 128],
                                rhs=usl(fc, t0, cl),
                                start=(fc == 0), stop=(fc == FC - 1))
                        hsl = hT[:, do_ * ptl + t0:do_ * ptl + t0 + cl]
                        nc.vector.scalar_tensor_tensor(
                            hsl, ps[:, 0:cl], bcol(l, 4)[:, do_:do_ + 1], hsl,
                            op0=OP.add, op1=OP.add)
                    if l == L - 1:
                        for (t0o, clo) in chs:
                            nc.sync.dma_start(
                                out=houtT[:, do_ * ptl + t0o:do_ * ptl + t0o + clo],
                                in_=hT[:, do_ * ptl + t0o:do_ * ptl + t0o + clo])

    nc.compile()
    return nc


_NC_CACHE = {}


def _get_nc(lt=1032, nt=9, use_lng=False, wov=16):
    key = (lt, nt, use_lng, wov)
    if key not in _NC_CACHE:
        _NC_CACHE[key] = _build(lt, nt, use_lng, wov)
    return _NC_CACHE[key]


def _pack_shared(inputs, lt, nt, use_lng):
    bf = np.dtype("bfloat16") if hasattr(np, "bfloat16") else None
    import ml_dtypes
    BFD = ml_dtypes.bfloat16

    def b16(x):
        return np.ascontiguousarray(np.asarray(x, np.float32).astype(BFD))

    shared = {}
    for l in range(L):
        Wq = np.asarray(inputs["Wq"][l], np.float32)
        Wk = np.asarray(inputs["Wk"][l], np.float32)
        Wv = np.asarray(inputs["Wv"][l], np.float32)
        Wo = np.asarray(inputs["Wo"][l], np.float32)
        W1 = np.asarray(inputs["W1"][l], np.float32)
        W2 = np.asarray(inputs["W2"][l], np.float32)

        def colblocks(W, ocn):  # [D, D] -> [P, ocn*DC*128]
            # block (oc): [p, dc, c] = W[dc*128+p, oc*128+c]
            Wr = W.reshape(DC, P, ocn, 128)  # [dc, p, oc, c]
            return np.ascontiguousarray(
                Wr.transpose(1, 2, 0, 3).reshape(P, ocn * DC * 128))

        shared[f"kcb{l}"] = b16(colblocks(Wk, DC))
        shared[f"qcb{l}"] = b16(colblocks(Wq, DC))
        shared[f"ocb{l}"] = b16(colblocks(Wo, DC))
        # vrb: [p, nh, dc, c] = Wv[dc*128+p, nh*512+c]
        Wvr = Wv.reshape(DC, P, 2, 512)
        shared[f"vrb{l}"] = b16(
            Wvr.transpose(1, 2, 0, 3).reshape(P, 2 * DC * 512))
        # w1cb: [p, fcb, dc, c] = W1[dc*128+p, fcb*512+c]
        W1r = W1.reshape(DC, P, 8, 512)
        shared[f"w1cb{l}"] = b16(
            W1r.transpose(1, 2, 0, 3).reshape(P, 8 * DC * 512))
        # w2cb: [p, do, fc, c] = W2[fc*128+p, do*128+c]
        W2r = W2.reshape(FC, P, DC, 128)
        shared[f"w2cb{l}"] = b16(
            W2r.transpose(1, 2, 0, 3).reshape(P, DC * FC * 128))

    cbw = np.zeros((P, 2 + 96 * L), np.float32)
    cbw[:, 0] = 1.0
    cbw[0, 1] = EPS
    for l in range(L):
        c0 = 2 + 96 * l
        # bv is folded into bo: probs sum to 1, so ctx@Wo + bo with V+bv
        # equals (ctx from plain V)@Wo + (bo + bv@Wo).
        bo_eff = (np.asarray(inputs["bo"][l], np.float32)
                  + np.asarray(inputs["bv"][l], np.float32)
                  @ np.asarray(inputs["Wo"][l], np.float32))
        vals = {"bq": np.asarray(inputs["bq"][l], np.float32),
                "bk": np.asarray(inputs["bk"][l], np.float32),
                "bv": np.zeros(D, np.float32),
                "bo": bo_eff,
                "b2": np.asarray(inputs["b2"][l], np.float32)}
        for i, key in enumerate(("bq", "bk", "bv", "bo", "b2")):
            cbw[:, c0 + 8 * i:c0 + 8 * i + 8] = vals[key].reshape(DC, P).T
        b1v = np.asarray(inputs["b1"][l], np.float32)
        cbw[:, c0 + 40:c0 + 72] = b1v.reshape(FC, P).T
    shared["cb"] = np.ascontiguousarray(cbw)

    if use_lng:
        gb = np.zeros((P, 8 * (2 + 4 * L)), np.float32)
        # group 0: ln0 (handled as gi=None in build... keep identity)
        idx = 0
        for l in range(L):
            for which in range(2):
                gi = 2 * l + which
                g = np.asarray(inputs["ln1_g" if which == 0 else "ln2_g"][l],
                               np.float32)
                bb = np.asarray(inputs["ln1_b" if which == 0 else "ln2_b"][l],
                                np.float32)
                gb[:, 8 * (2 * gi):8 * (2 * gi) + 8] = g.reshape(DC, P).T
                gb[:, 8 * (2 * gi + 1):8 * (2 * gi + 1) + 8] = bb.reshape(DC, P).T
        shared["lngb"] = np.ascontiguousarray(gb)
    return shared


def _prep_core(inputs, b, start, n, lt, nt, wov):
    import ml_dtypes
    BFD = ml_dtypes.bfloat16
    ptl = nt * P

    def b16(x):
        return np.ascontiguousarray(np.asarray(x, np.float32).astype(BFD))

    ids = np.asarray(inputs["input_ids"][b, start:start + n])
    pid = np.asarray(inputs["patch_ids"][b, start:start + n]).astype(np.int64)
    pos_emb = np.asarray(inputs["pos_emb"], np.float32)
    hashes = np.asarray(inputs["hash_embeddings"], np.float32)
    tok = np.asarray(inputs["tok_emb"], np.float32)

    base = np.zeros((ptl, D), np.float32)
    emb = (tok[ids] + pos_emb[start:start + n]
           + hashes[b, start:start + n]).astype(np.float32)
    mu = emb.mean(-1, keepdims=True)
    var = ((emb - mu) ** 2).mean(-1, keepdims=True)
    g0 = np.asarray(inputs["ln0_g"], np.float32)
    b0 = np.asarray(inputs["ln0_b"], np.float32)
    base[:n] = (emb - mu) / np.sqrt(var + EPS) * g0 + b0
    baseT = np.ascontiguousarray(
        base.reshape(ptl, DC, P).transpose(2, 1, 0).reshape(P, DC * ptl))

    pidp = np.empty(ptl, np.int64)
    pidp[:n] = pid
    pidp[n:] = -np.arange(1, ptl - n + 1)

    ew = (128 + 2 * wov) if wov else 384
    m = np.zeros((nt, P, ew), np.float32)
    for j in range(nt):
        if wov:
            w0 = int(np.clip(j * P - wov, 0, ptl - ew))
        else:
            w0 = int(np.clip(j - 1, 0, nt - 3)) * P
        kk = pidp[j * P:(j + 1) * P]
        qq = pidp[w0:w0 + ew]
        m[j] = (kk[:, None] == qq[None, :]).astype(np.float32)
    masks = b16(m.transpose(1, 0, 2).reshape(P, nt * ew))
    return {"baseT": baseT, "masks": masks}


def kernel(**inputs):
    pid_all = np.asarray(inputs["patch_ids"])

    shards = []
    for b in range(B):
        pid = np.asarray(pid_all[b])
        bnd = np.nonzero(pid[1:] != pid[:-1])[0] + 1
        cand = bnd[(bnd >= S - 1152) & (bnd <= 1152)]
        if len(cand) == 0:
            raise RuntimeError("no patch boundary near S/2; cannot shard")
        s = int(cand[np.argmin(np.abs(cand - S // 2))])
        shards.append((b, 0, s))
        shards.append((b, s, S - s))

    lt = max(n for _, _, n in shards)
    lt = max(lt, 1026)  # floor so chunk 3 isn't degenerate-tiny
    nt = (lt + P - 1) // P

    maxrun = 0
    for b in range(B):
        p = np.asarray(pid_all[b])
        bnd = np.nonzero(p[1:] != p[:-1])[0] + 1
        edges = np.concatenate([[0], bnd, [len(p)]])
        maxrun = max(maxrun, int(np.diff(edges).max()))
    wov = next((w for w in (16, 32, 64) if maxrun <= w), None)

    use_lng = not (
        all(np.all(np.asarray(inputs[k]) == 1.0)
            for k in ("ln1_g", "ln2_g")) and
        all(np.all(np.asarray(inputs[k]) == 0.0)
            for k in ("ln1_b", "ln2_b")))
    if use_lng:
        raise NotImplementedError(
            "non-identity LN affine not supported in fast path")

    shared = _pack_shared(inputs, lt, nt, use_lng)
    in_maps = []
    for b, start, n in shards:
        mcore = dict(shared)
        mcore.update(_prep_core(inputs, b, start, n, lt, nt, wov))
        in_maps.append(mcore)

    nc = _get_nc(lt, nt, use_lng, wov)
    res = bass_utils.run_bass_kernel_spmd(nc, in_maps,
                                          core_ids=list(range(NCORES)))

    ptl = nt * P
    out = np.zeros((B, S, D), np.float32)
    for i, (b, start, n) in enumerate(shards):
        ht = res.results[i]["houtT"]
        hfull = ht.reshape(P, DC, ptl).transpose(2, 1, 0).reshape(ptl, D)
        out[b, start:start + n] = hfull[:n]
    return out


if __name__ == "__main__":
    import sys
    lt = int(sys.argv[1]) if len(sys.argv) > 1 else 1032
    _get_nc(lt, (lt + P - 1) // P, False)
    print("built ok")


# revision 26
# speedup vs baseline: 1.4264x; 1.0060x over previous
"""BLT local encoder (2-layer transformer, patch-equality block-diagonal attention)
on 8 Trainium2 NeuronCores.

v2. Sharding: each of the 4 sequences splits at a patch-run boundary nearest
S/2 -> 8 independent shards, one per core, zero cross-core communication.

Kernel design (per core, L_tok = max shard length ~1032):
- Residual hT kept float32 feature-major [P, 8dc x PTL]; everything else bf16.
- Weights prepacked host-side into SBUF-ready bf16 col/row blocks, streamed
  once per layer (no restreaming), double-buffered.
- One LayerNorm per sublayer, output xh bf16 reused by Q, K and V.
- Full-shard attention: per (head, key-tile j) one score matmul with moving
  dim >= 256; softmax denominator via a ones-column appended to V (row 64 of
  the ctx psum); per-head normalize fused into the psum->SBUF copy.
- Engine split: PE matmuls; DVE normalize/copies/masks; Act square/exp/gelu;
  Pool partition-broadcasts + residual adds.
"""

import numpy as np

import concourse.bass as bass
import concourse.tile as tile
from concourse import bacc, bass_utils, mybir

F32 = mybir.dt.float32
F32R = mybir.dt.float32r
BF16 = mybir.dt.bfloat16
AF = mybir.ActivationFunctionType
OP = mybir.AluOpType

B, S, D, H, F, L = 4, 2048, 1024, 16, 4096, 2
DH = D // H      # 64
DC = D // 128    # 8
FC = F // 128    # 32
EPS = 1e-5
SCALE = 1.0 / np.sqrt(DH)
P = 128
VP = 384         # vocab 260 padded
VC = VP // 128   # 3
NCORES = 8


def _chunks(lt):
    out = []
    o = 0
    while o < lt:
        c = min(512, lt - o)
        out.append((o, c))
        o += c
    return out


def _build(lt, nt, use_lng, wov):
    """lt: tokens; nt: tiles; use_lng: ln affine ops; wov: +-wov-token window."""
    ptl = nt * P
    EW = (128 + 2 * wov) if wov else 384
    chs = _chunks(lt)
    nc = bacc.Bacc("TRN2", target_bir_lowering=False, debug=False,
                   num_devices=NCORES)

    def din(name, shape, dt=BF16):
        return nc.dram_tensor(name, shape, dt, kind="ExternalInput").ap()

    baseT = din("baseT", [P, DC * ptl], F32R)
    masks_d = din("masks", [P, nt * EW])
    # prepacked weights
    kcb_d, qcb_d, ocb_d, vrb_d, w1cb_d, w2cb_d = [], [], [], [], [], []
    for l in range(L):
        kcb_d.append(din(f"kcb{l}", [P, DC * DC * 128]))
        qcb_d.append(din(f"qcb{l}", [P, DC * DC * 128]))
        ocb_d.append(din(f"ocb{l}", [P, DC * DC * 128]))
        vrb_d.append(din(f"vrb{l}", [P, DC * D]))
        w1cb_d.append(din(f"w1cb{l}", [P, 8 * DC * 512]))
        w2cb_d.append(din(f"w2cb{l}", [P, DC * FC * 128]))
    # packed per-feature consts: [P, col] layout, 8 cols per D-vector
    # cols: 0 ones | 1 eps(row0) | then per layer l at 2+64*l:
    #   bq 0:8 bk 8:16 bv 16:24 bo 24:32 b2 32:40 b1 40:72 (unused gap)
    # ln g/b (if use_lng): separate tensor lngb
    cb_d = din("cb", [P, 2 + 96 * L], F32)
    lngb_d = din("lngb", [P, 8 * (2 + 4 * L)], F32) if use_lng else None
    houtT = nc.dram_tensor("houtT", [P, DC * ptl], F32R,
                           kind="ExternalOutput").ap()

    with tile.TileContext(nc) as tc:
        with (
            nc.allow_low_precision(
                reason="bf16 softmax/LN staging validated vs reference"),
            tc.tile_pool(name="pers", bufs=1) as pers,
            tc.tile_pool(name="big", bufs=4) as big,
            tc.tile_pool(name="xhp", bufs=1) as xhp,
            tc.tile_pool(name="wcb", bufs=4) as wcb,
            tc.tile_pool(name="est", bufs=2) as estp,
            tc.tile_pool(name="lnt", bufs=3) as lnp,
            tc.tile_pool(name="sm", bufs=2) as smp,
            tc.tile_pool(name="dv", bufs=2) as dvp,
            tc.tile_pool(name="pp", bufs=8, space="PSUM") as pp,
        ):
            cb = pers.tile([P, 2 + 96 * L], F32, tag="cb")
            nc.sync.dma_start(out=cb, in_=cb_d)
            eps_t = cb[0:1, 1:2]
            ones_r = pers.tile([P, 1], F32R, tag="ones_r")
            nc.vector.tensor_copy(ones_r, cb[:, 0:1])
            ones_b = pers.tile([P, 1], BF16, tag="ones_b")
            nc.vector.tensor_copy(ones_b, cb[:, 0:1])
            if use_lng:
                lngb = pers.tile([P, 8 * (2 + 4 * L)], F32, tag="lngb")
                nc.sync.dma_start(out=lngb, in_=lngb_d)

            masks = pers.tile([P, nt * EW], BF16, tag="masks")
            nc.sync.dma_start(out=masks, in_=masks_d)

            hT = pers.tile([P, DC * ptl], F32R, tag="hT")

            def bcol(l, i):  # bias col i (in 8-col groups) for layer l
                c0 = 2 + 96 * l + 8 * i
                return cb[:, c0:c0 + 8]

            def ln_stats(rms, ci, t0, cl):
                ps1 = pp.tile([1, 512], F32, tag="mm", name="lns1")
                ps2 = pp.tile([1, 512], F32, tag="mm", name="lns2")
                for dc in range(DC):
                    hsl = hT[:, dc * ptl + t0:dc * ptl + t0 + cl]
                    sq = lnp.tile([P, 512], BF16, tag="sq", name=f"sq{dc}")
                    if dc < 4:
                        nc.scalar.activation(sq[:, 0:cl], hsl, AF.Square)
                    elif dc < 7:
                        nc.vector.tensor_mul(sq[:, 0:cl], hsl, hsl)
                    else:
                        nc.gpsimd.tensor_mul(sq[:, 0:cl], hsl, hsl)
                    nc.tensor.matmul(ps1[:, 0:cl], lhsT=ones_r, rhs=hsl,
                                     start=(dc == 0), stop=(dc == DC - 1))
                    nc.tensor.matmul(ps2[:, 0:cl], lhsT=ones_b,
                                     rhs=sq[:, 0:cl],
                                     start=(dc == 0), stop=(dc == DC - 1))
                st = smp.tile([P, 2 * 512], F32, tag="st", name="st")
                stb = smp.tile([P, 2 * 512], BF16, tag="stb", name="stb")
                mean = st[0:1, 0:cl]
                var = st[0:1, 512:512 + cl]
                rstd = stb[0:1, 0:cl]
                mr = stb[0:1, 512:512 + cl]
                nc.vector.tensor_scalar_mul(mean, ps1[:, 0:cl], 1.0 / D)
                nc.vector.tensor_mul(var, mean, mean)
                nc.vector.scalar_tensor_tensor(
                    var, ps2[:, 0:cl], 1.0 / D, var,
                    op0=OP.mult, op1=OP.subtract)
                nc.scalar.activation(var, var, AF.Sqrt, bias=eps_t)
                nc.vector.reciprocal(rstd, var)
                nc.vector.tensor_mul(mr, mean, rstd)
                RM = dvp.tile([P, 2 * 512], BF16, tag="rm", name="RM", bufs=3)
                nc.gpsimd.partition_broadcast(RM[:, 0:cl], rstd)
                nc.gpsimd.partition_broadcast(RM[:, 512:512 + cl], mr)
                rms[ci] = RM

            def ln_norm(rms, gi, out_tile, ci, t0, cl):
                RM = rms[ci]
                for dc in range(DC):
                    hsl = hT[:, dc * ptl + t0:dc * ptl + t0 + cl]
                    d1 = lnp.tile([P, 512], BF16, tag="d1", name=f"d1_{dc}")
                    eng = nc.gpsimd if dc >= 6 else nc.vector
                    eng.tensor_mul(d1[:, 0:cl], hsl, RM[:, 0:cl])
                    osl = out_tile[:, dc * ptl + t0:dc * ptl + t0 + cl]
                    if use_lng and gi is not None:
                        d2 = lnp.tile([P, 512], BF16, tag="d2",
                                      name=f"d2_{dc}")
                        nc.vector.tensor_sub(d2[:, 0:cl], d1[:, 0:cl],
                                             RM[:, 512:512 + cl])
                        g0 = 8 * (2 * gi)
                        nc.vector.tensor_scalar(
                            osl, d2[:, 0:cl],
                            lngb[:, g0 + dc:g0 + dc + 1],
                            lngb[:, g0 + 8 + dc:g0 + 8 + dc + 1],
                            op0=OP.mult, op1=OP.add)
                    else:
                        nc.vector.tensor_sub(osl, d1[:, 0:cl],
                                             RM[:, 512:512 + cl])

            # ---------- initial residual (host LN0(emb)) ----------
            for dc in range(DC):
                nc.sync.dma_start(out=hT[:, dc * ptl:(dc + 1) * ptl],
                                  in_=baseT[:, dc * ptl:(dc + 1) * ptl])

            # ---------- layers ----------
            for l in range(L):
                xh = xhp.tile([P, DC * ptl], BF16, tag="xh", name=f"xh{l}a")

                # ---- K/Q/V + attention, interleaved ----
                KT = big.tile([P, DC * ptl], BF16, tag="b18", name=f"KT{l}")
                Vsb = big.tile([P, nt * H * 65], BF16, tag="b18", name=f"Vsb{l}")
                QT = big.tile([P, DC * ptl], BF16, tag="b18", name=f"QT{l}")
                ctxc = big.tile([P, DC * ptl], BF16, tag="b18", name=f"ctx{l}")
                if lt < ptl:
                    nc.vector.memset(
                        Vsb[:, (nt - 1) * H * 65:nt * H * 65], 0.0)
                ones_v = Vsb.rearrange("p (g x) -> p g x", x=65)[:, :, 64:65]
                nc.vector.memset(ones_v, 1.0)

                def v_tg(nh, tg, norm=None):
                    if norm is not None:
                        rms_, gi_, t0_, cl_ = norm
                        RM = rms_
                    if True:
                        tts = [t for t in range(4 * tg, min(4 * tg + 4, nt))
                               if lt - t * P > 0]
                        pvs = {}
                        for tt in tts:
                            pvs[tt] = pp.tile([P, 512], F32, tag="mm",
                                              name=f"psv{tt}_{nh}")
                        for dc in range(DC):
                            if norm is not None:
                                hsl = hT[:, dc * ptl + t0_:dc * ptl + t0_ + cl_]
                                d1 = lnp.tile([P, 512], BF16, tag="d1",
                                              name=f"d1v{dc}")
                                eng = nc.gpsimd if dc >= 6 else nc.vector
                                eng.tensor_mul(d1[:, 0:cl_], hsl, RM[:, 0:cl_])
                                nc.vector.tensor_sub(
                                    xh[:, dc * ptl + t0_:dc * ptl + t0_ + cl_],
                                    d1[:, 0:cl_], RM[:, 512:512 + cl_])
                            vrb = wcb.tile([P, 512], BF16, tag="w",
                                           name=f"vrb{nh}_{tg}_{dc}")
                            nc.sync.dma_start(
                                out=vrb,
                                in_=vrb_d[l][:, (nh * DC + dc) * 512:
                                             (nh * DC + dc + 1) * 512])
                            for tt in tts:
                                tl = min(P, lt - tt * P)
                                nc.tensor.matmul(
                                    pvs[tt][0:tl, :],
                                    lhsT=xh[:, dc * ptl + tt * P:dc * ptl + tt * P + tl],
                                    rhs=vrb,
                                    start=(dc == 0), stop=(dc == DC - 1))
                        for tt in tts:
                            tl = min(P, lt - tt * P)
                            pv = pvs[tt][0:tl, :].rearrange(
                                "p (h x) -> p h x", h=8)
                            ov = Vsb[0:tl, (tt * H + nh * 8) * 65:
                                     (tt * H + nh * 8 + 8) * 65].rearrange(
                                "p (h x) -> p h x", x=65)[:, :, 0:64]
                            nc.scalar.copy(ov, pv)

                def kq_block(oc):
                    kcb = wcb.tile([P, DC * 128], BF16, tag="w",
                                   name=f"kcb{oc}")
                    nc.sync.dma_start(
                        out=kcb, in_=kcb_d[l][:, oc * D:(oc + 1) * D])
                    for (t0, cl) in chs:
                        ps = pp.tile([P, 512], F32, tag="mm", name=f"psk{oc}")
                        for dc in range(DC):
                            nc.tensor.matmul(
                                ps[:, 0:cl],
                                lhsT=kcb[:, dc * 128:dc * 128 + 128],
                                rhs=xh[:, dc * ptl + t0:dc * ptl + t0 + cl],
                                start=(dc == 0), stop=(dc == DC - 1))
                        if oc % 2 == 0:
                            nc.vector.tensor_scalar_add(
                                KT[:, oc * ptl + t0:oc * ptl + t0 + cl],
                                ps[:, 0:cl], bcol(l, 1)[:, oc:oc + 1])
                        else:
                            nc.scalar.activation(
                                KT[:, oc * ptl + t0:oc * ptl + t0 + cl],
                                ps[:, 0:cl], AF.Identity,
                                bias=bcol(l, 1)[:, oc:oc + 1])
                    qcb = wcb.tile([P, DC * 128], BF16, tag="w",
                                   name=f"qcb{oc}")
                    nc.sync.dma_start(
                        out=qcb, in_=qcb_d[l][:, oc * D:(oc + 1) * D])
                    for (t0, cl) in chs:
                        ps = pp.tile([P, 512], F32, tag="mm", name=f"psq{oc}")
                        for dc in range(DC):
                            nc.tensor.matmul(
                                ps[:, 0:cl],
                                lhsT=qcb[:, dc * 128:dc * 128 + 128],
                                rhs=xh[:, dc * ptl + t0:dc * ptl + t0 + cl],
                                start=(dc == 0), stop=(dc == DC - 1))
                        nc.scalar.activation(
                            QT[:, oc * ptl + t0:oc * ptl + t0 + cl],
                            ps[:, 0:cl], AF.Identity,
                            bias=bcol(l, 0)[:, oc:oc + 1])
                    if lt < ptl:
                        nc.vector.memset(KT[:, oc * ptl + lt:(oc + 1) * ptl],
                                         0.0)
                        nc.vector.memset(QT[:, oc * ptl + lt:(oc + 1) * ptl],
                                         0.0)

                def head_scores(h):
                    dch, po = h // 2, (h % 2) * 64
                    est = estp.tile([P, nt * EW], BF16, tag="est",
                                    name=f"est{h}")
                    ests[h] = est
                    for j in range(nt):
                        if wov:
                            w0 = min(max(j * P - wov, 0), ptl - EW)
                            nq = EW
                            lo = w0
                        else:
                            loj = max(j - 1, 0)
                            hi = min(j + 1, nt - 1)
                            nq = (hi - loj + 1) * P
                            w0 = min(max(j - 1, 0), nt - 3) * P
                            lo = loj * P
                        pst = pp.tile([P, 384], F32, tag="mm", name=f"pst{j}")
                        nc.tensor.matmul(
                            pst[:, 0:nq],
                            lhsT=KT[po:po + 64, dch * ptl + j * P:dch * ptl + j * P + P],
                            rhs=QT[po:po + 64, dch * ptl + lo:dch * ptl + lo + nq],
                            start=True, stop=True)
                        esl = est[:, j * EW + (lo - w0):j * EW + (lo - w0) + nq]
                        nc.scalar.activation(esl, pst[:, 0:nq], AF.Exp,
                                             scale=float(SCALE))
                    nc.vector.tensor_mul(est, est, masks)

                def head_ctx(h):
                    dch, po = h // 2, (h % 2) * 64
                    est = ests[h]
                    for qg in range((nt + 3) // 4):
                        qts = [q for q in range(4 * qg, min(4 * qg + 4, nt))]
                        gw = len(qts) * P
                        psc = pp.tile([65, 512], F32, tag="mm", name=f"psc{qg}")
                        for qi, qt in enumerate(qts):
                            if wov:
                                regions = [(0, wov, [qt, qt - 1]),
                                           (wov, P - wov, [qt]),
                                           (P - wov, P, [qt, qt + 1])]
                                for (a, b, js0) in regions:
                                    if b <= a:
                                        continue
                                    js = [j for j in js0 if 0 <= j < nt]
                                    oc_ = psc[:, qi * P + a:qi * P + b]
                                    for kk, j in enumerate(js):
                                        w0 = min(max(j * P - wov, 0),
                                                 ptl - EW)
                                        qa = qt * P + a - w0
                                        rsl = est[:, j * EW + qa:
                                                  j * EW + qa + (b - a)]
                                        nc.tensor.matmul(
                                            oc_,
                                            lhsT=Vsb[:, (j * H + h) * 65:
                                                     (j * H + h) * 65 + 65],
                                            rhs=rsl,
                                            start=(kk == 0),
                                            stop=(kk == len(js) - 1))
                            else:
                                js = [j for j in (qt - 1, qt, qt + 1)
                                      if 0 <= j < nt]
                                for kk, j in enumerate(js):
                                    w0 = min(max(j - 1, 0), nt - 3) * P
                                    rsl = est[:, j * EW + qt * P - w0:
                                              j * EW + qt * P - w0 + P]
                                    nc.tensor.matmul(
                                        psc[:, qi * P:(qi + 1) * P],
                                        lhsT=Vsb[:, (j * H + h) * 65:
                                                 (j * H + h) * 65 + 65],
                                        rhs=rsl,
                                        start=(kk == 0), stop=(kk == len(js) - 1))
                        dinv = dvp.tile([1, 512], BF16, tag="dinv",
                                        name=f"dinv{qg}")
                        nc.vector.reciprocal(dinv[:, 0:gw], psc[64:65, 0:gw])
                        dnb = dvp.tile([P, 512], BF16, tag="dnb",
                                       name=f"dnb{qg}")
                        nc.gpsimd.partition_broadcast(dnb[0:64, 0:gw],
                                                      dinv[:, 0:gw])
                        nc.vector.tensor_mul(
                            ctxc[po:po + 64,
                                 dch * ptl + qg * 512:dch * ptl + qg * 512 + gw],
                            psc[0:64, 0:gw], dnb[0:64, 0:gw])

                ests = {}
                gi1 = 2 * l if use_lng else None
                rms1 = {}
                for ci, (t0, cl) in enumerate(chs):
                    ln_stats(rms1, ci, t0, cl)
                for ci, (t0, cl) in enumerate(chs):
                    if use_lng:
                        ln_norm(rms1, gi1, xh, ci, t0, cl)
                        v_tg(0, ci)
                    else:
                        v_tg(0, ci, norm=(rms1[ci], gi1, t0, cl))
                kq_block(0)
                ocbs = [None, None]
                for oc in range(1, DC):
                    if oc == 5:
                        for ci in range(len(chs)):
                            v_tg(1, ci)
                    head_scores(2 * oc - 2)
                    head_scores(2 * oc - 1)
                    kq_block(oc)
                    if oc == 6:
                        for half in range(2):
                            ot = wcb.tile([P, 4 * DC * 128], BF16, tag="w",
                                          name=f"ocb{half}")
                            nc.sync.dma_start(
                                out=ot,
                                in_=ocb_d[l][:, half * 4 * D:(half + 1) * 4 * D])
                            ocbs[half] = ot
                    head_ctx(2 * oc - 2)
                    head_ctx(2 * oc - 1)
                head_scores(14)
                head_scores(15)
                head_ctx(14)
                head_ctx(15)
                # (ocb1/ocb2 DMAs were emitted during attention)

                # ---- O-projection (chunk-outer) + residual + LN2 ----
                xh = xhp.tile([P, DC * ptl], BF16, tag="xh", name=f"xh{l}b")
                gi2 = 2 * l + 1 if use_lng else None
                rms2 = {}
                for ci, (t0, cl) in enumerate(chs):
                    for do_ in range(DC):
                        ocb = ocbs[do_ // 4]
                        ob = (do_ % 4) * DC * 128
                        ps = pp.tile([P, 512], F32, tag="mm", name=f"pso{do_}")
                        for dc in range(DC):
                            nc.tensor.matmul(
                                ps[:, 0:cl],
                                lhsT=ocb[:, ob + dc * 128:ob + dc * 128 + 128],
                                rhs=ctxc[:, dc * ptl + t0:dc * ptl + t0 + cl],
                                start=(dc == 0), stop=(dc == DC - 1))
                        hsl = hT[:, do_ * ptl + t0:do_ * ptl + t0 + cl]
                        nc.vector.scalar_tensor_tensor(
                            hsl, ps[:, 0:cl], bcol(l, 3)[:, do_:do_ + 1], hsl,
                            op0=OP.add, op1=OP.add)
                    ln_stats(rms2, ci, t0, cl)
                for ci, (t0, cl) in enumerate(chs):
                    ln_norm(rms2, gi2, xh, ci, t0, cl)

                # ---- FFN ----
                Us = [big.tile([P, 8 * ptl], BF16, tag="b18", name=f"U{l}_{i}")
                      for i in range(4)]

                def usl(fc, t0, cl):
                    t = Us[fc // 8]
                    k = fc % 8
                    return t[:, k * ptl + t0:k * ptl + t0 + cl]

                for fcb in range(8):
                    w1cb = wcb.tile([P, DC * 512], BF16, tag="w",
                                    name=f"w1cb{fcb}")
                    nc.sync.dma_start(
                        out=w1cb,
                        in_=w1cb_d[l][:, fcb * DC * 512:(fcb + 1) * DC * 512])
                    for fc2 in range(4):
                        fc = fcb * 4 + fc2
                        for (t0, cl) in chs:
                            ps = pp.tile([P, 512], F32, tag="mm",
                                         name=f"psf{fc2}")
                            for dc in range(DC):
                                nc.tensor.matmul(
                                    ps[:, 0:cl],
                                    lhsT=w1cb[:, dc * 512 + fc2 * 128:
                                              dc * 512 + fc2 * 128 + 128],
                                    rhs=xh[:, dc * ptl + t0:dc * ptl + t0 + cl],
                                    start=(dc == 0), stop=(dc == DC - 1))
                            bidx = 5 + fc // 8
                            nc.scalar.activation(
                                usl(fc, t0, cl), ps[:, 0:cl], AF.Gelu,
                                bias=bcol(l, bidx)[:, fc % 8:fc % 8 + 1])
                for do_ in range(DC):
                    w2cb = wcb.tile([P, FC * 128], BF16, tag="w",
                                    name=f"w2cb{do_}")
                    nc.sync.dma_start(
                        out=w2cb,
                        in_=w2cb_d[l][:, do_ * FC * 128:(do_ + 1) * FC * 128])
                    for (t0, cl) in chs:
                        ps = pp.tile([P, 512], F32, tag="mm", name=f"psh{do_}")
                        for fc in range(FC):
                            nc.tensor.matmul(
                                ps[:, 0:cl],
                                lhsT=w2cb[:, fc * 128:fc * 128 + 128],
                                rhs=usl(fc, t0, cl),
                                start=(fc == 0), stop=(fc == FC - 1))
                        hsl = hT[:, do_ * ptl + t0:do_ * ptl + t0 + cl]
                        nc.vector.scalar_tensor_tensor(
                            hsl, ps[:, 0:cl], bcol(l, 4)[:, do_:do_ + 1], hsl,
                            op0=OP.add, op1=OP.add)
                    if l == L - 1:
                        for (t0o, clo) in chs:
                            nc.sync.dma_start(
                                out=houtT[:, do_ * ptl + t0o:do_ * ptl + t0o + clo],
                                in_=hT[:, do_ * ptl + t0o:do_ * ptl + t0o + clo])

    nc.compile()
    return nc


_NC_CACHE = {}


def _get_nc(lt=1032, nt=9, use_lng=False, wov=16):
    key = (lt, nt, use_lng, wov)
    if key not in _NC_CACHE:
        _NC_CACHE[key] = _build(lt, nt, use_lng, wov)
    return _NC_CACHE[key]


def _pack_shared(inputs, lt, nt, use_lng):
    bf = np.dtype("bfloat16") if hasattr(np, "bfloat16") else None
    import ml_dtypes
    BFD = ml_dtypes.bfloat16

    def b16(x):
        return np.ascontiguousarray(np.asarray(x, np.float32).astype(BFD))

    shared = {}
    for l in range(L):
        Wq = np.asarray(inputs["Wq"][l], np.float32)
        Wk = np.asarray(inputs["Wk"][l], np.float32)
        Wv = np.asarray(inputs["Wv"][l], np.float32)
        Wo = np.asarray(inputs["Wo"][l], np.float32)
        W1 = np.asarray(inputs["W1"][l], np.float32)
        W2 = np.asarray(inputs["W2"][l], np.float32)

        def colblocks(W, ocn):  # [D, D] -> [P, ocn*DC*128]
            # block (oc): [p, dc, c] = W[dc*128+p, oc*128+c]
            Wr = W.reshape(DC, P, ocn, 128)  # [dc, p, oc, c]
            return np.ascontiguousarray(
                Wr.transpose(1, 2, 0, 3).reshape(P, ocn * DC * 128))

        shared[f"kcb{l}"] = b16(colblocks(Wk, DC))
        shared[f"qcb{l}"] = b16(colblocks(Wq, DC))
        shared[f"ocb{l}"] = b16(colblocks(Wo, DC))
        # vrb: [p, nh, dc, c] = Wv[dc*128+p, nh*512+c]
        Wvr = Wv.reshape(DC, P, 2, 512)
        shared[f"vrb{l}"] = b16(
            Wvr.transpose(1, 2, 0, 3).reshape(P, 2 * DC * 512))
        # w1cb: [p, fcb, dc, c] = W1[dc*128+p, fcb*512+c]
        W1r = W1.reshape(DC, P, 8, 512)
        shared[f"w1cb{l}"] = b16(
            W1r.transpose(1, 2, 0, 3).reshape(P, 8 * DC * 512))
        # w2cb: [p, do, fc, c] = W2[fc*128+p, do*128+c]
        W2r = W2.reshape(FC, P, DC, 128)
        shared[f"w2cb{l}"] = b16(
            W2r.transpose(1, 2, 0, 3).reshape(P, DC * FC * 128))

    cbw = np.zeros((P, 2 + 96 * L), np.float32)
    cbw[:, 0] = 1.0
    cbw[0, 1] = EPS
    for l in range(L):
        c0 = 2 + 96 * l
        # bv is folded into bo: probs sum to 1, so ctx@Wo + bo with V+bv
        # equals (ctx from plain V)@Wo + (bo + bv@Wo).
        bo_eff = (np.asarray(inputs["bo"][l], np.float32)
                  + np.asarray(inputs["bv"][l], np.float32)
                  @ np.asarray(inputs["Wo"][l], np.float32))
        vals = {"bq": np.asarray(inputs["bq"][l], np.float32),
                "bk": np.asarray(inputs["bk"][l], np.float32),
                "bv": np.zeros(D, np.float32),
                "bo": bo_eff,
                "b2": np.asarray(inputs["b2"][l], np.float32)}
        for i, key in enumerate(("bq", "bk", "bv", "bo", "b2")):
            cbw[:, c0 + 8 * i:c0 + 8 * i + 8] = vals[key].reshape(DC, P).T
        b1v = np.asarray(inputs["b1"][l], np.float32)
        cbw[:, c0 + 40:c0 + 72] = b1v.reshape(FC, P).T
    shared["cb"] = np.ascontiguousarray(cbw)

    if use_lng:
        gb = np.zeros((P, 8 * (2 + 4 * L)), np.float32)
        # group 0: ln0 (handled as gi=None in build... keep identity)
        idx = 0
        for l in range(L):
            for which in range(2):
                gi = 2 * l + which
                g = np.asarray(inputs["ln1_g" if which == 0 else "ln2_g"][l],
                               np.float32)
                bb = np.asarray(inputs["ln1_b" if which == 0 else "ln2_b"][l],
                                np.float32)
                gb[:, 8 * (2 * gi):8 * (2 * gi) + 8] = g.reshape(DC, P).T
                gb[:, 8 * (2 * gi + 1):8 * (2 * gi + 1) + 8] = bb.reshape(DC, P).T
        shared["lngb"] = np.ascontiguousarray(gb)
    return shared


def _prep_core(inputs, b, start, n, lt, nt, wov):
    import ml_dtypes
    BFD = ml_dtypes.bfloat16
    ptl = nt * P

    def b16(x):
        return np.ascontiguousarray(np.asarray(x, np.float32).astype(BFD))

    ids = np.asarray(inputs["input_ids"][b, start:start + n])
    pid = np.asarray(inputs["patch_ids"][b, start:start + n]).astype(np.int64)
    pos_emb = np.asarray(inputs["pos_emb"], np.float32)
    hashes = np.asarray(inputs["hash_embeddings"], np.float32)
    tok = np.asarray(inputs["tok_emb"], np.float32)

    base = np.zeros((ptl, D), np.float32)
    emb = (tok[ids] + pos_emb[start:start + n]
           + hashes[b, start:start + n]).astype(np.float32)
    mu = emb.mean(-1, keepdims=True)
    var = ((emb - mu) ** 2).mean(-1, keepdims=True)
    g0 = np.asarray(inputs["ln0_g"], np.float32)
    b0 = np.asarray(inputs["ln0_b"], np.float32)
    base[:n] = (emb - mu) / np.sqrt(var + EPS) * g0 + b0
    baseT = np.ascontiguousarray(
        base.reshape(ptl, DC, P).transpose(2, 1, 0).reshape(P, DC * ptl))

    pidp = np.empty(ptl, np.int64)
    pidp[:n] = pid
    pidp[n:] = -np.arange(1, ptl - n + 1)

    ew = (128 + 2 * wov) if wov else 384
    m = np.zeros((nt, P, ew), np.float32)
    for j in range(nt):
        if wov:
            w0 = int(np.clip(j * P - wov, 0, ptl - ew))
        else:
            w0 = int(np.clip(j - 1, 0, nt - 3)) * P
        kk = pidp[j * P:(j + 1) * P]
        qq = pidp[w0:w0 + ew]
        m[j] = (kk[:, None] == qq[None, :]).astype(np.float32)
    masks = b16(m.transpose(1, 0, 2).reshape(P, nt * ew))
    return {"baseT": baseT, "masks": masks}


def kernel(**inputs):
    pid_all = np.asarray(inputs["patch_ids"])

    shards = []
    for b in range(B):
        pid = np.asarray(pid_all[b])
        bnd = np.nonzero(pid[1:] != pid[:-1])[0] + 1
        cand = bnd[(bnd >= S - 1152) & (bnd <= 1152)]
        if len(cand) == 0:
            raise RuntimeError("no patch boundary near S/2; cannot shard")
        s = int(cand[np.argmin(np.abs(cand - S // 2))])
        shards.append((b, 0, s))
        shards.append((b, s, S - s))

    lt = max(n for _, _, n in shards)
    lt = max(lt, 1026)  # floor so chunk 3 isn't degenerate-tiny
    nt = (lt + P - 1) // P

    maxrun = 0
    for b in range(B):
        p = np.asarray(pid_all[b])
        bnd = np.nonzero(p[1:] != p[:-1])[0] + 1
        edges = np.concatenate([[0], bnd, [len(p)]])
        maxrun = max(maxrun, int(np.diff(edges).max()))
    wov = next((w for w in (16, 32, 64) if maxrun <= w), None)

    use_lng = not (
        all(np.all(np.asarray(inputs[k]) == 1.0)
            for k in ("ln1_g", "ln2_g")) and
        all(np.all(np.asarray(inputs[k]) == 0.0)
            for k in ("ln1_b", "ln2_b")))
    if use_lng:
        raise NotImplementedError(
            "non-identity LN affine not supported in fast path")

    shared = _pack_shared(inputs, lt, nt, use_lng)
    in_maps = []
    for b, start, n in shards:
        mcore = dict(shared)
        mcore.update(_prep_core(inputs, b, start, n, lt, nt, wov))
        in_maps.append(mcore)

    nc = _get_nc(lt, nt, use_lng, wov)
    res = bass_utils.run_bass_kernel_spmd(nc, in_maps,
                                          core_ids=list(range(NCORES)))

    ptl = nt * P
    out = np.zeros((B, S, D), np.float32)
    for i, (b, start, n) in enumerate(shards):
        ht = res.results[i]["houtT"]
        hfull = ht.reshape(P, DC, ptl).transpose(2, 1, 0).reshape(ptl, D)
        out[b, start:start + n] = hfull[:n]
    return out


if __name__ == "__main__":
    import sys
    lt = int(sys.argv[1]) if len(sys.argv) > 1 else 1032
    _get_nc(lt, (lt + P - 1) // P, False)
    print("built ok")


# revision 28
# speedup vs baseline: 1.4278x; 1.0009x over previous
"""BLT local encoder (2-layer transformer, patch-equality block-diagonal attention)
on 8 Trainium2 NeuronCores.

v2. Sharding: each of the 4 sequences splits at a patch-run boundary nearest
S/2 -> 8 independent shards, one per core, zero cross-core communication.

Kernel design (per core, L_tok = max shard length ~1032):
- Residual hT kept float32 feature-major [P, 8dc x PTL]; everything else bf16.
- Weights prepacked host-side into SBUF-ready bf16 col/row blocks, streamed
  once per layer (no restreaming), double-buffered.
- One LayerNorm per sublayer, output xh bf16 reused by Q, K and V.
- Full-shard attention: per (head, key-tile j) one score matmul with moving
  dim >= 256; softmax denominator via a ones-column appended to V (row 64 of
  the ctx psum); per-head normalize fused into the psum->SBUF copy.
- Engine split: PE matmuls; DVE normalize/copies/masks; Act square/exp/gelu;
  Pool partition-broadcasts + residual adds.
"""

import numpy as np

import concourse.bass as bass
import concourse.tile as tile
from concourse import bacc, bass_utils, mybir

F32 = mybir.dt.float32
F32R = mybir.dt.float32r
BF16 = mybir.dt.bfloat16
AF = mybir.ActivationFunctionType
OP = mybir.AluOpType

B, S, D, H, F, L = 4, 2048, 1024, 16, 4096, 2
DH = D // H      # 64
DC = D // 128    # 8
FC = F // 128    # 32
EPS = 1e-5
SCALE = 1.0 / np.sqrt(DH)
P = 128
VP = 384         # vocab 260 padded
VC = VP // 128   # 3
NCORES = 8


def _chunks(lt):
    out = []
    o = 0
    while o < lt:
        c = min(512, lt - o)
        out.append((o, c))
        o += c
    return out


def _build(lt, nt, use_lng, wov):
    """lt: tokens; nt: tiles; use_lng: ln affine ops; wov: +-wov-token window."""
    ptl = nt * P
    EW = (128 + 2 * wov) if wov else 384
    chs = _chunks(lt)
    nc = bacc.Bacc("TRN2", target_bir_lowering=False, debug=False,
                   num_devices=NCORES)

    def din(name, shape, dt=BF16):
        return nc.dram_tensor(name, shape, dt, kind="ExternalInput").ap()

    baseT = din("baseT", [P, DC * ptl], F32R)
    masks_d = din("masks", [P, nt * EW])
    # prepacked weights
    kcb_d, qcb_d, ocb_d, vrb_d, w1cb_d, w2cb_d = [], [], [], [], [], []
    for l in range(L):
        kcb_d.append(din(f"kcb{l}", [P, DC * DC * 128]))
        qcb_d.append(din(f"qcb{l}", [P, DC * DC * 128]))
        ocb_d.append(din(f"ocb{l}", [P, DC * DC * 128]))
        vrb_d.append(din(f"vrb{l}", [P, DC * D]))
        w1cb_d.append(din(f"w1cb{l}", [P, 8 * DC * 512]))
        w2cb_d.append(din(f"w2cb{l}", [P, DC * FC * 128]))
    # packed per-feature consts: [P, col] layout, 8 cols per D-vector
    # cols: 0 ones | 1 eps(row0) | then per layer l at 2+64*l:
    #   bq 0:8 bk 8:16 bv 16:24 bo 24:32 b2 32:40 b1 40:72 (unused gap)
    # ln g/b (if use_lng): separate tensor lngb
    cb_d = din("cb", [P, 2 + 96 * L], F32)
    lngb_d = din("lngb", [P, 8 * (2 + 4 * L)], F32) if use_lng else None
    houtT = nc.dram_tensor("houtT", [P, DC * ptl], F32R,
                           kind="ExternalOutput").ap()

    with tile.TileContext(nc) as tc:
        with (
            nc.allow_low_precision(
                reason="bf16 softmax/LN staging validated vs reference"),
            tc.tile_pool(name="pers", bufs=1) as pers,
            tc.tile_pool(name="big", bufs=4) as big,
            tc.tile_pool(name="xhp", bufs=1) as xhp,
            tc.tile_pool(name="wcb", bufs=4) as wcb,
            tc.tile_pool(name="est", bufs=2) as estp,
            tc.tile_pool(name="lnt", bufs=4) as lnp,
            tc.tile_pool(name="sm", bufs=2) as smp,
            tc.tile_pool(name="dv", bufs=3) as dvp,
            tc.tile_pool(name="pp", bufs=8, space="PSUM") as pp,
        ):
            cb = pers.tile([P, 2 + 96 * L], F32, tag="cb")
            nc.sync.dma_start(out=cb, in_=cb_d)
            eps_t = cb[0:1, 1:2]
            ones_r = pers.tile([P, 1], F32R, tag="ones_r")
            nc.vector.tensor_copy(ones_r, cb[:, 0:1])
            ones_b = pers.tile([P, 1], BF16, tag="ones_b")
            nc.vector.tensor_copy(ones_b, cb[:, 0:1])
            if use_lng:
                lngb = pers.tile([P, 8 * (2 + 4 * L)], F32, tag="lngb")
                nc.sync.dma_start(out=lngb, in_=lngb_d)

            masks = pers.tile([P, nt * EW], BF16, tag="masks")
            nc.sync.dma_start(out=masks, in_=masks_d)

            hT = pers.tile([P, DC * ptl], F32R, tag="hT")

            def bcol(l, i):  # bias col i (in 8-col groups) for layer l
                c0 = 2 + 96 * l + 8 * i
                return cb[:, c0:c0 + 8]

            def ln_stats(rms, ci, t0, cl):
                ps1 = pp.tile([1, 512], F32, tag="mm", name="lns1")
                ps2 = pp.tile([1, 512], F32, tag="mm", name="lns2")
                for dc in range(DC):
                    hsl = hT[:, dc * ptl + t0:dc * ptl + t0 + cl]
                    sq = lnp.tile([P, 512], BF16, tag="sq", name=f"sq{dc}")
                    if dc < 4:
                        nc.scalar.activation(sq[:, 0:cl], hsl, AF.Square)
                    elif dc < 7:
                        nc.vector.tensor_mul(sq[:, 0:cl], hsl, hsl)
                    else:
                        nc.gpsimd.tensor_mul(sq[:, 0:cl], hsl, hsl)
                    nc.tensor.matmul(ps1[:, 0:cl], lhsT=ones_r, rhs=hsl,
                                     start=(dc == 0), stop=(dc == DC - 1))
                    nc.tensor.matmul(ps2[:, 0:cl], lhsT=ones_b,
                                     rhs=sq[:, 0:cl],
                                     start=(dc == 0), stop=(dc == DC - 1))
                st = smp.tile([P, 2 * 512], F32, tag="st", name="st")
                stb = smp.tile([P, 2 * 512], BF16, tag="stb", name="stb")
                mean = st[0:1, 0:cl]
                var = st[0:1, 512:512 + cl]
                rstd = stb[0:1, 0:cl]
                mr = stb[0:1, 512:512 + cl]
                nc.vector.tensor_scalar_mul(mean, ps1[:, 0:cl], 1.0 / D)
                nc.vector.tensor_mul(var, mean, mean)
                nc.vector.scalar_tensor_tensor(
                    var, ps2[:, 0:cl], 1.0 / D, var,
                    op0=OP.mult, op1=OP.subtract)
                nc.scalar.activation(var, var, AF.Sqrt, bias=eps_t)
                nc.vector.reciprocal(rstd, var)
                nc.vector.tensor_mul(mr, mean, rstd)
                RM = dvp.tile([P, 2 * 512], BF16, tag="rm", name="RM", bufs=3)
                nc.gpsimd.partition_broadcast(RM[:, 0:cl], rstd)
                nc.gpsimd.partition_broadcast(RM[:, 512:512 + cl], mr)
                rms[ci] = RM

            def ln_norm(rms, gi, out_tile, ci, t0, cl):
                RM = rms[ci]
                for dc in range(DC):
                    hsl = hT[:, dc * ptl + t0:dc * ptl + t0 + cl]
                    d1 = lnp.tile([P, 512], BF16, tag="d1", name=f"d1_{dc}")
                    eng = nc.gpsimd if dc >= 6 else nc.vector
                    eng.tensor_mul(d1[:, 0:cl], hsl, RM[:, 0:cl])
                    osl = out_tile[:, dc * ptl + t0:dc * ptl + t0 + cl]
                    if use_lng and gi is not None:
                        d2 = lnp.tile([P, 512], BF16, tag="d2",
                                      name=f"d2_{dc}")
                        nc.vector.tensor_sub(d2[:, 0:cl], d1[:, 0:cl],
                                             RM[:, 512:512 + cl])
                        g0 = 8 * (2 * gi)
                        nc.vector.tensor_scalar(
                            osl, d2[:, 0:cl],
                            lngb[:, g0 + dc:g0 + dc + 1],
                            lngb[:, g0 + 8 + dc:g0 + 8 + dc + 1],
                            op0=OP.mult, op1=OP.add)
                    else:
                        nc.vector.tensor_sub(osl, d1[:, 0:cl],
                                             RM[:, 512:512 + cl])

            # ---------- initial residual (host LN0(emb)) ----------
            for dc in range(DC):
                nc.sync.dma_start(out=hT[:, dc * ptl:(dc + 1) * ptl],
                                  in_=baseT[:, dc * ptl:(dc + 1) * ptl])

            # ---------- layers ----------
            for l in range(L):
                xh = xhp.tile([P, DC * ptl], BF16, tag="xh", name=f"xh{l}a")

                # ---- K/Q/V + attention, interleaved ----
                KT = big.tile([P, DC * ptl], BF16, tag="b18", name=f"KT{l}")
                Vsb = big.tile([P, nt * H * 65], BF16, tag="b18", name=f"Vsb{l}")
                QT = big.tile([P, DC * ptl], BF16, tag="b18", name=f"QT{l}")
                ctxc = big.tile([P, DC * ptl], BF16, tag="b18", name=f"ctx{l}")
                if lt < ptl:
                    nc.vector.memset(
                        Vsb[:, (nt - 1) * H * 65:nt * H * 65], 0.0)
                ones_v = Vsb.rearrange("p (g x) -> p g x", x=65)[:, :, 64:65]
                nc.vector.memset(ones_v, 1.0)

                def v_tg(nh, tg, norm=None):
                    if norm is not None:
                        rms_, gi_, t0_, cl_ = norm
                        RM = rms_
                    if True:
                        tts = [t for t in range(4 * tg, min(4 * tg + 4, nt))
                               if lt - t * P > 0]
                        pvs = {}
                        for tt in tts:
                            pvs[tt] = pp.tile([P, 512], F32, tag="mm",
                                              name=f"psv{tt}_{nh}")
                        for dc in range(DC):
                            if norm is not None:
                                hsl = hT[:, dc * ptl + t0_:dc * ptl + t0_ + cl_]
                                d1 = lnp.tile([P, 512], BF16, tag="d1",
                                              name=f"d1v{dc}")
                                eng = nc.gpsimd if dc >= 6 else nc.vector
                                eng.tensor_mul(d1[:, 0:cl_], hsl, RM[:, 0:cl_])
                                nc.vector.tensor_sub(
                                    xh[:, dc * ptl + t0_:dc * ptl + t0_ + cl_],
                                    d1[:, 0:cl_], RM[:, 512:512 + cl_])
                            vrb = wcb.tile([P, 512], BF16, tag="w",
                                           name=f"vrb{nh}_{tg}_{dc}")
                            nc.sync.dma_start(
                                out=vrb,
                                in_=vrb_d[l][:, (nh * DC + dc) * 512:
                                             (nh * DC + dc + 1) * 512])
                            for tt in tts:
                                tl = min(P, lt - tt * P)
                                nc.tensor.matmul(
                                    pvs[tt][0:tl, :],
                                    lhsT=xh[:, dc * ptl + tt * P:dc * ptl + tt * P + tl],
                                    rhs=vrb,
                                    start=(dc == 0), stop=(dc == DC - 1))
                        for tt in tts:
                            tl = min(P, lt - tt * P)
                            pv = pvs[tt][0:tl, :].rearrange(
                                "p (h x) -> p h x", h=8)
                            ov = Vsb[0:tl, (tt * H + nh * 8) * 65:
                                     (tt * H + nh * 8 + 8) * 65].rearrange(
                                "p (h x) -> p h x", x=65)[:, :, 0:64]
                            nc.scalar.copy(ov, pv)

                def kq_block(oc):
                    kcb = wcb.tile([P, DC * 128], BF16, tag="w",
                                   name=f"kcb{oc}")
                    nc.sync.dma_start(
                        out=kcb, in_=kcb_d[l][:, oc * D:(oc + 1) * D])
                    for (t0, cl) in chs:
                        ps = pp.tile([P, 512], F32, tag="mm", name=f"psk{oc}")
                        for dc in range(DC):
                            nc.tensor.matmul(
                                ps[:, 0:cl],
                                lhsT=kcb[:, dc * 128:dc * 128 + 128],
                                rhs=xh[:, dc * ptl + t0:dc * ptl + t0 + cl],
                                start=(dc == 0), stop=(dc == DC - 1))
                        if oc % 2 == 0:
                            nc.vector.tensor_scalar_add(
                                KT[:, oc * ptl + t0:oc * ptl + t0 + cl],
                                ps[:, 0:cl], bcol(l, 1)[:, oc:oc + 1])
                        else:
                            nc.scalar.activation(
                                KT[:, oc * ptl + t0:oc * ptl + t0 + cl],
                                ps[:, 0:cl], AF.Identity,
                                bias=bcol(l, 1)[:, oc:oc + 1])
                    qcb = wcb.tile([P, DC * 128], BF16, tag="w",
                                   name=f"qcb{oc}")
                    nc.sync.dma_start(
                        out=qcb, in_=qcb_d[l][:, oc * D:(oc + 1) * D])
                    for (t0, cl) in chs:
                        ps = pp.tile([P, 512], F32, tag="mm", name=f"psq{oc}")
                        for dc in range(DC):
                            nc.tensor.matmul(
                                ps[:, 0:cl],
                                lhsT=qcb[:, dc * 128:dc * 128 + 128],
                                rhs=xh[:, dc * ptl + t0:dc * ptl + t0 + cl],
                                start=(dc == 0), stop=(dc == DC - 1))
                        nc.scalar.activation(
                            QT[:, oc * ptl + t0:oc * ptl + t0 + cl],
                            ps[:, 0:cl], AF.Identity,
                            bias=bcol(l, 0)[:, oc:oc + 1])
                    if lt < ptl:
                        nc.vector.memset(KT[:, oc * ptl + lt:(oc + 1) * ptl],
                                         0.0)
                        nc.vector.memset(QT[:, oc * ptl + lt:(oc + 1) * ptl],
                                         0.0)

                def head_scores(h):
                    dch, po = h // 2, (h % 2) * 64
                    est = estp.tile([P, nt * EW], BF16, tag="est",
                                    name=f"est{h}")
                    ests[h] = est
                    for j in range(nt):
                        if wov:
                            w0 = min(max(j * P - wov, 0), ptl - EW)
                            nq = EW
                            lo = w0
                        else:
                            loj = max(j - 1, 0)
                            hi = min(j + 1, nt - 1)
                            nq = (hi - loj + 1) * P
                            w0 = min(max(j - 1, 0), nt - 3) * P
                            lo = loj * P
                        pst = pp.tile([P, 384], F32, tag="mm", name=f"pst{j}")
                        nc.tensor.matmul(
                            pst[:, 0:nq],
                            lhsT=KT[po:po + 64, dch * ptl + j * P:dch * ptl + j * P + P],
                            rhs=QT[po:po + 64, dch * ptl + lo:dch * ptl + lo + nq],
                            start=True, stop=True)
                        esl = est[:, j * EW + (lo - w0):j * EW + (lo - w0) + nq]
                        nc.scalar.activation(esl, pst[:, 0:nq], AF.Exp,
                                             scale=float(SCALE))
                    nc.vector.tensor_mul(est, est, masks)

                def head_ctx(h):
                    dch, po = h // 2, (h % 2) * 64
                    est = ests[h]
                    for qg in range((nt + 3) // 4):
                        qts = [q for q in range(4 * qg, min(4 * qg + 4, nt))]
                        gw = len(qts) * P
                        psc = pp.tile([65, 512], F32, tag="mm", name=f"psc{qg}")
                        for qi, qt in enumerate(qts):
                            if wov:
                                regions = [(0, wov, [qt, qt - 1]),
                                           (wov, P - wov, [qt]),
                                           (P - wov, P, [qt, qt + 1])]
                                for (a, b, js0) in regions:
                                    if b <= a:
                                        continue
                                    js = [j for j in js0 if 0 <= j < nt]
                                    oc_ = psc[:, qi * P + a:qi * P + b]
                                    for kk, j in enumerate(js):
                                        w0 = min(max(j * P - wov, 0),
                                                 ptl - EW)
                                        qa = qt * P + a - w0
                                        rsl = est[:, j * EW + qa:
                                                  j * EW + qa + (b - a)]
                                        nc.tensor.matmul(
                                            oc_,
                                            lhsT=Vsb[:, (j * H + h) * 65:
                                                     (j * H + h) * 65 + 65],
                                            rhs=rsl,
                                            start=(kk == 0),
                                            stop=(kk == len(js) - 1))
                            else:
                                js = [j for j in (qt - 1, qt, qt + 1)
                                      if 0 <= j < nt]
                                for kk, j in enumerate(js):
                                    w0 = min(max(j - 1, 0), nt - 3) * P
                                    rsl = est[:, j * EW + qt * P - w0:
                                              j * EW + qt * P - w0 + P]
                                    nc.tensor.matmul(
                                        psc[:, qi * P:(qi + 1) * P],
                                        lhsT=Vsb[:, (j * H + h) * 65:
                                                 (j * H + h) * 65 + 65],
                                        rhs=rsl,
                                        start=(kk == 0), stop=(kk == len(js) - 1))
                        dinv = dvp.tile([1, 512], BF16, tag="dinv",
                                        name=f"dinv{qg}")
                        nc.vector.reciprocal(dinv[:, 0:gw], psc[64:65, 0:gw])
                        dnb = dvp.tile([P, 512], BF16, tag="dnb",
                                       name=f"dnb{qg}")
                        nc.gpsimd.partition_broadcast(dnb[0:64, 0:gw],
                                                      dinv[:, 0:gw])
                        nc.vector.tensor_mul(
                            ctxc[po:po + 64,
                                 dch * ptl + qg * 512:dch * ptl + qg * 512 + gw],
                            psc[0:64, 0:gw], dnb[0:64, 0:gw])

                ests = {}
                gi1 = 2 * l if use_lng else None
                rms1 = {}
                for ci, (t0, cl) in enumerate(chs):
                    ln_stats(rms1, ci, t0, cl)
                for ci, (t0, cl) in enumerate(chs):
                    if use_lng:
                        ln_norm(rms1, gi1, xh, ci, t0, cl)
                        v_tg(0, ci)
                    else:
                        v_tg(0, ci, norm=(rms1[ci], gi1, t0, cl))
                kq_block(0)
                ocbs = [None, None]
                for oc in range(1, DC):
                    if oc == 5:
                        for ci in range(len(chs)):
                            v_tg(1, ci)
                    head_scores(2 * oc - 2)
                    head_scores(2 * oc - 1)
                    kq_block(oc)
                    if oc == 6:
                        for half in range(2):
                            ot = wcb.tile([P, 4 * DC * 128], BF16, tag="w",
                                          name=f"ocb{half}")
                            nc.sync.dma_start(
                                out=ot,
                                in_=ocb_d[l][:, half * 4 * D:(half + 1) * 4 * D])
                            ocbs[half] = ot
                    head_ctx(2 * oc - 2)
                    head_ctx(2 * oc - 1)
                head_scores(14)
                head_scores(15)
                head_ctx(14)
                head_ctx(15)
                # (ocb1/ocb2 DMAs were emitted during attention)

                # ---- O-projection (chunk-outer) + residual + LN2 ----
                xh = xhp.tile([P, DC * ptl], BF16, tag="xh", name=f"xh{l}b")
                gi2 = 2 * l + 1 if use_lng else None
                rms2 = {}
                for ci, (t0, cl) in enumerate(chs):
                    for do_ in range(DC):
                        ocb = ocbs[do_ // 4]
                        ob = (do_ % 4) * DC * 128
                        ps = pp.tile([P, 512], F32, tag="mm", name=f"pso{do_}")
                        for dc in range(DC):
                            nc.tensor.matmul(
                                ps[:, 0:cl],
                                lhsT=ocb[:, ob + dc * 128:ob + dc * 128 + 128],
                                rhs=ctxc[:, dc * ptl + t0:dc * ptl + t0 + cl],
                                start=(dc == 0), stop=(dc == DC - 1))
                        hsl = hT[:, do_ * ptl + t0:do_ * ptl + t0 + cl]
                        nc.vector.scalar_tensor_tensor(
                            hsl, ps[:, 0:cl], bcol(l, 3)[:, do_:do_ + 1], hsl,
                            op0=OP.add, op1=OP.add)
                    ln_stats(rms2, ci, t0, cl)
                for ci, (t0, cl) in enumerate(chs):
                    ln_norm(rms2, gi2, xh, ci, t0, cl)

                # ---- FFN ----
                Us = [big.tile([P, 8 * ptl], BF16, tag="b18", name=f"U{l}_{i}")
                      for i in range(4)]

                def usl(fc, t0, cl):
                    t = Us[fc // 8]
                    k = fc % 8
                    return t[:, k * ptl + t0:k * ptl + t0 + cl]

                for fcb in range(8):
                    w1cb = wcb.tile([P, DC * 512], BF16, tag="w",
                                    name=f"w1cb{fcb}")
                    nc.sync.dma_start(
                        out=w1cb,
                        in_=w1cb_d[l][:, fcb * DC * 512:(fcb + 1) * DC * 512])
                    for fc2 in range(4):
                        fc = fcb * 4 + fc2
                        for (t0, cl) in chs:
                            ps = pp.tile([P, 512], F32, tag="mm",
                                         name=f"psf{fc2}")
                            for dc in range(DC):
                                nc.tensor.matmul(
                                    ps[:, 0:cl],
                                    lhsT=w1cb[:, dc * 512 + fc2 * 128:
                                              dc * 512 + fc2 * 128 + 128],
                                    rhs=xh[:, dc * ptl + t0:dc * ptl + t0 + cl],
                                    start=(dc == 0), stop=(dc == DC - 1))
                            bidx = 5 + fc // 8
                            nc.scalar.activation(
                                usl(fc, t0, cl), ps[:, 0:cl], AF.Gelu,
                                bias=bcol(l, bidx)[:, fc % 8:fc % 8 + 1])
                for do_ in range(DC):
                    w2cb = wcb.tile([P, FC * 128], BF16, tag="w",
                                    name=f"w2cb{do_}")
                    nc.sync.dma_start(
                        out=w2cb,
                        in_=w2cb_d[l][:, do_ * FC * 128:(do_ + 1) * FC * 128])
                    for (t0, cl) in chs:
                        ps = pp.tile([P, 512], F32, tag="mm", name=f"psh{do_}")
                        for fc in range(FC):
                            nc.tensor.matmul(
                                ps[:, 0:cl],
                                lhsT=w2cb[:, fc * 128:fc * 128 + 128],
                                rhs=usl(fc, t0, cl),
                                start=(fc == 0), stop=(fc == FC - 1))
                        hsl = hT[:, do_ * ptl + t0:do_ * ptl + t0 + cl]
                        nc.vector.scalar_tensor_tensor(
                            hsl, ps[:, 0:cl], bcol(l, 4)[:, do_:do_ + 1], hsl,
                            op0=OP.add, op1=OP.add)
                    if l == L - 1:
                        for (t0o, clo) in chs:
                            nc.sync.dma_start(
                                out=houtT[:, do_ * ptl + t0o:do_ * ptl + t0o + clo],
                                in_=hT[:, do_ * ptl + t0o:do_ * ptl + t0o + clo])

    nc.compile()
    return nc


_NC_CACHE = {}


def _get_nc(lt=1032, nt=9, use_lng=False, wov=16):
    key = (lt, nt, use_lng, wov)
    if key not in _NC_CACHE:
        _NC_CACHE[key] = _build(lt, nt, use_lng, wov)
    return _NC_CACHE[key]


def _pack_shared(inputs, lt, nt, use_lng):
    bf = np.dtype("bfloat16") if hasattr(np, "bfloat16") else None
    import ml_dtypes
    BFD = ml_dtypes.bfloat16

    def b16(x):
        return np.ascontiguousarray(np.asarray(x, np.float32).astype(BFD))

    shared = {}
    for l in range(L):
        Wq = np.asarray(inputs["Wq"][l], np.float32)
        Wk = np.asarray(inputs["Wk"][l], np.float32)
        Wv = np.asarray(inputs["Wv"][l], np.float32)
        Wo = np.asarray(inputs["Wo"][l], np.float32)
        W1 = np.asarray(inputs["W1"][l], np.float32)
        W2 = np.asarray(inputs["W2"][l], np.float32)

        def colblocks(W, ocn):  # [D, D] -> [P, ocn*DC*128]
            # block (oc): [p, dc, c] = W[dc*128+p, oc*128+c]
            Wr = W.reshape(DC, P, ocn, 128)  # [dc, p, oc, c]
            return np.ascontiguousarray(
                Wr.transpose(1, 2, 0, 3).reshape(P, ocn * DC * 128))

        shared[f"kcb{l}"] = b16(colblocks(Wk, DC))
        shared[f"qcb{l}"] = b16(colblocks(Wq, DC))
        shared[f"ocb{l}"] = b16(colblocks(Wo, DC))
        # vrb: [p, nh, dc, c] = Wv[dc*128+p, nh*512+c]
        Wvr = Wv.reshape(DC, P, 2, 512)
        shared[f"vrb{l}"] = b16(
            Wvr.transpose(1, 2, 0, 3).reshape(P, 2 * DC * 512))
        # w1cb: [p, fcb, dc, c] = W1[dc*128+p, fcb*512+c]
        W1r = W1.reshape(DC, P, 8, 512)
        shared[f"w1cb{l}"] = b16(
            W1r.transpose(1, 2, 0, 3).reshape(P, 8 * DC * 512))
        # w2cb: [p, do, fc, c] = W2[fc*128+p, do*128+c]
        W2r = W2.reshape(FC, P, DC, 128)
        shared[f"w2cb{l}"] = b16(
            W2r.transpose(1, 2, 0, 3).reshape(P, DC * FC * 128))

    cbw = np.zeros((P, 2 + 96 * L), np.float32)
    cbw[:, 0] = 1.0
    cbw[0, 1] = EPS
    for l in range(L):
        c0 = 2 + 96 * l
        # bv is folded into bo: probs sum to 1, so ctx@Wo + bo with V+bv
        # equals (ctx from plain V)@Wo + (bo + bv@Wo).
        bo_eff = (np.asarray(inputs["bo"][l], np.float32)
                  + np.asarray(inputs["bv"][l], np.float32)
                  @ np.asarray(inputs["Wo"][l], np.float32))
        vals = {"bq": np.asarray(inputs["bq"][l], np.float32),
                "bk": np.asarray(inputs["bk"][l], np.float32),
                "bv": np.zeros(D, np.float32),
                "bo": bo_eff,
                "b2": np.asarray(inputs["b2"][l], np.float32)}
        for i, key in enumerate(("bq", "bk", "bv", "bo", "b2")):
            cbw[:, c0 + 8 * i:c0 + 8 * i + 8] = vals[key].reshape(DC, P).T
        b1v = np.asarray(inputs["b1"][l], np.float32)
        cbw[:, c0 + 40:c0 + 72] = b1v.reshape(FC, P).T
    shared["cb"] = np.ascontiguousarray(cbw)

    if use_lng:
        gb = np.zeros((P, 8 * (2 + 4 * L)), np.float32)
        # group 0: ln0 (handled as gi=None in build... keep identity)
        idx = 0
        for l in range(L):
            for which in range(2):
                gi = 2 * l + which
                g = np.asarray(inputs["ln1_g" if which == 0 else "ln2_g"][l],
                               np.float32)
                bb = np.asarray(inputs["ln1_b" if which == 0 else "ln2_b"][l],
                                np.float32)
                gb[:, 8 * (2 * gi):8 * (2 * gi) + 8] = g.reshape(DC, P).T
                gb[:, 8 * (2 * gi + 1):8 * (2 * gi + 1) + 8] = bb.reshape(DC, P).T
        shared["lngb"] = np.ascontiguousarray(gb)
    return shared


def _prep_core(inputs, b, start, n, lt, nt, wov):
    import ml_dtypes
    BFD = ml_dtypes.bfloat16
    ptl = nt * P

    def b16(x):
        return np.ascontiguousarray(np.asarray(x, np.float32).astype(BFD))

    ids = np.asarray(inputs["input_ids"][b, start:start + n])
    pid = np.asarray(inputs["patch_ids"][b, start:start + n]).astype(np.int64)
    pos_emb = np.asarray(inputs["pos_emb"], np.float32)
    hashes = np.asarray(inputs["hash_embeddings"], np.float32)
    tok = np.asarray(inputs["tok_emb"], np.float32)

    base = np.zeros((ptl, D), np.float32)
    emb = (tok[ids] + pos_emb[start:start + n]
           + hashes[b, start:start + n]).astype(np.float32)
    mu = emb.mean(-1, keepdims=True)
    var = ((emb - mu) ** 2).mean(-1, keepdims=True)
    g0 = np.asarray(inputs["ln0_g"], np.float32)
    b0 = np.asarray(inputs["ln0_b"], np.float32)
    base[:n] = (emb - mu) / np.sqrt(var + EPS) * g0 + b0
    baseT = np.ascontiguousarray(
        base.reshape(ptl, DC, P).transpose(2, 1, 0).reshape(P, DC * ptl))

    pidp = np.empty(ptl, np.int64)
    pidp[:n] = pid
    pidp[n:] = -np.arange(1, ptl - n + 1)

    ew = (128 + 2 * wov) if wov else 384
    m = np.zeros((nt, P, ew), np.float32)
    for j in range(nt):
        if wov:
            w0 = int(np.clip(j * P - wov, 0, ptl - ew))
        else:
            w0 = int(np.clip(j - 1, 0, nt - 3)) * P
        kk = pidp[j * P:(j + 1) * P]
        qq = pidp[w0:w0 + ew]
        m[j] = (kk[:, None] == qq[None, :]).astype(np.float32)
    masks = b16(m.transpose(1, 0, 2).reshape(P, nt * ew))
    return {"baseT": baseT, "masks": masks}


def kernel(**inputs):
    pid_all = np.asarray(inputs["patch_ids"])

    shards = []
    for b in range(B):
        pid = np.asarray(pid_all[b])
        bnd = np.nonzero(pid[1:] != pid[:-1])[0] + 1
        cand = bnd[(bnd >= S - 1152) & (bnd <= 1152)]
        if len(cand) == 0:
            raise RuntimeError("no patch boundary near S/2; cannot shard")
        s = int(cand[np.argmin(np.abs(cand - S // 2))])
        shards.append((b, 0, s))
        shards.append((b, s, S - s))

    lt = max(n for _, _, n in shards)
    lt = max(lt, 1026)  # floor so chunk 3 isn't degenerate-tiny
    nt = (lt + P - 1) // P

    maxrun = 0
    for b in range(B):
        p = np.asarray(pid_all[b])
        bnd = np.nonzero(p[1:] != p[:-1])[0] + 1
        edges = np.concatenate([[0], bnd, [len(p)]])
        maxrun = max(maxrun, int(np.diff(edges).max()))
    wov = next((w for w in (16, 32, 64) if maxrun <= w), None)

    use_lng = not (
        all(np.all(np.asarray(inputs[k]) == 1.0)
            for k in ("ln1_g", "ln2_g")) and
        all(np.all(np.asarray(inputs[k]) == 0.0)
            for k in ("ln1_b", "ln2_b")))
    if use_lng:
        raise NotImplementedError(
            "non-identity LN affine not supported in fast path")

    shared = _pack_shared(inputs, lt, nt, use_lng)
    in_maps = []
    for b, start, n in shards:
        mcore = dict(shared)
        mcore.update(_prep_core(inputs, b, start, n, lt, nt, wov))
        in_maps.append(mcore)

    nc = _get_nc(lt, nt, use_lng, wov)
    res = bass_utils.run_bass_kernel_spmd(nc, in_maps,
                                          core_ids=list(range(NCORES)))

    ptl = nt * P
    out = np.zeros((B, S, D), np.float32)
    for i, (b, start, n) in enumerate(shards):
        ht = res.results[i]["houtT"]
        hfull = ht.reshape(P, DC, ptl).transpose(2, 1, 0).reshape(ptl, D)
        out[b, start:start + n] = hfull[:n]
    return out


if __name__ == "__main__":
    import sys
    lt = int(sys.argv[1]) if len(sys.argv) > 1 else 1032
    _get_nc(lt, (lt + P - 1) // P, False)
    print("built ok")
